# revision 9
# baseline (speedup 1.0000x reference)
"""Trainium2 Bass kernel for nn_NodeInfoPropagate (GNN message passing).

Strategy (8 NeuronCores, node-parallel):
  - Shard the 20000 nodes across 8 cores (2500/core, padded to 2560 = 5 tiles
    of 512).  Weights replicated, all matmul operands bf16 (PSUM accumulates
    f32).
  - Activations live on-chip in "transposed" layout [feature-on-partition,
    node-on-free], so every matmul chains with zero transposes.
  - Per layer, the full x table [N, 256] bf16 is materialized in each core's
    HBM via AllGather (Shared scratchpad); parent + neighbor rows are fetched
    with dma_gather(transpose=False).  Gather descriptor generation runs at
    ~8ns/row on ONE Q7 core pair, so gathers are spread over all 4 SWDGE
    queues (4 Q7 pairs -> ~4x descgen).  transpose=True gathers corrupt each
    other when concurrent (shared XBAR spray state), so rows are gathered
    row-major and flipped to [feat, idx] layout with one HWDGE transpose-DMA
    per gather; those are all issued on the Sync queue (cross-engine
    transpose DMAs also corrupt each other).  All other work (reduces, GRU
    element-wise, activations, PSUM copies, table-write transposes) spreads
    over Vector/Scalar/PE so it hides under the gathers.
  - gather commutes with the linear maps: only the x table is gathered;
    summary = x[par] @ Wp.T + mean_k x[nbr_k] @ Wn.T + (b_p + b_n) accumulates
    in one PSUM bank.  Invalid (-1) neighbors point at an all-zero table row.
  - The final output is written directly in transposed layout [128, 2, NCP]
    f32 and unshuffled on the host (no on-chip output transposes).
"""

import sys

sys.path.insert(0, "/opt/trn_rl_repo")

import numpy as np
import ml_dtypes

import concourse.bass as bass
import concourse.bacc as bacc
import concourse.tile as tile
import concourse.mybir as mybir
from concourse import bass_utils

N = 20000
K = 16
H = 256
DIN = 256
NCORES = 8
NC_REAL = N // NCORES          # 2500 real nodes per core
NT = 512                       # node tile (matmul free dim / PSUM bank)
T = 5                          # tiles per core
NCP = NT * T                   # 2560 padded nodes per core
SHARD = 2528                   # table shard rows per core (28 zero pad rows)
ZROW = NC_REAL                 # all-zero table row (core0 pad) for invalid nbrs
NTAB = SHARD * NCORES          # 20224 table rows
NQ = (NT // 4) * K             # 2048 neighbor idxs per quarter-tile

F32 = mybir.dt.float32
BF16 = mybir.dt.bfloat16
I16 = mybir.dt.int16
BF = ml_dtypes.bfloat16

_CACHE = {}


def _build(depth: int):
    nc = bacc.Bacc("TRN2", target_bir_lowering=False, debug=False,
                   num_devices=NCORES, num_swdge_queues=4)

    featT = nc.dram_tensor("featT", [128, 2, NCP], BF16, kind="ExternalInput")
    invcnt = nc.dram_tensor("invcnt", [128, NCP], BF16, kind="ExternalInput")
    nbr_idx = nc.dram_tensor("nbr_idx", [128, T, NT], I16, kind="ExternalInput")
    par_idx = nc.dram_tensor("par_idx", [128, NCP // 16], I16, kind="ExternalInput")
    w_in = nc.dram_tensor("w_in", [128, 2, H], BF16, kind="ExternalInput")
    w_ih = nc.dram_tensor("w_ih", [128, 2, 3 * H], BF16, kind="ExternalInput")
    w_hh = nc.dram_tensor("w_hh", [128, 2, 3 * H], BF16, kind="ExternalInput")
    w_p = nc.dram_tensor("w_p", [128, 2, H], BF16, kind="ExternalInput")
    w_n = nc.dram_tensor("w_n", [128, 2, H], BF16, kind="ExternalInput")
    # bias columns: 0-1 b_in, 2-3 b_p+b_n, 4-5 b_r, 6-7 b_z, 8-9 b_ih_n,
    # 10-11 b_hh_n  (per 128-feature chunk)
    biases = nc.dram_tensor("biases", [128, 12], F32, kind="ExternalInput")
    ident_b = nc.dram_tensor("ident_b", [128, 128], BF16, kind="ExternalInput")
    y = nc.dram_tensor("y", [128, 2, NCP], F32, kind="ExternalOutput")

    SIG = mybir.ActivationFunctionType.Sigmoid
    TANH = mybir.ActivationFunctionType.Tanh
    COPY = mybir.ActivationFunctionType.Copy
    ADD = mybir.AluOpType.add
    MULT = mybir.AluOpType.mult

    with tile.TileContext(nc) as tc:
        with (
            tc.tile_pool(name="const", bufs=1) as constp,
            tc.tile_pool(name="state", bufs=1) as statep,
            tc.tile_pool(name="dram", bufs=1, space="DRAM") as dramp,
            tc.tile_pool(name="pgat", bufs=2) as pgatp,
            tc.tile_pool(name="prm", bufs=1) as prmp,
            tc.tile_pool(name="gath", bufs=4) as gathp,
            tc.tile_pool(name="tt", bufs=3) as ttp,
            tc.tile_pool(name="work", bufs=2) as workp,
            tc.tile_pool(name="tmp", bufs=2) as tmpp,
            tc.tile_pool(name="yout", bufs=2) as youtp,
            tc.tile_pool(name="ps", bufs=2, space="PSUM") as psp,
            tc.tile_pool(name="psg", bufs=6, space="PSUM") as psgp,
        ):
            # ---- resident constants -------------------------------------
            win_sb = constp.tile([128, 2, H], BF16, name="win_sb")
            nc.sync.dma_start(win_sb[:], w_in.ap())
            wih_sb = constp.tile([128, 2, 3 * H], BF16, name="wih_sb")
            nc.sync.dma_start(wih_sb[:], w_ih.ap())
            whh_sb = constp.tile([128, 2, 3 * H], BF16, name="whh_sb")
            nc.sync.dma_start(whh_sb[:], w_hh.ap())
            wp_sb = constp.tile([128, 2, H], BF16, name="wp_sb")
            nc.sync.dma_start(wp_sb[:], w_p.ap())
            wn_sb = constp.tile([128, 2, H], BF16, name="wn_sb")
            nc.sync.dma_start(wn_sb[:], w_n.ap())
            bias_sb = constp.tile([128, 12], F32, name="bias_sb")
            nc.sync.dma_start(bias_sb[:], biases.ap())
            idb_sb = constp.tile([128, 128], BF16, name="idb_sb")
            nc.sync.dma_start(idb_sb[:], ident_b.ap())
            feat_sb = constp.tile([128, 2, NCP], BF16, name="feat_sb")
            nc.sync.dma_start(feat_sb[:], featT.ap())
            inv_sb = constp.tile([128, NCP], BF16, name="inv_sb")
            nc.sync.dma_start(inv_sb[:], invcnt.ap())
            nbr_sb = constp.tile([128, T, NT], I16, name="nbr_sb")
            nc.sync.dma_start(nbr_sb[:], nbr_idx.ap())
            par_sb = constp.tile([128, NCP // 16], I16, name="par_sb")
            nc.sync.dma_start(par_sb[:], par_idx.ap())

            xF = [statep.tile([128, 2, NCP], BF16, name=f"xF{i}") for i in range(2)]
            qctr = [0]  # SWDGE queue round-robin across all gathers

            def next_q():
                q = qctr[0] % 4
                qctr[0] += 1
                return q

            xloc = dramp.tile([SHARD, H], BF16, name="xloc")
            ntabs = max(depth, 1)
            xtabs = [dramp.tile([NTAB, H], BF16, name=f"xtab{i}",
                                addr_space="Shared") for i in range(ntabs)]

            # zero rows (shard pad; serve as invalid-neighbor targets)
            zero_sb = constp.tile([128, H], BF16, name="zero_sb")
            nc.vector.memset(zero_sb[:], 0.0)

            def write_table_tile(xf, t):
                """transpose tile t of xf (bf16) to row-major, one batched DMA
                to xloc (plus a partial-block DMA on the last tile)."""
                ts0 = t * NT
                nb = NT // 128
                rm = workp.tile([128, nb, H], BF16, tag="rm", name="rm")
                nfull = min(NT, NC_REAL - ts0) // 128      # full 128-row blocks
                nblk = nb if ts0 + NT <= NC_REAL else nfull + 1
                for b in range(nblk):
                    for c in range(2):
                        pst = psp.tile([128, 128], BF16, tag="sum", name="pst")
                        nc.tensor.transpose(
                            pst[:], xf[:, c, ts0 + b * 128:ts0 + (b + 1) * 128],
                            idb_sb[:])
                        nc.scalar.activation(rm[:, b, c * 128:(c + 1) * 128],
                                             pst[:], COPY)
                if nfull > 0:
                    nc.sync.dma_start(
                        xloc[ts0:ts0 + nfull * 128, :].rearrange(
                            "(b p) f -> p b f", p=128),
                        rm[:, 0:nfull, :])
                rem = min(NT, NC_REAL - ts0) - nfull * 128  # partial tail rows
                if rem > 0:
                    r0 = ts0 + nfull * 128
                    nc.sync.dma_start(xloc[r0:r0 + rem, :],
                                      rm[0:rem, nfull, :])

            def zero_pad_rows():
                nc.sync.dma_start(xloc[NC_REAL:SHARD, :],
                                  zero_sb[0:SHARD - NC_REAL, :])

            def allgather(xtab):
                nc.gpsimd.collective_compute(
                    "AllGather", mybir.AluOpType.bypass,
                    replica_groups=[list(range(NCORES))],
                    ins=[xloc[0:SHARD, :].opt()],
                    outs=[xtab[0:NTAB, :].opt()],
                )

            # ---- layer 0: x0 = W_in @ feat + b_in ------------------------
            for t in range(T):
                ts = slice(t * NT, (t + 1) * NT)
                for oc in range(2):
                    ps = psp.tile([128, NT], F32, tag="sum", name="ps0")
                    for dc in range(2):
                        nc.tensor.matmul(ps[:], win_sb[:, dc, oc * 128:(oc + 1) * 128],
                                         feat_sb[:, dc, ts],
                                         start=(dc == 0), stop=(dc == 1))
                    if depth == 0:
                        yt = youtp.tile([128, NT], F32, tag="y", name="yt0")
                        nc.vector.tensor_scalar_add(yt[:], ps[:],
                                                    bias_sb[:, oc:oc + 1])
                        nc.sync.dma_start(y.ap()[:, oc, ts], yt[:])
                    else:
                        nc.vector.tensor_scalar_add(xF[0][:, oc, ts], ps[:],
                                                    bias_sb[:, oc:oc + 1])
                if depth > 0:
                    write_table_tile(xF[0], t)
            if depth > 0:
                zero_pad_rows()
                allgather(xtabs[0])

            # ---- GRU layers ---------------------------------------------
            cur = 0
            for layer in range(depth):
                last = layer == depth - 1
                xf_in, xf_out = xF[cur], xF[1 - cur]
                xtab = xtabs[layer]
                # one merged parent gather for the whole layer (row-major,
                # then HWDGE transpose-DMA + re-layout copy)
                prm = prmp.tile([128, NCP // 128, H], BF16, tag="prm", name="prm")
                nc.gpsimd.dma_gather(prm[:], xtab[:], par_sb[:],
                                     NCP, NCP, H, transpose=False,
                                     single_packet=False, queue_num=next_q())
                ptt = prmp.tile([128, NCP // 128, 2, 128], BF16, tag="ptt",
                                name="ptt")
                half = NCP // 256
                for hb in range(2):
                    nc.sync.dma_start_transpose(
                        ptt[:, hb * half:(hb + 1) * half, :, :],
                        prm[:, hb * half:(hb + 1) * half, :])
                pgat = pgatp.tile([128, 2, NCP], BF16, tag="pgat", name="pgat")
                for hc in range(2):
                    nc.vector.tensor_copy(
                        pgat[:, hc, :].rearrange("p (c j) -> p c j", j=128),
                        ptt[:, :, hc, :])
                def gru_block(c0, w, nsum):
                    """summary + GRU for columns [c0, c0+w); nsum covers the
                    owning tile, sliced locally."""
                    bs = slice(c0, c0 + w)
                    t0 = c0 - (c0 % NT)
                    ls = slice(c0 - t0, c0 - t0 + w)
                    nmean = workp.tile([128, 2, NT], BF16, tag="nmean",
                                       name="nmean")
                    for c in range(2):
                        nc.vector.tensor_mul(nmean[:, c, ls], nsum[:, c, ls],
                                             inv_sb[:, bs])
                    # summary = pgat @ Wp.T + nmean @ Wn.T + (b_p + b_n)
                    sT = workp.tile([128, 2, NT], BF16, tag="sT", name="sT")
                    for oc in range(2):
                        ps = psp.tile([128, NT], F32, tag="sum", name="psS")
                        for hc in range(2):
                            nc.tensor.matmul(ps[:, 0:w],
                                             wp_sb[:, hc, oc * 128:(oc + 1) * 128],
                                             pgat[:, hc, bs],
                                             start=(hc == 0), stop=False)
                        for hc in range(2):
                            nc.tensor.matmul(ps[:, 0:w],
                                             wn_sb[:, hc, oc * 128:(oc + 1) * 128],
                                             nmean[:, hc, ls],
                                             start=False, stop=(hc == 1))
                        nc.vector.tensor_scalar_add(sT[:, oc, ls], ps[:, 0:w],
                                                    bias_sb[:, 2 + oc:3 + oc])
                    # GRU gates, per output chunk
                    for oc in range(2):
                        rp = psgp.tile([128, NT], F32, tag="gate", name="rp")
                        zp = psgp.tile([128, NT], F32, tag="gate", name="zp")
                        ip = psgp.tile([128, NT], F32, tag="gate", name="ip")
                        hp = psgp.tile([128, NT], F32, tag="gate", name="hp")
                        for gate, pst in ((0, rp), (1, zp)):
                            o0 = gate * H + oc * 128
                            for hc in range(2):
                                nc.tensor.matmul(pst[:, 0:w],
                                                 wih_sb[:, hc, o0:o0 + 128],
                                                 xf_in[:, hc, bs],
                                                 start=(hc == 0), stop=False)
                            for hc in range(2):
                                nc.tensor.matmul(pst[:, 0:w],
                                                 whh_sb[:, hc, o0:o0 + 128],
                                                 sT[:, hc, ls],
                                                 start=False, stop=(hc == 1))
                        o0 = 2 * H + oc * 128
                        for hc in range(2):
                            nc.tensor.matmul(ip[:, 0:w], wih_sb[:, hc, o0:o0 + 128],
                                             xf_in[:, hc, bs],
                                             start=(hc == 0), stop=(hc == 1))
                        for hc in range(2):
                            nc.tensor.matmul(hp[:, 0:w], whh_sb[:, hc, o0:o0 + 128],
                                             sT[:, hc, ls],
                                             start=(hc == 0), stop=(hc == 1))
                        r = tmpp.tile([128, NT], F32, tag="r", name="r")
                        nc.scalar.activation(r[:, 0:w], rp[:, 0:w], SIG,
                                             bias=bias_sb[:, 4 + oc:5 + oc])
                        z = tmpp.tile([128, NT], F32, tag="z", name="z")
                        nc.scalar.activation(z[:, 0:w], zp[:, 0:w], SIG,
                                             bias=bias_sb[:, 6 + oc:7 + oc])
                        # n = tanh((i_n + b_ih_n) + r * (h_n + b_hh_n))
                        hnr = tmpp.tile([128, NT], F32, tag="hnr", name="hnr")
                        nc.vector.scalar_tensor_tensor(
                            hnr[:, 0:w], hp[:, 0:w], bias_sb[:, 10 + oc:11 + oc],
                            r[:, 0:w], op0=ADD, op1=MULT)
                        npre = tmpp.tile([128, NT], F32, tag="r", name="npre")
                        nc.vector.scalar_tensor_tensor(
                            npre[:, 0:w], ip[:, 0:w], bias_sb[:, 8 + oc:9 + oc],
                            hnr[:, 0:w], op0=ADD, op1=ADD)
                        nt_ = tmpp.tile([128, NT], F32, tag="nt", name="nt")
                        nc.scalar.activation(nt_[:, 0:w], npre[:, 0:w], TANH)
                        # x_new = n + z * (summary - n)
                        d = tmpp.tile([128, NT], F32, tag="d", name="d")
                        nc.vector.tensor_sub(d[:, 0:w], sT[:, oc, ls], nt_[:, 0:w])
                        dz = tmpp.tile([128, NT], F32, tag="d", name="dz")
                        nc.vector.tensor_mul(dz[:, 0:w], d[:, 0:w], z[:, 0:w])
                        if last:
                            yt = youtp.tile([128, NT], F32, tag="y", name="yt")
                            nc.vector.tensor_add(yt[:, 0:w], dz[:, 0:w], nt_[:, 0:w])
                            nc.sync.dma_start(y.ap()[:, oc, bs], yt[:, 0:w])
                        else:
                            nc.vector.tensor_add(xf_out[:, oc, bs], dz[:, 0:w],
                                                 nt_[:, 0:w])

                for t in range(T):
                    ts0 = t * NT
                    # neighbor rows, four quarter-tiles (2048-desc gathers
                    # chain with no retire stall); sum groups of K=16
                    nsum = workp.tile([128, 2, NT], F32, tag="nsum", name="nsum")
                    for qf in range(4):
                        q0 = qf * (NT // 4)
                        # last quarter of the last tile: only 68 real nodes
                        nreal = min(NT // 4, max(NC_REAL - ts0 - q0, 0))
                        nq = ((nreal * K + 127) // 128) * 128
                        if nq == 0:
                            continue
                        nch = nq // 128
                        nrm = gathp.tile([128, 16, H], BF16, tag="nrm",
                                         name="nrm")
                        nc.gpsimd.dma_gather(
                            nrm[:, 0:nch, :], xtab[:],
                            nbr_sb[:, t, q0:q0 + nq // K],
                            nq, nq, H, transpose=False, single_packet=False,
                            queue_num=next_q())
                        ntt = ttp.tile([128, 16, 2, 128], BF16, tag="ntt",
                                       name="ntt")
                        nc.sync.dma_start_transpose(ntt[:, 0:nch, :, :],
                                                    nrm[:, 0:nch, :])
                        for c in range(2):
                            nc.vector.tensor_reduce(
                                nsum[:, c, q0:q0 + nq // K],
                                ntt[:, 0:nch, c, :].rearrange(
                                    "p n (g k) -> p n g k", k=K),
                                axis=mybir.AxisListType.X, op=ADD)
                        if t == T - 1 and qf == 1:
                            gru_block(ts0, NT // 2, nsum)
                    if t == T - 1:
                        gru_block(ts0 + NT // 2, NT // 2, nsum)
                    else:
                        gru_block(ts0, NT, nsum)
                    if not last:
                        write_table_tile(xf_out, t)
                if not last:
                    allgather(xtabs[layer + 1])
                cur = 1 - cur

    nc.compile()
    return nc


def _get_nc(depth: int):
    if depth not in _CACHE:
        _CACHE[depth] = _build(depth)
    return _CACHE[depth]


def _idx_layout(lin):
    """linear int16 idx list (len % 16 == 0) -> [128, len//16] wrapped in 16
    partitions, replicated across the 8 gpsimd core groups."""
    v = lin.reshape(-1, 16).T.astype(np.int16)        # [16, len//16]
    return np.tile(v, (8, 1))                         # [128, len//16]


def _chunk2(w):
    """[256, M] -> [128, 2, M] with [p, c, m] = w[c*128+p, m]."""
    M = w.shape[1]
    return np.ascontiguousarray(w.reshape(2, 128, M).transpose(1, 0, 2))


def prepare_inputs(inputs):
    """host-side preprocessing: returns in_maps for the 8 cores."""
    adj = np.asarray(inputs["nodeAdjacencySpecTensor"]).astype(np.int64)
    names = np.asarray(inputs["nodeNamesEncoded"], dtype=np.float32)
    attrs = np.asarray(inputs["nodeAttributesEncoded"], dtype=np.float32)

    parent = adj[:, 0]
    parent = np.clip(np.where(parent < 0, parent + N, parent), 0, N - 1)
    nbr = adj[:, 1:]
    mask = nbr >= 0
    cnt = np.maximum(mask.sum(1), 1).astype(np.float32)
    safe_n = np.where(mask, np.clip(nbr, 0, N - 1), 0).astype(np.int64)
    # node id -> table row in the [8 x SHARD] allgathered layout
    def _row(n):
        return SHARD * (n // NC_REAL) + (n % NC_REAL)
    safe = np.where(mask, _row(safe_n), ZROW).astype(np.int64)
    parent = _row(parent)
    inv = (1.0 / cnt).astype(np.float32)

    feat = np.concatenate([names, attrs], axis=1)      # [N, 256] f32

    W_in = np.asarray(inputs["W_in"], np.float32)
    W_p = np.asarray(inputs["W_parent"], np.float32)
    W_n = np.asarray(inputs["W_neighbor"], np.float32)
    W_ih = np.asarray(inputs["W_ih"], np.float32)
    W_hh = np.asarray(inputs["W_hh"], np.float32)
    b_in = np.asarray(inputs["b_in"], np.float32)
    b_p = np.asarray(inputs["b_parent"], np.float32)
    b_n = np.asarray(inputs["b_neighbor"], np.float32)
    b_ih = np.asarray(inputs["b_ih"], np.float32)
    b_hh = np.asarray(inputs["b_hh"], np.float32)

    w_in_a = _chunk2(W_in.T).astype(BF)                 # [128, 2, 256]
    w_ih_a = _chunk2(W_ih.T).astype(BF)                 # [128, 2, 768]
    w_hh_a = _chunk2(W_hh.T).astype(BF)
    w_p_a = _chunk2(W_p.T).astype(BF)
    w_n_a = _chunk2(W_n.T).astype(BF)

    bias = np.zeros((128, 12), np.float32)
    for col, vec in ((0, b_in), (2, b_p + b_n), (4, (b_ih + b_hh)[0:H]),
                     (6, (b_ih + b_hh)[H:2 * H]), (8, b_ih[2 * H:3 * H]),
                     (10, b_hh[2 * H:3 * H])):
        bias[:, col] = vec[0:128]
        bias[:, col + 1] = vec[128:256]

    ident_b = np.eye(128, dtype=BF)

    shared = dict(w_in=w_in_a, w_ih=w_ih_a, w_hh=w_hh_a, w_p=w_p_a, w_n=w_n_a,
                  biases=bias, ident_b=ident_b)

    in_maps = []
    for c in range(NCORES):
        g0 = c * NC_REAL
        # features, transposed + padded
        f = np.zeros((NCP, DIN), np.float32)
        f[:NC_REAL] = feat[g0:g0 + NC_REAL]
        featT_c = np.ascontiguousarray(
            f.T.reshape(2, 128, NCP).transpose(1, 0, 2)).astype(BF)
        # inv count broadcast
        iv = np.ones(NCP, np.float32)
        iv[:NC_REAL] = inv[g0:g0 + NC_REAL]
        inv_c = np.broadcast_to(iv.astype(BF), (128, NCP)).copy()
        # indices
        par = np.full(NCP, ZROW, np.int64)
        par[:NC_REAL] = parent[g0:g0 + NC_REAL]
        nbrs = np.full((NCP, K), ZROW, np.int64)
        nbrs[:NC_REAL] = safe[g0:g0 + NC_REAL]
        nbr_t = np.zeros((128, T, NT), np.int16)
        for t in range(T):
            nbr_t[:, t, :] = _idx_layout(nbrs[t * NT:(t + 1) * NT].reshape(-1))
        par_t = _idx_layout(par)                        # [128, 160]
        in_maps.append(dict(featT=featT_c, invcnt=inv_c, nbr_idx=nbr_t,
                            par_idx=par_t, **shared))
    return in_maps


def run(inputs, trace=False, **kw):
    depth = int(np.asarray(inputs["depth"]))
    nc = _get_nc(depth)
    in_maps = prepare_inputs(inputs)
    res = bass_utils.run_bass_kernel_spmd(nc, in_maps,
                                          core_ids=list(range(NCORES)),
                                          trace=trace, **kw)
    outs = []
    for c in range(NCORES):
        yc = np.asarray(res.results[c]["y"])            # [128, 2, NCP]
        xc = yc.transpose(2, 1, 0).reshape(NCP, H)      # [NCP, 256]
        outs.append(xc[:NC_REAL])
    return np.ascontiguousarray(
        np.concatenate(outs, axis=0).astype(np.float32)), res


def kernel(**inputs) -> np.ndarray:
    out, _ = run(inputs, trace=False)
    return out



# revision 10
# speedup vs baseline: 1.0016x; 1.0016x over previous
"""Trainium2 Bass kernel for nn_NodeInfoPropagate (GNN message passing).

Strategy (8 NeuronCores, node-parallel):
  - Shard the 20000 nodes across 8 cores (2500/core, padded to 2560 = 5 tiles
    of 512).  Weights replicated, all matmul operands bf16 (PSUM accumulates
    f32).
  - Activations live on-chip in "transposed" layout [feature-on-partition,
    node-on-free], so every matmul chains with zero transposes.
  - Per layer, the full x table [N, 256] bf16 is materialized in each core's
    HBM via AllGather (Shared scratchpad); parent + neighbor rows are fetched
    with dma_gather(transpose=False).  Gather descriptor generation runs at
    ~8ns/row on ONE Q7 core pair, so gathers are spread over all 4 SWDGE
    queues (4 Q7 pairs -> ~4x descgen).  transpose=True gathers corrupt each
    other when concurrent (shared XBAR spray state), so rows are gathered
    row-major and flipped to [feat, idx] layout with one HWDGE transpose-DMA
    per gather; those are all issued on the Sync queue (cross-engine
    transpose DMAs also corrupt each other).  All other work (reduces, GRU
    element-wise, activations, PSUM copies, table-write transposes) spreads
    over Vector/Scalar/PE so it hides under the gathers.
  - gather commutes with the linear maps: only the x table is gathered;
    summary = x[par] @ Wp.T + mean_k x[nbr_k] @ Wn.T + (b_p + b_n) accumulates
    in one PSUM bank.  Invalid (-1) neighbors point at an all-zero table row.
  - The final output is written directly in transposed layout [128, 2, NCP]
    f32 and unshuffled on the host (no on-chip output transposes).
"""

import os
import sys

# The tile framework's elide-DMA-wait pass coarsens DMA waits to the
# FIFO-max entry of the issuing ring, scheduled against CoreSim timings.
# On hardware (where SWDGE gathers run ~10x slower than modeled) those
# coarsened waits serialize the gather pipeline — keep precise waits.
os.environ["BACC_ELIDE_DMA_OPT_LIMIT"] = "0"

sys.path.insert(0, "/opt/trn_rl_repo")

import numpy as np
import ml_dtypes

import concourse.bass as bass
import concourse.bacc as bacc
import concourse.tile as tile
import concourse.mybir as mybir
from concourse import bass_utils

N = 20000
K = 16
H = 256
DIN = 256
NCORES = 8
NC_REAL = N // NCORES          # 2500 real nodes per core
NT = 512                       # node tile (matmul free dim / PSUM bank)
T = 5                          # tiles per core
NCP = NT * T                   # 2560 padded nodes per core
SHARD = 2528                   # table shard rows per core (28 zero pad rows)
ZROW = NC_REAL                 # all-zero table row (core0 pad) for invalid nbrs
NTAB = SHARD * NCORES          # 20224 table rows
NQ = (NT // 4) * K             # 2048 neighbor idxs per quarter-tile

F32 = mybir.dt.float32
BF16 = mybir.dt.bfloat16
I16 = mybir.dt.int16
BF = ml_dtypes.bfloat16

_CACHE = {}


def _build(depth: int):
    nc = bacc.Bacc("TRN2", target_bir_lowering=False, debug=False,
                   num_devices=NCORES, num_swdge_queues=4)

    featT = nc.dram_tensor("featT", [128, 2, NCP], BF16, kind="ExternalInput")
    invcnt = nc.dram_tensor("invcnt", [128, NCP], BF16, kind="ExternalInput")
    nbr_idx = nc.dram_tensor("nbr_idx", [128, T, NT], I16, kind="ExternalInput")
    par_idx = nc.dram_tensor("par_idx", [128, NCP // 16], I16, kind="ExternalInput")
    w_in = nc.dram_tensor("w_in", [128, 2, H], BF16, kind="ExternalInput")
    w_ih = nc.dram_tensor("w_ih", [128, 2, 3 * H], BF16, kind="ExternalInput")
    w_hh = nc.dram_tensor("w_hh", [128, 2, 3 * H], BF16, kind="ExternalInput")
    w_p = nc.dram_tensor("w_p", [128, 2, H], BF16, kind="ExternalInput")
    w_n = nc.dram_tensor("w_n", [128, 2, H], BF16, kind="ExternalInput")
    # bias columns: 0-1 b_in, 2-3 b_p+b_n, 4-5 b_r, 6-7 b_z, 8-9 b_ih_n,
    # 10-11 b_hh_n  (per 128-feature chunk)
    biases = nc.dram_tensor("biases", [128, 12], F32, kind="ExternalInput")
    ident_b = nc.dram_tensor("ident_b", [128, 128], BF16, kind="ExternalInput")
    y = nc.dram_tensor("y", [128, 2, NCP], F32, kind="ExternalOutput")

    SIG = mybir.ActivationFunctionType.Sigmoid
    TANH = mybir.ActivationFunctionType.Tanh
    COPY = mybir.ActivationFunctionType.Copy
    ADD = mybir.AluOpType.add
    MULT = mybir.AluOpType.mult

    with tile.TileContext(nc) as tc:
        with (
            tc.tile_pool(name="const", bufs=1) as constp,
            tc.tile_pool(name="state", bufs=1) as statep,
            tc.tile_pool(name="dram", bufs=1, space="DRAM") as dramp,
            tc.tile_pool(name="pgat", bufs=2) as pgatp,
            tc.tile_pool(name="prm", bufs=1) as prmp,
            tc.tile_pool(name="gath", bufs=4) as gathp,
            tc.tile_pool(name="tt", bufs=3) as ttp,
            tc.tile_pool(name="work", bufs=2) as workp,
            tc.tile_pool(name="tmp", bufs=2) as tmpp,
            tc.tile_pool(name="yout", bufs=2) as youtp,
            tc.tile_pool(name="ps", bufs=2, space="PSUM") as psp,
            tc.tile_pool(name="psg", bufs=6, space="PSUM") as psgp,
        ):
            # ---- resident constants -------------------------------------
            win_sb = constp.tile([128, 2, H], BF16, name="win_sb")
            nc.sync.dma_start(win_sb[:], w_in.ap())
            wih_sb = constp.tile([128, 2, 3 * H], BF16, name="wih_sb")
            nc.sync.dma_start(wih_sb[:], w_ih.ap())
            whh_sb = constp.tile([128, 2, 3 * H], BF16, name="whh_sb")
            nc.sync.dma_start(whh_sb[:], w_hh.ap())
            wp_sb = constp.tile([128, 2, H], BF16, name="wp_sb")
            nc.sync.dma_start(wp_sb[:], w_p.ap())
            wn_sb = constp.tile([128, 2, H], BF16, name="wn_sb")
            nc.sync.dma_start(wn_sb[:], w_n.ap())
            bias_sb = constp.tile([128, 12], F32, name="bias_sb")
            nc.sync.dma_start(bias_sb[:], biases.ap())
            idb_sb = constp.tile([128, 128], BF16, name="idb_sb")
            nc.sync.dma_start(idb_sb[:], ident_b.ap())
            feat_sb = constp.tile([128, 2, NCP], BF16, name="feat_sb")
            nc.sync.dma_start(feat_sb[:], featT.ap())
            inv_sb = constp.tile([128, NCP], BF16, name="inv_sb")
            nc.sync.dma_start(inv_sb[:], invcnt.ap())
            nbr_sb = constp.tile([128, T, NT], I16, name="nbr_sb")
            nc.sync.dma_start(nbr_sb[:], nbr_idx.ap())
            par_sb = constp.tile([128, NCP // 16], I16, name="par_sb")
            nc.sync.dma_start(par_sb[:], par_idx.ap())

            xF = [statep.tile([128, 2, NCP], BF16, name=f"xF{i}") for i in range(2)]
            qctr = [0]  # SWDGE queue round-robin across all gathers

            def next_q():
                q = qctr[0] % 4
                qctr[0] += 1
                return q

            xloc = dramp.tile([SHARD, H], BF16, name="xloc")
            ntabs = max(depth, 1)
            xtabs = [dramp.tile([NTAB, H], BF16, name=f"xtab{i}",
                                addr_space="Shared") for i in range(ntabs)]

            # zero rows (shard pad; serve as invalid-neighbor targets)
            zero_sb = constp.tile([128, H], BF16, name="zero_sb")
            nc.vector.memset(zero_sb[:], 0.0)

            def write_table_tile(xf, t):
                """transpose tile t of xf (bf16) to row-major, one batched DMA
                to xloc (plus a partial-block DMA on the last tile)."""
                ts0 = t * NT
                nb = NT // 128
                rm = workp.tile([128, nb, H], BF16, tag="rm", name="rm")
                nfull = min(NT, NC_REAL - ts0) // 128      # full 128-row blocks
                nblk = nb if ts0 + NT <= NC_REAL else nfull + 1
                for b in range(nblk):
                    for c in range(2):
                        pst = psp.tile([128, 128], BF16, tag="sum", name="pst")
                        nc.tensor.transpose(
                            pst[:], xf[:, c, ts0 + b * 128:ts0 + (b + 1) * 128],
                            idb_sb[:])
                        nc.scalar.activation(rm[:, b, c * 128:(c + 1) * 128],
                                             pst[:], COPY)
                if nfull > 0:
                    nc.sync.dma_start(
                        xloc[ts0:ts0 + nfull * 128, :].rearrange(
                            "(b p) f -> p b f", p=128),
                        rm[:, 0:nfull, :])
                rem = min(NT, NC_REAL - ts0) - nfull * 128  # partial tail rows
                if rem > 0:
                    r0 = ts0 + nfull * 128
                    nc.sync.dma_start(xloc[r0:r0 + rem, :],
                                      rm[0:rem, nfull, :])

            def zero_pad_rows():
                nc.sync.dma_start(xloc[NC_REAL:SHARD, :],
                                  zero_sb[0:SHARD - NC_REAL, :])

            def allgather(xtab):
                nc.gpsimd.collective_compute(
                    "AllGather", mybir.AluOpType.bypass,
                    replica_groups=[list(range(NCORES))],
                    ins=[xloc[0:SHARD, :].opt()],
                    outs=[xtab[0:NTAB, :].opt()],
                )

            # ---- layer 0: x0 = W_in @ feat + b_in ------------------------
            for t in range(T):
                ts = slice(t * NT, (t + 1) * NT)
                for oc in range(2):
                    ps = psp.tile([128, NT], F32, tag="sum", name="ps0")
                    for dc in range(2):
                        nc.tensor.matmul(ps[:], win_sb[:, dc, oc * 128:(oc + 1) * 128],
                                         feat_sb[:, dc, ts],
                                         start=(dc == 0), stop=(dc == 1))
                    if depth == 0:
                        yt = youtp.tile([128, NT], F32, tag="y", name="yt0")
                        nc.vector.tensor_scalar_add(yt[:], ps[:],
                                                    bias_sb[:, oc:oc + 1])
                        nc.sync.dma_start(y.ap()[:, oc, ts], yt[:])
                    else:
                        nc.vector.tensor_scalar_add(xF[0][:, oc, ts], ps[:],
                                                    bias_sb[:, oc:oc + 1])
                if depth > 0:
                    write_table_tile(xF[0], t)
            if depth > 0:
                zero_pad_rows()
                allgather(xtabs[0])

            # ---- GRU layers ---------------------------------------------
            cur = 0
            for layer in range(depth):
                last = layer == depth - 1
                xf_in, xf_out = xF[cur], xF[1 - cur]
                xtab = xtabs[layer]
                # one merged parent gather for the whole layer (row-major,
                # then HWDGE transpose-DMA + re-layout copy)
                prm = prmp.tile([128, NCP // 128, H], BF16, tag="prm", name="prm")
                nc.gpsimd.dma_gather(prm[:], xtab[:], par_sb[:],
                                     NCP, NCP, H, transpose=False,
                                     single_packet=False, queue_num=next_q())
                ptt = prmp.tile([128, NCP // 128, 2, 128], BF16, tag="ptt",
                                name="ptt")
                half = NCP // 256
                for hb in range(2):
                    nc.sync.dma_start_transpose(
                        ptt[:, hb * half:(hb + 1) * half, :, :],
                        prm[:, hb * half:(hb + 1) * half, :])
                pgat = pgatp.tile([128, 2, NCP], BF16, tag="pgat", name="pgat")
                for hc in range(2):
                    nc.vector.tensor_copy(
                        pgat[:, hc, :].rearrange("p (c j) -> p c j", j=128),
                        ptt[:, :, hc, :])
                def gru_block(c0, w, nsum):
                    """summary + GRU for columns [c0, c0+w); nsum covers the
                    owning tile, sliced locally."""
                    bs = slice(c0, c0 + w)
                    t0 = c0 - (c0 % NT)
                    ls = slice(c0 - t0, c0 - t0 + w)
                    nmean = workp.tile([128, 2, NT], BF16, tag="nmean",
                                       name="nmean")
                    for c in range(2):
                        nc.vector.tensor_mul(nmean[:, c, ls], nsum[:, c, ls],
                                             inv_sb[:, bs])
                    # summary = pgat @ Wp.T + nmean @ Wn.T + (b_p + b_n)
                    sT = workp.tile([128, 2, NT], BF16, tag="sT", name="sT")
                    for oc in range(2):
                        ps = psp.tile([128, NT], F32, tag="sum", name="psS")
                        for hc in range(2):
                            nc.tensor.matmul(ps[:, 0:w],
                                             wp_sb[:, hc, oc * 128:(oc + 1) * 128],
                                             pgat[:, hc, bs],
                                             start=(hc == 0), stop=False)
                        for hc in range(2):
                            nc.tensor.matmul(ps[:, 0:w],
                                             wn_sb[:, hc, oc * 128:(oc + 1) * 128],
                                             nmean[:, hc, ls],
                                             start=False, stop=(hc == 1))
                        nc.vector.tensor_scalar_add(sT[:, oc, ls], ps[:, 0:w],
                                                    bias_sb[:, 2 + oc:3 + oc])
                    # GRU gates, per output chunk
                    for oc in range(2):
                        rp = psgp.tile([128, NT], F32, tag="gate", name="rp")
                        zp = psgp.tile([128, NT], F32, tag="gate", name="zp")
                        ip = psgp.tile([128, NT], F32, tag="gate", name="ip")
                        hp = psgp.tile([128, NT], F32, tag="gate", name="hp")
                        for gate, pst in ((0, rp), (1, zp)):
                            o0 = gate * H + oc * 128
                            for hc in range(2):
                                nc.tensor.matmul(pst[:, 0:w],
                                                 wih_sb[:, hc, o0:o0 + 128],
                                                 xf_in[:, hc, bs],
                                                 start=(hc == 0), stop=False)
                            for hc in range(2):
                                nc.tensor.matmul(pst[:, 0:w],
                                                 whh_sb[:, hc, o0:o0 + 128],
                                                 sT[:, hc, ls],
                                                 start=False, stop=(hc == 1))
                        o0 = 2 * H + oc * 128
                        for hc in range(2):
                            nc.tensor.matmul(ip[:, 0:w], wih_sb[:, hc, o0:o0 + 128],
                                             xf_in[:, hc, bs],
                                             start=(hc == 0), stop=(hc == 1))
                        for hc in range(2):
                            nc.tensor.matmul(hp[:, 0:w], whh_sb[:, hc, o0:o0 + 128],
                                             sT[:, hc, ls],
                                             start=(hc == 0), stop=(hc == 1))
                        r = tmpp.tile([128, NT], F32, tag="r", name="r")
                        nc.scalar.activation(r[:, 0:w], rp[:, 0:w], SIG,
                                             bias=bias_sb[:, 4 + oc:5 + oc])
                        z = tmpp.tile([128, NT], F32, tag="z", name="z")
                        nc.scalar.activation(z[:, 0:w], zp[:, 0:w], SIG,
                                             bias=bias_sb[:, 6 + oc:7 + oc])
                        # n = tanh((i_n + b_ih_n) + r * (h_n + b_hh_n))
                        hnr = tmpp.tile([128, NT], F32, tag="hnr", name="hnr")
                        nc.vector.scalar_tensor_tensor(
                            hnr[:, 0:w], hp[:, 0:w], bias_sb[:, 10 + oc:11 + oc],
                            r[:, 0:w], op0=ADD, op1=MULT)
                        npre = tmpp.tile([128, NT], F32, tag="r", name="npre")
                        nc.vector.scalar_tensor_tensor(
                            npre[:, 0:w], ip[:, 0:w], bias_sb[:, 8 + oc:9 + oc],
                            hnr[:, 0:w], op0=ADD, op1=ADD)
                        nt_ = tmpp.tile([128, NT], F32, tag="nt", name="nt")
                        nc.scalar.activation(nt_[:, 0:w], npre[:, 0:w], TANH)
                        # x_new = n + z * (summary - n)
                        d = tmpp.tile([128, NT], F32, tag="d", name="d")
                        nc.vector.tensor_sub(d[:, 0:w], sT[:, oc, ls], nt_[:, 0:w])
                        dz = tmpp.tile([128, NT], F32, tag="d", name="dz")
                        nc.vector.tensor_mul(dz[:, 0:w], d[:, 0:w], z[:, 0:w])
                        if last:
                            yt = youtp.tile([128, NT], F32, tag="y", name="yt")
                            nc.vector.tensor_add(yt[:, 0:w], dz[:, 0:w], nt_[:, 0:w])
                            nc.sync.dma_start(y.ap()[:, oc, bs], yt[:, 0:w])
                        else:
                            nc.vector.tensor_add(xf_out[:, oc, bs], dz[:, 0:w],
                                                 nt_[:, 0:w])

                for t in range(T):
                    ts0 = t * NT
                    # neighbor rows, four quarter-tiles (2048-desc gathers
                    # chain with no retire stall); sum groups of K=16
                    nsum = workp.tile([128, 2, NT], F32, tag="nsum", name="nsum")
                    for qf in range(4):
                        q0 = qf * (NT // 4)
                        # last quarter of the last tile: only 68 real nodes
                        nreal = min(NT // 4, max(NC_REAL - ts0 - q0, 0))
                        nq = ((nreal * K + 127) // 128) * 128
                        if nq == 0:
                            continue
                        nch = nq // 128
                        nrm = gathp.tile([128, 16, H], BF16, tag="nrm",
                                         name="nrm")
                        nc.gpsimd.dma_gather(
                            nrm[:, 0:nch, :], xtab[:],
                            nbr_sb[:, t, q0:q0 + nq // K],
                            nq, nq, H, transpose=False, single_packet=False,
                            queue_num=next_q())
                        ntt = ttp.tile([128, 16, 2, 128], BF16, tag="ntt",
                                       name="ntt")
                        nc.sync.dma_start_transpose(ntt[:, 0:nch, :, :],
                                                    nrm[:, 0:nch, :])
                        for c in range(2):
                            nc.vector.tensor_reduce(
                                nsum[:, c, q0:q0 + nq // K],
                                ntt[:, 0:nch, c, :].rearrange(
                                    "p n (g k) -> p n g k", k=K),
                                axis=mybir.AxisListType.X, op=ADD)
                        if t == T - 1 and qf == 1:
                            gru_block(ts0, NT // 2, nsum)
                    if t == T - 1:
                        gru_block(ts0 + NT // 2, NT // 2, nsum)
                    else:
                        gru_block(ts0, NT, nsum)
                    if not last:
                        write_table_tile(xf_out, t)
                if not last:
                    allgather(xtabs[layer + 1])
                cur = 1 - cur

    nc.compile()
    return nc


def _get_nc(depth: int):
    if depth not in _CACHE:
        _CACHE[depth] = _build(depth)
    return _CACHE[depth]


def _idx_layout(lin):
    """linear int16 idx list (len % 16 == 0) -> [128, len//16] wrapped in 16
    partitions, replicated across the 8 gpsimd core groups."""
    v = lin.reshape(-1, 16).T.astype(np.int16)        # [16, len//16]
    return np.tile(v, (8, 1))                         # [128, len//16]


def _chunk2(w):
    """[256, M] -> [128, 2, M] with [p, c, m] = w[c*128+p, m]."""
    M = w.shape[1]
    return np.ascontiguousarray(w.reshape(2, 128, M).transpose(1, 0, 2))


def prepare_inputs(inputs):
    """host-side preprocessing: returns in_maps for the 8 cores."""
    adj = np.asarray(inputs["nodeAdjacencySpecTensor"]).astype(np.int64)
    names = np.asarray(inputs["nodeNamesEncoded"], dtype=np.float32)
    attrs = np.asarray(inputs["nodeAttributesEncoded"], dtype=np.float32)

    parent = adj[:, 0]
    parent = np.clip(np.where(parent < 0, parent + N, parent), 0, N - 1)
    nbr = adj[:, 1:]
    mask = nbr >= 0
    cnt = np.maximum(mask.sum(1), 1).astype(np.float32)
    safe_n = np.where(mask, np.clip(nbr, 0, N - 1), 0).astype(np.int64)
    # node id -> table row in the [8 x SHARD] allgathered layout
    def _row(n):
        return SHARD * (n // NC_REAL) + (n % NC_REAL)
    safe = np.where(mask, _row(safe_n), ZROW).astype(np.int64)
    parent = _row(parent)
    inv = (1.0 / cnt).astype(np.float32)

    feat = np.concatenate([names, attrs], axis=1)      # [N, 256] f32

    W_in = np.asarray(inputs["W_in"], np.float32)
    W_p = np.asarray(inputs["W_parent"], np.float32)
    W_n = np.asarray(inputs["W_neighbor"], np.float32)
    W_ih = np.asarray(inputs["W_ih"], np.float32)
    W_hh = np.asarray(inputs["W_hh"], np.float32)
    b_in = np.asarray(inputs["b_in"], np.float32)
    b_p = np.asarray(inputs["b_parent"], np.float32)
    b_n = np.asarray(inputs["b_neighbor"], np.float32)
    b_ih = np.asarray(inputs["b_ih"], np.float32)
    b_hh = np.asarray(inputs["b_hh"], np.float32)

    w_in_a = _chunk2(W_in.T).astype(BF)                 # [128, 2, 256]
    w_ih_a = _chunk2(W_ih.T).astype(BF)                 # [128, 2, 768]
    w_hh_a = _chunk2(W_hh.T).astype(BF)
    w_p_a = _chunk2(W_p.T).astype(BF)
    w_n_a = _chunk2(W_n.T).astype(BF)

    bias = np.zeros((128, 12), np.float32)
    for col, vec in ((0, b_in), (2, b_p + b_n), (4, (b_ih + b_hh)[0:H]),
                     (6, (b_ih + b_hh)[H:2 * H]), (8, b_ih[2 * H:3 * H]),
                     (10, b_hh[2 * H:3 * H])):
        bias[:, col] = vec[0:128]
        bias[:, col + 1] = vec[128:256]

    ident_b = np.eye(128, dtype=BF)

    shared = dict(w_in=w_in_a, w_ih=w_ih_a, w_hh=w_hh_a, w_p=w_p_a, w_n=w_n_a,
                  biases=bias, ident_b=ident_b)

    in_maps = []
    for c in range(NCORES):
        g0 = c * NC_REAL
        # features, transposed + padded
        f = np.zeros((NCP, DIN), np.float32)
        f[:NC_REAL] = feat[g0:g0 + NC_REAL]
        featT_c = np.ascontiguousarray(
            f.T.reshape(2, 128, NCP).transpose(1, 0, 2)).astype(BF)
        # inv count broadcast
        iv = np.ones(NCP, np.float32)
        iv[:NC_REAL] = inv[g0:g0 + NC_REAL]
        inv_c = np.broadcast_to(iv.astype(BF), (128, NCP)).copy()
        # indices
        par = np.full(NCP, ZROW, np.int64)
        par[:NC_REAL] = parent[g0:g0 + NC_REAL]
        nbrs = np.full((NCP, K), ZROW, np.int64)
        nbrs[:NC_REAL] = safe[g0:g0 + NC_REAL]
        nbr_t = np.zeros((128, T, NT), np.int16)
        for t in range(T):
            nbr_t[:, t, :] = _idx_layout(nbrs[t * NT:(t + 1) * NT].reshape(-1))
        par_t = _idx_layout(par)                        # [128, 160]
        in_maps.append(dict(featT=featT_c, invcnt=inv_c, nbr_idx=nbr_t,
                            par_idx=par_t, **shared))
    return in_maps


def run(inputs, trace=False, **kw):
    depth = int(np.asarray(inputs["depth"]))
    nc = _get_nc(depth)
    in_maps = prepare_inputs(inputs)
    res = bass_utils.run_bass_kernel_spmd(nc, in_maps,
                                          core_ids=list(range(NCORES)),
                                          trace=trace, **kw)
    outs = []
    for c in range(NCORES):
        yc = np.asarray(res.results[c]["y"])            # [128, 2, NCP]
        xc = yc.transpose(2, 1, 0).reshape(NCP, H)      # [NCP, 256]
        outs.append(xc[:NC_REAL])
    return np.ascontiguousarray(
        np.concatenate(outs, axis=0).astype(np.float32)), res


def kernel(**inputs) -> np.ndarray:
    out, _ = run(inputs, trace=False)
    return out



# revision 13
# speedup vs baseline: 1.2285x; 1.2265x over previous
"""Trainium2 Bass kernel for nn_NodeInfoPropagate (GNN message passing).

Strategy (8 NeuronCores, node-parallel):
  - Shard the 20000 nodes across 8 cores (2500/core, padded to 2560 = 5 tiles
    of 512).  Weights replicated, all matmul operands bf16 (PSUM accumulates
    f32).
  - Activations live on-chip in "transposed" layout [feature-on-partition,
    node-on-free], so every matmul chains with zero transposes.
  - Per layer, the full x table [N, 256] bf16 is materialized in each core's
    HBM via AllGather (Shared scratchpad); parent + neighbor rows are fetched
    with dma_gather(transpose=False).  Gather descriptor generation runs at
    ~8ns/row on ONE Q7 core pair, so gathers are spread over all 4 SWDGE
    queues (4 Q7 pairs -> ~4x descgen).  transpose=True gathers corrupt each
    other when concurrent (shared XBAR spray state), so rows are gathered
    row-major and flipped to [feat, idx] layout with one HWDGE transpose-DMA
    per gather; those are all issued on the Sync queue (cross-engine
    transpose DMAs also corrupt each other).  All other work (reduces, GRU
    element-wise, activations, PSUM copies, table-write transposes) spreads
    over Vector/Scalar/PE so it hides under the gathers.
  - gather commutes with the linear maps: only the x table is gathered;
    summary = x[par] @ Wp.T + mean_k x[nbr_k] @ Wn.T + (b_p + b_n) accumulates
    in one PSUM bank.  Invalid (-1) neighbors point at an all-zero table row.
  - The final output is written directly in transposed layout [128, 2, NCP]
    f32 and unshuffled on the host (no on-chip output transposes).
"""

import os
import sys

# The tile framework's elide-DMA-wait pass coarsens DMA waits to the
# FIFO-max entry of the issuing ring, scheduled against CoreSim timings.
# On hardware (where SWDGE gathers run ~10x slower than modeled) those
# coarsened waits serialize the gather pipeline — keep precise waits.
os.environ["BACC_ELIDE_DMA_OPT_LIMIT"] = "0"

sys.path.insert(0, "/opt/trn_rl_repo")

import numpy as np
import ml_dtypes

import concourse.bass as bass
import concourse.bacc as bacc
import concourse.tile as tile
import concourse.mybir as mybir
from concourse import bass_utils

N = 20000
K = 16
H = 256
DIN = 256
NCORES = 8
NC_REAL = N // NCORES          # 2500 real nodes per core
NT = 512                       # node tile (matmul free dim / PSUM bank)
T = 5                          # tiles per core
NCP = NT * T                   # 2560 padded nodes per core
SHARD = 2528                   # table shard rows per core (28 zero pad rows)
ZROW = NC_REAL                 # all-zero table row (core0 pad) for invalid nbrs
NTAB = SHARD * NCORES          # 20224 table rows
NQ = (NT // 4) * K             # 2048 neighbor idxs per quarter-tile

F32 = mybir.dt.float32
BF16 = mybir.dt.bfloat16
I16 = mybir.dt.int16
BF = ml_dtypes.bfloat16

_CACHE = {}


def _build(depth: int):
    nc = bacc.Bacc("TRN2", target_bir_lowering=False, debug=False,
                   num_devices=NCORES, num_swdge_queues=4)

    featT = nc.dram_tensor("featT", [128, 2, NCP], BF16, kind="ExternalInput")
    invcnt = nc.dram_tensor("invcnt", [128, NCP], BF16, kind="ExternalInput")
    nbr_idx = nc.dram_tensor("nbr_idx", [128, T, NT], I16, kind="ExternalInput")
    par_idx = nc.dram_tensor("par_idx", [128, NCP // 16], I16, kind="ExternalInput")
    w_in = nc.dram_tensor("w_in", [128, 2, H], BF16, kind="ExternalInput")
    w_ih = nc.dram_tensor("w_ih", [128, 2, 3 * H], BF16, kind="ExternalInput")
    w_hh = nc.dram_tensor("w_hh", [128, 2, 3 * H], BF16, kind="ExternalInput")
    w_p = nc.dram_tensor("w_p", [128, 2, H], BF16, kind="ExternalInput")
    w_n = nc.dram_tensor("w_n", [128, 2, H], BF16, kind="ExternalInput")
    # bias columns: 0-1 b_in, 2-3 b_p+b_n, 4-5 b_r, 6-7 b_z, 8-9 b_ih_n,
    # 10-11 b_hh_n  (per 128-feature chunk)
    biases = nc.dram_tensor("biases", [128, 12], F32, kind="ExternalInput")
    ident_b = nc.dram_tensor("ident_b", [128, 128], BF16, kind="ExternalInput")
    y = nc.dram_tensor("y", [128, 2, NCP], F32, kind="ExternalOutput")

    SIG = mybir.ActivationFunctionType.Sigmoid
    TANH = mybir.ActivationFunctionType.Tanh
    COPY = mybir.ActivationFunctionType.Copy
    ADD = mybir.AluOpType.add
    MULT = mybir.AluOpType.mult

    with tile.TileContext(nc) as tc:
        with (
            tc.tile_pool(name="const", bufs=1) as constp,
            tc.tile_pool(name="state", bufs=1) as statep,
            tc.tile_pool(name="dram", bufs=1, space="DRAM") as dramp,
            tc.tile_pool(name="pgat", bufs=2) as pgatp,
            tc.tile_pool(name="prm", bufs=1) as prmp,
            tc.tile_pool(name="gath", bufs=2) as gathp,
            tc.tile_pool(name="tt", bufs=2) as ttp,
            tc.tile_pool(name="work", bufs=2) as workp,
            tc.tile_pool(name="tmp", bufs=2) as tmpp,
            tc.tile_pool(name="yout", bufs=2) as youtp,
            tc.tile_pool(name="ps", bufs=2, space="PSUM") as psp,
            tc.tile_pool(name="psg", bufs=6, space="PSUM") as psgp,
        ):
            # ---- resident constants -------------------------------------
            win_sb = constp.tile([128, 2, H], BF16, name="win_sb")
            nc.sync.dma_start(win_sb[:], w_in.ap())
            wih_sb = constp.tile([128, 2, 3 * H], BF16, name="wih_sb")
            nc.sync.dma_start(wih_sb[:], w_ih.ap())
            whh_sb = constp.tile([128, 2, 3 * H], BF16, name="whh_sb")
            nc.sync.dma_start(whh_sb[:], w_hh.ap())
            wp_sb = constp.tile([128, 2, H], BF16, name="wp_sb")
            nc.sync.dma_start(wp_sb[:], w_p.ap())
            wn_sb = constp.tile([128, 2, H], BF16, name="wn_sb")
            nc.sync.dma_start(wn_sb[:], w_n.ap())
            bias_sb = constp.tile([128, 12], F32, name="bias_sb")
            nc.sync.dma_start(bias_sb[:], biases.ap())
            idb_sb = constp.tile([128, 128], BF16, name="idb_sb")
            nc.sync.dma_start(idb_sb[:], ident_b.ap())
            feat_sb = constp.tile([128, 2, NCP], BF16, name="feat_sb")
            nc.sync.dma_start(feat_sb[:], featT.ap())
            inv_sb = constp.tile([128, NCP], BF16, name="inv_sb")
            nc.sync.dma_start(inv_sb[:], invcnt.ap())
            nbr_sb = constp.tile([128, T, NT], I16, name="nbr_sb")
            nc.sync.dma_start(nbr_sb[:], nbr_idx.ap())
            par_sb = constp.tile([128, NCP // 16], I16, name="par_sb")
            nc.sync.dma_start(par_sb[:], par_idx.ap())

            xF = [statep.tile([128, 2, NCP], BF16, name=f"xF{i}") for i in range(2)]
            qctr = [0]  # SWDGE queue round-robin across all gathers

            def next_q():
                q = qctr[0] % 4
                qctr[0] += 1
                return q

            xloc = dramp.tile([SHARD, H], BF16, name="xloc")
            ntabs = max(depth, 1)
            xtabs = [dramp.tile([NTAB, H], BF16, name=f"xtab{i}",
                                addr_space="Shared") for i in range(ntabs)]

            # zero rows (shard pad; serve as invalid-neighbor targets)
            zero_sb = constp.tile([128, H], BF16, name="zero_sb")
            nc.vector.memset(zero_sb[:], 0.0)

            def write_table_tile(xf, t):
                """transpose tile t of xf (bf16) to row-major, one batched DMA
                to xloc (plus a partial-block DMA on the last tile)."""
                ts0 = t * NT
                nb = NT // 128
                rm = workp.tile([128, nb, H], BF16, tag="rm", name="rm")
                nfull = min(NT, NC_REAL - ts0) // 128      # full 128-row blocks
                nblk = nb if ts0 + NT <= NC_REAL else nfull + 1
                for b in range(nblk):
                    for c in range(2):
                        pst = psp.tile([128, 128], BF16, tag="sum", name="pst")
                        nc.tensor.transpose(
                            pst[:], xf[:, c, ts0 + b * 128:ts0 + (b + 1) * 128],
                            idb_sb[:])
                        nc.scalar.activation(rm[:, b, c * 128:(c + 1) * 128],
                                             pst[:], COPY)
                if nfull > 0:
                    nc.sync.dma_start(
                        xloc[ts0:ts0 + nfull * 128, :].rearrange(
                            "(b p) f -> p b f", p=128),
                        rm[:, 0:nfull, :])
                rem = min(NT, NC_REAL - ts0) - nfull * 128  # partial tail rows
                if rem > 0:
                    r0 = ts0 + nfull * 128
                    nc.sync.dma_start(xloc[r0:r0 + rem, :],
                                      rm[0:rem, nfull, :])

            def zero_pad_rows():
                nc.sync.dma_start(xloc[NC_REAL:SHARD, :],
                                  zero_sb[0:SHARD - NC_REAL, :])

            def allgather(xtab):
                nc.gpsimd.collective_compute(
                    "AllGather", mybir.AluOpType.bypass,
                    replica_groups=[list(range(NCORES))],
                    ins=[xloc[0:SHARD, :].opt()],
                    outs=[xtab[0:NTAB, :].opt()],
                )

            # ---- layer 0: x0 = W_in @ feat + b_in ------------------------
            for t in range(T):
                ts = slice(t * NT, (t + 1) * NT)
                for oc in range(2):
                    ps = psp.tile([128, NT], F32, tag="sum", name="ps0")
                    for dc in range(2):
                        nc.tensor.matmul(ps[:], win_sb[:, dc, oc * 128:(oc + 1) * 128],
                                         feat_sb[:, dc, ts],
                                         start=(dc == 0), stop=(dc == 1))
                    if depth == 0:
                        yt = youtp.tile([128, NT], F32, tag="y", name="yt0")
                        nc.vector.tensor_scalar_add(yt[:], ps[:],
                                                    bias_sb[:, oc:oc + 1])
                        nc.sync.dma_start(y.ap()[:, oc, ts], yt[:])
                    else:
                        nc.vector.tensor_scalar_add(xF[0][:, oc, ts], ps[:],
                                                    bias_sb[:, oc:oc + 1])
                if depth > 0:
                    write_table_tile(xF[0], t)
            if depth > 0:
                zero_pad_rows()
                allgather(xtabs[0])

            # ---- GRU layers ---------------------------------------------
            cur = 0
            for layer in range(depth):
                last = layer == depth - 1
                xf_in, xf_out = xF[cur], xF[1 - cur]
                xtab = xtabs[layer]
                # one merged parent gather for the whole layer (row-major,
                # then HWDGE transpose-DMA + re-layout copy)
                prm = prmp.tile([128, NCP // 128, H], BF16, tag="prm", name="prm")
                nc.gpsimd.dma_gather(prm[:], xtab[:], par_sb[:],
                                     NCP, NCP, H, transpose=False,
                                     single_packet=False, queue_num=next_q())
                ptt = prmp.tile([128, NCP // 128, 2, 128], BF16, tag="ptt",
                                name="ptt")
                nc.sync.dma_start_transpose(ptt[:, :, :, :], prm[:, :, :])
                pgat = pgatp.tile([128, 2, NCP], BF16, tag="pgat", name="pgat")
                for hc in range(2):
                    nc.vector.tensor_copy(
                        pgat[:, hc, :].rearrange("p (c j) -> p c j", j=128),
                        ptt[:, :, hc, :])
                def gru_block(c0, w, nsum):
                    """summary + GRU for columns [c0, c0+w); nsum covers the
                    owning tile, sliced locally."""
                    bs = slice(c0, c0 + w)
                    t0 = c0 - (c0 % NT)
                    ls = slice(c0 - t0, c0 - t0 + w)
                    nmean = workp.tile([128, 2, NT], BF16, tag="nmean",
                                       name="nmean")
                    for c in range(2):
                        nc.vector.tensor_mul(nmean[:, c, ls], nsum[:, c, ls],
                                             inv_sb[:, bs])
                    # summary = pgat @ Wp.T + nmean @ Wn.T + (b_p + b_n)
                    sT = workp.tile([128, 2, NT], BF16, tag="sT", name="sT")
                    for oc in range(2):
                        ps = psp.tile([128, NT], F32, tag="sum", name="psS")
                        for hc in range(2):
                            nc.tensor.matmul(ps[:, 0:w],
                                             wp_sb[:, hc, oc * 128:(oc + 1) * 128],
                                             pgat[:, hc, bs],
                                             start=(hc == 0), stop=False)
                        for hc in range(2):
                            nc.tensor.matmul(ps[:, 0:w],
                                             wn_sb[:, hc, oc * 128:(oc + 1) * 128],
                                             nmean[:, hc, ls],
                                             start=False, stop=(hc == 1))
                        nc.vector.tensor_scalar_add(sT[:, oc, ls], ps[:, 0:w],
                                                    bias_sb[:, 2 + oc:3 + oc])
                    # GRU gates, per output chunk
                    for oc in range(2):
                        rp = psgp.tile([128, NT], F32, tag="gate", name="rp")
                        zp = psgp.tile([128, NT], F32, tag="gate", name="zp")
                        ip = psgp.tile([128, NT], F32, tag="gate", name="ip")
                        hp = psgp.tile([128, NT], F32, tag="gate", name="hp")
                        for gate, pst in ((0, rp), (1, zp)):
                            o0 = gate * H + oc * 128
                            for hc in range(2):
                                nc.tensor.matmul(pst[:, 0:w],
                                                 wih_sb[:, hc, o0:o0 + 128],
                                                 xf_in[:, hc, bs],
                                                 start=(hc == 0), stop=False)
                            for hc in range(2):
                                nc.tensor.matmul(pst[:, 0:w],
                                                 whh_sb[:, hc, o0:o0 + 128],
                                                 sT[:, hc, ls],
                                                 start=False, stop=(hc == 1))
                        o0 = 2 * H + oc * 128
                        for hc in range(2):
                            nc.tensor.matmul(ip[:, 0:w], wih_sb[:, hc, o0:o0 + 128],
                                             xf_in[:, hc, bs],
                                             start=(hc == 0), stop=(hc == 1))
                        for hc in range(2):
                            nc.tensor.matmul(hp[:, 0:w], whh_sb[:, hc, o0:o0 + 128],
                                             sT[:, hc, ls],
                                             start=(hc == 0), stop=(hc == 1))
                        r = tmpp.tile([128, NT], F32, tag="r", name="r")
                        nc.scalar.activation(r[:, 0:w], rp[:, 0:w], SIG,
                                             bias=bias_sb[:, 4 + oc:5 + oc])
                        z = tmpp.tile([128, NT], F32, tag="z", name="z")
                        nc.scalar.activation(z[:, 0:w], zp[:, 0:w], SIG,
                                             bias=bias_sb[:, 6 + oc:7 + oc])
                        # n = tanh((i_n + b_ih_n) + r * (h_n + b_hh_n))
                        hnr = tmpp.tile([128, NT], F32, tag="hnr", name="hnr")
                        nc.vector.scalar_tensor_tensor(
                            hnr[:, 0:w], hp[:, 0:w], bias_sb[:, 10 + oc:11 + oc],
                            r[:, 0:w], op0=ADD, op1=MULT)
                        npre = tmpp.tile([128, NT], F32, tag="r", name="npre")
                        nc.vector.scalar_tensor_tensor(
                            npre[:, 0:w], ip[:, 0:w], bias_sb[:, 8 + oc:9 + oc],
                            hnr[:, 0:w], op0=ADD, op1=ADD)
                        nt_ = tmpp.tile([128, NT], F32, tag="nt", name="nt")
                        nc.scalar.activation(nt_[:, 0:w], npre[:, 0:w], TANH)
                        # x_new = n + z * (summary - n)
                        d = tmpp.tile([128, NT], F32, tag="d", name="d")
                        nc.vector.tensor_sub(d[:, 0:w], sT[:, oc, ls], nt_[:, 0:w])
                        dz = tmpp.tile([128, NT], F32, tag="d", name="dz")
                        nc.vector.tensor_mul(dz[:, 0:w], d[:, 0:w], z[:, 0:w])
                        if last:
                            yt = youtp.tile([128, NT], F32, tag="y", name="yt")
                            nc.vector.tensor_add(yt[:, 0:w], dz[:, 0:w], nt_[:, 0:w])
                            nc.sync.dma_start(y.ap()[:, oc, bs], yt[:, 0:w])
                        else:
                            nc.vector.tensor_add(xf_out[:, oc, bs], dz[:, 0:w],
                                                 nt_[:, 0:w])

                for t in range(T):
                    ts0 = t * NT
                    # neighbor rows, two half-tile gathers (4096 idxs each;
                    # few big DMAs keep the SW/HW sem lanes from recycling
                    # mid-layer); sum groups of K=16
                    nsum = workp.tile([128, 2, NT], F32, tag="nsum", name="nsum")
                    for hf in range(2):
                        q0 = hf * (NT // 2)
                        # tail of the last tile: only 196 real nodes
                        nreal = min(NT // 2, max(NC_REAL - ts0 - q0, 0))
                        nq = ((nreal * K + 127) // 128) * 128
                        if nq == 0:
                            continue
                        nch = nq // 128
                        nrm = gathp.tile([128, 32, H], BF16, tag="nrm",
                                         name="nrm")
                        nc.gpsimd.dma_gather(
                            nrm[:, 0:nch, :], xtab[:],
                            nbr_sb[:, t, q0:q0 + nq // K],
                            nq, nq, H, transpose=False, single_packet=False,
                            queue_num=next_q())
                        ntt = ttp.tile([128, 32, 2, 128], BF16, tag="ntt",
                                       name="ntt")
                        nc.sync.dma_start_transpose(ntt[:, 0:nch, :, :],
                                                    nrm[:, 0:nch, :])
                        for c in range(2):
                            nc.vector.tensor_reduce(
                                nsum[:, c, q0:q0 + nq // K],
                                ntt[:, 0:nch, c, :].rearrange(
                                    "p n (g k) -> p n g k", k=K),
                                axis=mybir.AxisListType.X, op=ADD)
                        if t == T - 1 and hf == 0:
                            gru_block(ts0, NT // 2, nsum)
                    if t == T - 1:
                        gru_block(ts0 + NT // 2, NT // 2, nsum)
                    else:
                        gru_block(ts0, NT, nsum)
                    if not last:
                        write_table_tile(xf_out, t)
                if not last:
                    allgather(xtabs[layer + 1])
                cur = 1 - cur

    nc.compile()
    return nc


def _get_nc(depth: int):
    if depth not in _CACHE:
        _CACHE[depth] = _build(depth)
    return _CACHE[depth]


def _idx_layout(lin):
    """linear int16 idx list (len % 16 == 0) -> [128, len//16] wrapped in 16
    partitions, replicated across the 8 gpsimd core groups."""
    v = lin.reshape(-1, 16).T.astype(np.int16)        # [16, len//16]
    return np.tile(v, (8, 1))                         # [128, len//16]


def _chunk2(w):
    """[256, M] -> [128, 2, M] with [p, c, m] = w[c*128+p, m]."""
    M = w.shape[1]
    return np.ascontiguousarray(w.reshape(2, 128, M).transpose(1, 0, 2))


def prepare_inputs(inputs):
    """host-side preprocessing: returns in_maps for the 8 cores."""
    adj = np.asarray(inputs["nodeAdjacencySpecTensor"]).astype(np.int64)
    names = np.asarray(inputs["nodeNamesEncoded"], dtype=np.float32)
    attrs = np.asarray(inputs["nodeAttributesEncoded"], dtype=np.float32)

    parent = adj[:, 0]
    parent = np.clip(np.where(parent < 0, parent + N, parent), 0, N - 1)
    nbr = adj[:, 1:]
    mask = nbr >= 0
    cnt = np.maximum(mask.sum(1), 1).astype(np.float32)
    safe_n = np.where(mask, np.clip(nbr, 0, N - 1), 0).astype(np.int64)
    # node id -> table row in the [8 x SHARD] allgathered layout
    def _row(n):
        return SHARD * (n // NC_REAL) + (n % NC_REAL)
    safe = np.where(mask, _row(safe_n), ZROW).astype(np.int64)
    parent = _row(parent)
    inv = (1.0 / cnt).astype(np.float32)

    feat = np.concatenate([names, attrs], axis=1)      # [N, 256] f32

    W_in = np.asarray(inputs["W_in"], np.float32)
    W_p = np.asarray(inputs["W_parent"], np.float32)
    W_n = np.asarray(inputs["W_neighbor"], np.float32)
    W_ih = np.asarray(inputs["W_ih"], np.float32)
    W_hh = np.asarray(inputs["W_hh"], np.float32)
    b_in = np.asarray(inputs["b_in"], np.float32)
    b_p = np.asarray(inputs["b_parent"], np.float32)
    b_n = np.asarray(inputs["b_neighbor"], np.float32)
    b_ih = np.asarray(inputs["b_ih"], np.float32)
    b_hh = np.asarray(inputs["b_hh"], np.float32)

    w_in_a = _chunk2(W_in.T).astype(BF)                 # [128, 2, 256]
    w_ih_a = _chunk2(W_ih.T).astype(BF)                 # [128, 2, 768]
    w_hh_a = _chunk2(W_hh.T).astype(BF)
    w_p_a = _chunk2(W_p.T).astype(BF)
    w_n_a = _chunk2(W_n.T).astype(BF)

    bias = np.zeros((128, 12), np.float32)
    for col, vec in ((0, b_in), (2, b_p + b_n), (4, (b_ih + b_hh)[0:H]),
                     (6, (b_ih + b_hh)[H:2 * H]), (8, b_ih[2 * H:3 * H]),
                     (10, b_hh[2 * H:3 * H])):
        bias[:, col] = vec[0:128]
        bias[:, col + 1] = vec[128:256]

    ident_b = np.eye(128, dtype=BF)

    shared = dict(w_in=w_in_a, w_ih=w_ih_a, w_hh=w_hh_a, w_p=w_p_a, w_n=w_n_a,
                  biases=bias, ident_b=ident_b)

    in_maps = []
    for c in range(NCORES):
        g0 = c * NC_REAL
        # features, transposed + padded
        f = np.zeros((NCP, DIN), np.float32)
        f[:NC_REAL] = feat[g0:g0 + NC_REAL]
        featT_c = np.ascontiguousarray(
            f.T.reshape(2, 128, NCP).transpose(1, 0, 2)).astype(BF)
        # inv count broadcast
        iv = np.ones(NCP, np.float32)
        iv[:NC_REAL] = inv[g0:g0 + NC_REAL]
        inv_c = np.broadcast_to(iv.astype(BF), (128, NCP)).copy()
        # indices
        par = np.full(NCP, ZROW, np.int64)
        par[:NC_REAL] = parent[g0:g0 + NC_REAL]
        nbrs = np.full((NCP, K), ZROW, np.int64)
        nbrs[:NC_REAL] = safe[g0:g0 + NC_REAL]
        nbr_t = np.zeros((128, T, NT), np.int16)
        for t in range(T):
            nbr_t[:, t, :] = _idx_layout(nbrs[t * NT:(t + 1) * NT].reshape(-1))
        par_t = _idx_layout(par)                        # [128, 160]
        in_maps.append(dict(featT=featT_c, invcnt=inv_c, nbr_idx=nbr_t,
                            par_idx=par_t, **shared))
    return in_maps


def run(inputs, trace=False, **kw):
    depth = int(np.asarray(inputs["depth"]))
    nc = _get_nc(depth)
    in_maps = prepare_inputs(inputs)
    res = bass_utils.run_bass_kernel_spmd(nc, in_maps,
                                          core_ids=list(range(NCORES)),
                                          trace=trace, **kw)
    outs = []
    for c in range(NCORES):
        yc = np.asarray(res.results[c]["y"])            # [128, 2, NCP]
        xc = yc.transpose(2, 1, 0).reshape(NCP, H)      # [NCP, 256]
        outs.append(xc[:NC_REAL])
    return np.ascontiguousarray(
        np.concatenate(outs, axis=0).astype(np.float32)), res


def kernel(**inputs) -> np.ndarray:
    out, _ = run(inputs, trace=False)
    return out



# revision 21
# speedup vs baseline: 1.5737x; 1.2810x over previous
"""Trainium2 Bass kernel for nn_NodeInfoPropagate (GNN message passing).

Strategy (8 NeuronCores, node-parallel):
  - Shard the 20000 nodes across 8 cores (2500/core, padded to 2560 = 5 tiles
    of 512).  Weights replicated, all matmul operands bf16 (PSUM accumulates
    f32).
  - Activations live on-chip in "transposed" layout [feature-on-partition,
    node-on-free], so every matmul chains with zero transposes.
  - Per layer, the full x table [N, 256] bf16 is materialized in each core's
    HBM via AllGather (Shared scratchpad); parent + neighbor rows are fetched
    with dma_gather(transpose=False).  Gather descriptor generation runs at
    ~8ns/row on ONE Q7 core pair, so gathers are spread over all 4 SWDGE
    queues (4 Q7 pairs -> ~4x descgen).  transpose=True gathers corrupt each
    other when concurrent (shared XBAR spray state), so rows are gathered
    row-major and flipped to [feat, idx] layout with one HWDGE transpose-DMA
    per gather; those are all issued on the Sync queue (cross-engine
    transpose DMAs also corrupt each other).  All other work (reduces, GRU
    element-wise, activations, PSUM copies, table-write transposes) spreads
    over Vector/Scalar/PE so it hides under the gathers.
  - gather commutes with the linear maps: only the x table is gathered;
    summary = x[par] @ Wp.T + mean_k x[nbr_k] @ Wn.T + (b_p + b_n) accumulates
    in one PSUM bank.  Invalid (-1) neighbors point at an all-zero table row.
  - The final output is written directly in transposed layout [128, 2, NCP]
    f32 and unshuffled on the host (no on-chip output transposes).
"""

import os
import sys

# The tile framework's elide-DMA-wait pass coarsens DMA waits to the
# FIFO-max entry of the issuing ring, scheduled against CoreSim timings.
# On hardware (where SWDGE gathers run ~10x slower than modeled) those
# coarsened waits serialize the gather pipeline — keep precise waits.
os.environ["BACC_ELIDE_DMA_OPT_LIMIT"] = "0"

sys.path.insert(0, "/opt/trn_rl_repo")

import numpy as np
import ml_dtypes

import concourse.bass as bass
import concourse.bacc as bacc
import concourse.tile as tile
import concourse.mybir as mybir
from concourse import bass_utils

N = 20000
K = 16
H = 256
DIN = 256
NCORES = 8
NC_REAL = N // NCORES          # 2500 real nodes per core
NT = 512                       # node tile (matmul free dim / PSUM bank)
T = 5                          # tiles per core
NCP = NT * T                   # 2560 padded nodes per core
SHARD = 2528                   # table shard rows per core (28 zero pad rows)
ZROW = NC_REAL                 # all-zero table row (core0 pad) for invalid nbrs
NTAB = SHARD * NCORES          # 20224 table rows
NQ = (NT // 4) * K             # 2048 neighbor idxs per quarter-tile

F32 = mybir.dt.float32
BF16 = mybir.dt.bfloat16
I16 = mybir.dt.int16
BF = ml_dtypes.bfloat16

_CACHE = {}


def _build(depth: int):
    nc = bacc.Bacc("TRN2", target_bir_lowering=False, debug=False,
                   num_devices=NCORES, num_swdge_queues=4)

    featT = nc.dram_tensor("featT", [128, 2, NCP], BF16, kind="ExternalInput")
    invcnt = nc.dram_tensor("invcnt", [128, NCP // 128], F32,
                            kind="ExternalInput")
    nbr_idx = nc.dram_tensor("nbr_idx", [128, T, NT], I16, kind="ExternalInput")
    par_idx = nc.dram_tensor("par_idx", [128, NCP // 16], I16, kind="ExternalInput")
    w_in = nc.dram_tensor("w_in", [128, 2, H], BF16, kind="ExternalInput")
    w_ih = nc.dram_tensor("w_ih", [128, 2, 3 * H], BF16, kind="ExternalInput")
    w_hh = nc.dram_tensor("w_hh", [128, 2, 3 * H], BF16, kind="ExternalInput")
    w_p = nc.dram_tensor("w_p", [128, 2, H], BF16, kind="ExternalInput")
    w_n = nc.dram_tensor("w_n", [128, 2, H], BF16, kind="ExternalInput")
    # bias columns: 0-1 b_in, 2-3 b_p+b_n, 4-5 b_r, 6-7 b_z, 8-9 b_ih_n,
    # 10-11 b_hh_n  (per 128-feature chunk)
    biases = nc.dram_tensor("biases", [128, 12], F32, kind="ExternalInput")
    ident_b = nc.dram_tensor("ident_b", [128, 128], BF16, kind="ExternalInput")
    y = nc.dram_tensor("y", [128, 2, NCP], F32, kind="ExternalOutput")

    SIG = mybir.ActivationFunctionType.Sigmoid
    TANH = mybir.ActivationFunctionType.Tanh
    COPY = mybir.ActivationFunctionType.Copy
    ADD = mybir.AluOpType.add
    MULT = mybir.AluOpType.mult

    with tile.TileContext(nc) as tc:
        with (
            tc.tile_pool(name="const", bufs=1) as constp,
            tc.tile_pool(name="state", bufs=1) as statep,
            tc.tile_pool(name="dram", bufs=1, space="DRAM") as dramp,
            tc.tile_pool(name="pgat", bufs=2) as pgatp,
            tc.tile_pool(name="prm", bufs=1) as prmp,
            tc.tile_pool(name="gath", bufs=6) as gathp,
            tc.tile_pool(name="work", bufs=2) as workp,
            tc.tile_pool(name="tmp", bufs=2) as tmpp,
            tc.tile_pool(name="yout", bufs=2) as youtp,
            tc.tile_pool(name="ps", bufs=2, space="PSUM") as psp,
            tc.tile_pool(name="psg", bufs=6, space="PSUM") as psgp,
        ):
            # ---- resident constants -------------------------------------
            win_sb = constp.tile([128, 2, H], BF16, name="win_sb")
            nc.sync.dma_start(win_sb[:], w_in.ap())
            wih_sb = constp.tile([128, 2, 3 * H], BF16, name="wih_sb")
            nc.sync.dma_start(wih_sb[:], w_ih.ap())
            whh_sb = constp.tile([128, 2, 3 * H], BF16, name="whh_sb")
            nc.sync.dma_start(whh_sb[:], w_hh.ap())
            wp_sb = constp.tile([128, 2, H], BF16, name="wp_sb")
            nc.sync.dma_start(wp_sb[:], w_p.ap())
            wn_sb = constp.tile([128, 2, H], BF16, name="wn_sb")
            nc.sync.dma_start(wn_sb[:], w_n.ap())
            bias_sb = constp.tile([128, 12], F32, name="bias_sb")
            nc.sync.dma_start(bias_sb[:], biases.ap())
            idb_sb = constp.tile([128, 128], BF16, name="idb_sb")
            nc.sync.dma_start(idb_sb[:], ident_b.ap())
            feat_sb = constp.tile([128, 2, NCP], BF16, name="feat_sb")
            nc.sync.dma_start(feat_sb[:], featT.ap())
            inv_rm_sb = constp.tile([128, NCP // 128], F32, name="inv_rm_sb")
            nc.sync.dma_start(inv_rm_sb[:], invcnt.ap())
            nbr_sb = constp.tile([128, T, NT], I16, name="nbr_sb")
            nc.sync.dma_start(nbr_sb[:], nbr_idx.ap())
            par_sb = constp.tile([128, NCP // 16], I16, name="par_sb")
            nc.sync.dma_start(par_sb[:], par_idx.ap())

            xF = [statep.tile([128, 2, NCP], BF16, name=f"xF{i}") for i in range(2)]
            qctr = [0]  # SWDGE queue round-robin across all gathers

            def next_q():
                q = qctr[0] % 4
                qctr[0] += 1
                return q

            xloc = dramp.tile([SHARD, H], BF16, name="xloc")
            ntabs = max(depth, 1)
            xtabs = [dramp.tile([NTAB, H], BF16, name=f"xtab{i}",
                                addr_space="Shared") for i in range(ntabs)]

            # zero rows (shard pad; serve as invalid-neighbor targets)
            zero_sb = constp.tile([128, H], BF16, name="zero_sb")
            nc.vector.memset(zero_sb[:], 0.0)

            def write_table_tile(xf, t):
                """transpose tile t of xf (bf16) to row-major, one batched DMA
                to xloc (plus a partial-block DMA on the last tile)."""
                ts0 = t * NT
                nb = NT // 128
                rm = workp.tile([128, nb, H], BF16, tag="rm", name="rm")
                nfull = min(NT, NC_REAL - ts0) // 128      # full 128-row blocks
                nblk = nb if ts0 + NT <= NC_REAL else nfull + 1
                for b in range(nblk):
                    for c in range(2):
                        pst = psp.tile([128, 128], BF16, tag="sum", name="pst")
                        nc.tensor.transpose(
                            pst[:], xf[:, c, ts0 + b * 128:ts0 + (b + 1) * 128],
                            idb_sb[:])
                        nc.scalar.activation(rm[:, b, c * 128:(c + 1) * 128],
                                             pst[:], COPY)
                if nfull > 0:
                    nc.sync.dma_start(
                        xloc[ts0:ts0 + nfull * 128, :].rearrange(
                            "(b p) f -> p b f", p=128),
                        rm[:, 0:nfull, :])
                rem = min(NT, NC_REAL - ts0) - nfull * 128  # partial tail rows
                if rem > 0:
                    r0 = ts0 + nfull * 128
                    nc.sync.dma_start(xloc[r0:r0 + rem, :],
                                      rm[0:rem, nfull, :])

            def zero_pad_rows():
                nc.sync.dma_start(xloc[NC_REAL:SHARD, :],
                                  zero_sb[0:SHARD - NC_REAL, :])

            def allgather(xtab):
                nc.gpsimd.collective_compute(
                    "AllGather", mybir.AluOpType.bypass,
                    replica_groups=[list(range(NCORES))],
                    ins=[xloc[0:SHARD, :].opt()],
                    outs=[xtab[0:NTAB, :].opt()],
                )

            # ---- layer 0: x0 = W_in @ feat + b_in ------------------------
            for t in range(T):
                ts = slice(t * NT, (t + 1) * NT)
                for oc in range(2):
                    ps = psp.tile([128, NT], F32, tag="sum", name="ps0")
                    for dc in range(2):
                        nc.tensor.matmul(ps[:], win_sb[:, dc, oc * 128:(oc + 1) * 128],
                                         feat_sb[:, dc, ts],
                                         start=(dc == 0), stop=(dc == 1))
                    if depth == 0:
                        yt = youtp.tile([128, NT], F32, tag="y", name="yt0")
                        nc.vector.tensor_scalar_add(yt[:], ps[:],
                                                    bias_sb[:, oc:oc + 1])
                        nc.sync.dma_start(y.ap()[:, oc, ts], yt[:])
                    else:
                        nc.vector.tensor_scalar_add(xF[0][:, oc, ts], ps[:],
                                                    bias_sb[:, oc:oc + 1])
                if depth > 0:
                    write_table_tile(xF[0], t)
            if depth > 0:
                zero_pad_rows()
                allgather(xtabs[0])

            # ---- GRU layers ---------------------------------------------
            cur = 0
            for layer in range(depth):
                last = layer == depth - 1
                xf_in, xf_out = xF[cur], xF[1 - cur]
                xtab = xtabs[layer]
                # one merged parent gather for the whole layer; row-major
                # node-major blocks [128 nodes, 256 feat], PE-transposed
                # per block into [feat, node] matmul layout
                prm = prmp.tile([128, NCP // 128, H], BF16, tag="prm", name="prm")
                nc.gpsimd.dma_gather(prm[:], xtab[:], par_sb[:],
                                     NCP, NCP, H, transpose=False,
                                     single_packet=False, queue_num=next_q())
                pgat = pgatp.tile([128, 2, NCP], BF16, tag="pgat", name="pgat")
                for blk in range(NCP // 128):
                    for hc in range(2):
                        pps = psp.tile([128, 128], BF16, tag="sum", name="pps")
                        nc.tensor.transpose(
                            pps[:], prm[:, blk, hc * 128:(hc + 1) * 128],
                            idb_sb[:])
                        nc.scalar.activation(
                            pgat[:, hc, blk * 128:(blk + 1) * 128], pps[:],
                            COPY)
                def gru_block(c0, w, nmean):
                    """summary + GRU for columns [c0, c0+w); nmean covers the
                    owning tile, sliced locally."""
                    bs = slice(c0, c0 + w)
                    t0 = c0 - (c0 % NT)
                    ls = slice(c0 - t0, c0 - t0 + w)
                    # summary = pgat @ Wp.T + nmean @ Wn.T + (b_p + b_n)
                    sT = workp.tile([128, 2, NT], BF16, tag="sT", name="sT")
                    for oc in range(2):
                        ps = psp.tile([128, NT], F32, tag="sum", name="psS")
                        for hc in range(2):
                            nc.tensor.matmul(ps[:, 0:w],
                                             wp_sb[:, hc, oc * 128:(oc + 1) * 128],
                                             pgat[:, hc, bs],
                                             start=(hc == 0), stop=False)
                        for hc in range(2):
                            nc.tensor.matmul(ps[:, 0:w],
                                             wn_sb[:, hc, oc * 128:(oc + 1) * 128],
                                             nmean[:, hc, ls],
                                             start=False, stop=(hc == 1))
                        nc.vector.tensor_scalar_add(sT[:, oc, ls], ps[:, 0:w],
                                                    bias_sb[:, 2 + oc:3 + oc])
                    # GRU gates, per output chunk
                    for oc in range(2):
                        rp = psgp.tile([128, NT], F32, tag="gate", name="rp")
                        zp = psgp.tile([128, NT], F32, tag="gate", name="zp")
                        ip = psgp.tile([128, NT], F32, tag="gate", name="ip")
                        hp = psgp.tile([128, NT], F32, tag="gate", name="hp")
                        for gate, pst in ((0, rp), (1, zp)):
                            o0 = gate * H + oc * 128
                            for hc in range(2):
                                nc.tensor.matmul(pst[:, 0:w],
                                                 wih_sb[:, hc, o0:o0 + 128],
                                                 xf_in[:, hc, bs],
                                                 start=(hc == 0), stop=False)
                            for hc in range(2):
                                nc.tensor.matmul(pst[:, 0:w],
                                                 whh_sb[:, hc, o0:o0 + 128],
                                                 sT[:, hc, ls],
                                                 start=False, stop=(hc == 1))
                        o0 = 2 * H + oc * 128
                        for hc in range(2):
                            nc.tensor.matmul(ip[:, 0:w], wih_sb[:, hc, o0:o0 + 128],
                                             xf_in[:, hc, bs],
                                             start=(hc == 0), stop=(hc == 1))
                        for hc in range(2):
                            nc.tensor.matmul(hp[:, 0:w], whh_sb[:, hc, o0:o0 + 128],
                                             sT[:, hc, ls],
                                             start=(hc == 0), stop=(hc == 1))
                        r = tmpp.tile([128, NT], F32, tag="r", name="r")
                        nc.scalar.activation(r[:, 0:w], rp[:, 0:w], SIG,
                                             bias=bias_sb[:, 4 + oc:5 + oc])
                        z = tmpp.tile([128, NT], F32, tag="z", name="z")
                        nc.scalar.activation(z[:, 0:w], zp[:, 0:w], SIG,
                                             bias=bias_sb[:, 6 + oc:7 + oc])
                        # n = tanh((i_n + b_ih_n) + r * (h_n + b_hh_n))
                        hnr = tmpp.tile([128, NT], F32, tag="hnr", name="hnr")
                        nc.vector.scalar_tensor_tensor(
                            hnr[:, 0:w], hp[:, 0:w], bias_sb[:, 10 + oc:11 + oc],
                            r[:, 0:w], op0=ADD, op1=MULT)
                        npre = tmpp.tile([128, NT], F32, tag="r", name="npre")
                        nc.vector.scalar_tensor_tensor(
                            npre[:, 0:w], ip[:, 0:w], bias_sb[:, 8 + oc:9 + oc],
                            hnr[:, 0:w], op0=ADD, op1=ADD)
                        nt_ = tmpp.tile([128, NT], F32, tag="nt", name="nt")
                        nc.scalar.activation(nt_[:, 0:w], npre[:, 0:w], TANH)
                        # x_new = n + z * (summary - n)
                        d = tmpp.tile([128, NT], F32, tag="d", name="d")
                        nc.vector.tensor_sub(d[:, 0:w], sT[:, oc, ls], nt_[:, 0:w])
                        dz = tmpp.tile([128, NT], F32, tag="d", name="dz")
                        nc.vector.tensor_mul(dz[:, 0:w], d[:, 0:w], z[:, 0:w])
                        if last:
                            yt = youtp.tile([128, NT], F32, tag="y", name="yt")
                            nc.vector.tensor_add(yt[:, 0:w], dz[:, 0:w], nt_[:, 0:w])
                            nc.sync.dma_start(y.ap()[:, oc, bs], yt[:, 0:w])
                        else:
                            nc.vector.tensor_add(xf_out[:, oc, bs], dz[:, 0:w],
                                                 nt_[:, 0:w])

                for t in range(T):
                    ts0 = t * NT
                    # neighbor rows: four node-major 2048-idx gathers per
                    # tile (idx i -> neighbor i//128 of node i%128), so each
                    # node's K=16 rows land on ONE partition across the free
                    # chunk axis: reduce = strided DVE free-dim reduce, no
                    # transpose DMAs.  Then scale by 1/cnt (per-partition
                    # scalar) and PE-transpose the 16x-smaller result into
                    # [feat, node] matmul layout.
                    nmean_t = workp.tile([128, 2, NT], BF16, tag="nmean",
                                         name="nmean")
                    for qf in range(4):
                        q0 = qf * 128
                        blk = t * 4 + qf
                        nrm = gathp.tile([128, K, H], BF16, tag="nrm",
                                         name="nrm")
                        nc.gpsimd.dma_gather(
                            nrm[:], xtab[:],
                            nbr_sb[:, t, q0:q0 + 128],
                            NQ, NQ, H, transpose=False, single_packet=False,
                            queue_num=next_q())
                        nsum_rm = workp.tile([128, H], F32, tag="nsr",
                                             name="nsr")
                        nc.vector.tensor_reduce(
                            nsum_rm[:], nrm[:].rearrange("p c f -> p f c"),
                            axis=mybir.AxisListType.X, op=ADD)
                        nmean_rm = workp.tile([128, H], BF16, tag="nmr",
                                              name="nmr")
                        nc.vector.tensor_scalar_mul(
                            nmean_rm[:], nsum_rm[:],
                            inv_rm_sb[:, blk:blk + 1])
                        for hc in range(2):
                            nps = psp.tile([128, 128], BF16, tag="sum",
                                           name="nps")
                            nc.tensor.transpose(
                                nps[:], nmean_rm[:, hc * 128:(hc + 1) * 128],
                                idb_sb[:])
                            nc.scalar.activation(
                                nmean_t[:, hc, q0:q0 + 128], nps[:], COPY)
                        if t == T - 1 and qf == 1:
                            gru_block(ts0, NT // 2, nmean_t)
                    if t == T - 1:
                        gru_block(ts0 + NT // 2, NT // 2, nmean_t)
                    else:
                        gru_block(ts0, NT, nmean_t)
                    if not last:
                        write_table_tile(xf_out, t)
                if not last:
                    allgather(xtabs[layer + 1])
                cur = 1 - cur

    nc.compile()
    return nc


def _get_nc(depth: int):
    if depth not in _CACHE:
        _CACHE[depth] = _build(depth)
    return _CACHE[depth]


def _idx_layout(lin):
    """linear int16 idx list (len % 16 == 0) -> [128, len//16] wrapped in 16
    partitions, replicated across the 8 gpsimd core groups."""
    v = lin.reshape(-1, 16).T.astype(np.int16)        # [16, len//16]
    return np.tile(v, (8, 1))                         # [128, len//16]


def _chunk2(w):
    """[256, M] -> [128, 2, M] with [p, c, m] = w[c*128+p, m]."""
    M = w.shape[1]
    return np.ascontiguousarray(w.reshape(2, 128, M).transpose(1, 0, 2))


def prepare_inputs(inputs):
    """host-side preprocessing: returns in_maps for the 8 cores."""
    adj = np.asarray(inputs["nodeAdjacencySpecTensor"]).astype(np.int64)
    names = np.asarray(inputs["nodeNamesEncoded"], dtype=np.float32)
    attrs = np.asarray(inputs["nodeAttributesEncoded"], dtype=np.float32)

    parent = adj[:, 0]
    parent = np.clip(np.where(parent < 0, parent + N, parent), 0, N - 1)
    nbr = adj[:, 1:]
    mask = nbr >= 0
    cnt = np.maximum(mask.sum(1), 1).astype(np.float32)
    safe_n = np.where(mask, np.clip(nbr, 0, N - 1), 0).astype(np.int64)
    # node id -> table row in the [8 x SHARD] allgathered layout
    def _row(n):
        return SHARD * (n // NC_REAL) + (n % NC_REAL)
    safe = np.where(mask, _row(safe_n), ZROW).astype(np.int64)
    parent = _row(parent)
    inv = (1.0 / cnt).astype(np.float32)

    feat = np.concatenate([names, attrs], axis=1)      # [N, 256] f32

    W_in = np.asarray(inputs["W_in"], np.float32)
    W_p = np.asarray(inputs["W_parent"], np.float32)
    W_n = np.asarray(inputs["W_neighbor"], np.float32)
    W_ih = np.asarray(inputs["W_ih"], np.float32)
    W_hh = np.asarray(inputs["W_hh"], np.float32)
    b_in = np.asarray(inputs["b_in"], np.float32)
    b_p = np.asarray(inputs["b_parent"], np.float32)
    b_n = np.asarray(inputs["b_neighbor"], np.float32)
    b_ih = np.asarray(inputs["b_ih"], np.float32)
    b_hh = np.asarray(inputs["b_hh"], np.float32)

    w_in_a = _chunk2(W_in.T).astype(BF)                 # [128, 2, 256]
    w_ih_a = _chunk2(W_ih.T).astype(BF)                 # [128, 2, 768]
    w_hh_a = _chunk2(W_hh.T).astype(BF)
    w_p_a = _chunk2(W_p.T).astype(BF)
    w_n_a = _chunk2(W_n.T).astype(BF)

    bias = np.zeros((128, 12), np.float32)
    for col, vec in ((0, b_in), (2, b_p + b_n), (4, (b_ih + b_hh)[0:H]),
                     (6, (b_ih + b_hh)[H:2 * H]), (8, b_ih[2 * H:3 * H]),
                     (10, b_hh[2 * H:3 * H])):
        bias[:, col] = vec[0:128]
        bias[:, col + 1] = vec[128:256]

    ident_b = np.eye(128, dtype=BF)

    shared = dict(w_in=w_in_a, w_ih=w_ih_a, w_hh=w_hh_a, w_p=w_p_a, w_n=w_n_a,
                  biases=bias, ident_b=ident_b)

    in_maps = []
    for c in range(NCORES):
        g0 = c * NC_REAL
        # features, transposed + padded
        f = np.zeros((NCP, DIN), np.float32)
        f[:NC_REAL] = feat[g0:g0 + NC_REAL]
        featT_c = np.ascontiguousarray(
            f.T.reshape(2, 128, NCP).transpose(1, 0, 2)).astype(BF)
        # inv count, node-major blocks: inv_rm[p, b] = 1/cnt[128b + p]
        iv = np.ones(NCP, np.float32)
        iv[:NC_REAL] = inv[g0:g0 + NC_REAL]
        inv_c = np.ascontiguousarray(iv.reshape(NCP // 128, 128).T)
        # indices
        par = np.full(NCP, ZROW, np.int64)
        par[:NC_REAL] = parent[g0:g0 + NC_REAL]
        nbrs = np.full((NCP, K), ZROW, np.int64)
        nbrs[:NC_REAL] = safe[g0:g0 + NC_REAL]
        # node-major gather order: idx i of block b is neighbor i//128 of
        # node 128b + i%128 -> block's linear list = nbrs[block].T.ravel()
        nbr_t = np.zeros((128, T, NT), np.int16)
        for t in range(T):
            blks = [nbrs[t * NT + 128 * q: t * NT + 128 * (q + 1)].T.reshape(-1)
                    for q in range(4)]
            nbr_t[:, t, :] = _idx_layout(np.concatenate(blks))
        par_t = _idx_layout(par)                        # [128, 160]
        in_maps.append(dict(featT=featT_c, invcnt=inv_c, nbr_idx=nbr_t,
                            par_idx=par_t, **shared))
    return in_maps


def run(inputs, trace=False, **kw):
    depth = int(np.asarray(inputs["depth"]))
    nc = _get_nc(depth)
    in_maps = prepare_inputs(inputs)
    res = bass_utils.run_bass_kernel_spmd(nc, in_maps,
                                          core_ids=list(range(NCORES)),
                                          trace=trace, **kw)
    outs = []
    for c in range(NCORES):
        yc = np.asarray(res.results[c]["y"])            # [128, 2, NCP]
        xc = yc.transpose(2, 1, 0).reshape(NCP, H)      # [NCP, 256]
        outs.append(xc[:NC_REAL])
    return np.ascontiguousarray(
        np.concatenate(outs, axis=0).astype(np.float32)), res


def kernel(**inputs) -> np.ndarray:
    out, _ = run(inputs, trace=False)
    return out



# revision 27
# speedup vs baseline: 1.8363x; 1.1669x over previous
"""Trainium2 Bass kernel for nn_NodeInfoPropagate (GNN message passing).

Strategy (8 NeuronCores, node-parallel):
  - Shard the 20000 nodes across 8 cores (2500/core, padded to 2560 = 5 tiles
    of 512).  Weights replicated, all matmul operands bf16 (PSUM accumulates
    f32).
  - Activations live on-chip in "transposed" layout [feature-on-partition,
    node-on-free], so every matmul chains with zero transposes.
  - Per layer, the full x table [N, 256] bf16 is materialized in each core's
    HBM via AllGather (Shared scratchpad); parent + neighbor rows are fetched
    with dma_gather(transpose=False).  Gather descriptor generation runs at
    ~8ns/row on ONE Q7 core pair, so gathers are spread over all 4 SWDGE
    queues (4 Q7 pairs -> ~4x descgen).  transpose=True gathers corrupt each
    other when concurrent (shared XBAR spray state), so rows are gathered
    row-major and flipped to [feat, idx] layout with one HWDGE transpose-DMA
    per gather; those are all issued on the Sync queue (cross-engine
    transpose DMAs also corrupt each other).  All other work (reduces, GRU
    element-wise, activations, PSUM copies, table-write transposes) spreads
    over Vector/Scalar/PE so it hides under the gathers.
  - gather commutes with the linear maps: only the x table is gathered;
    summary = x[par] @ Wp.T + mean_k x[nbr_k] @ Wn.T + (b_p + b_n) accumulates
    in one PSUM bank.  Invalid (-1) neighbors point at an all-zero table row.
  - The final output is written directly in transposed layout [128, 2, NCP]
    f32 and unshuffled on the host (no on-chip output transposes).
"""

import os
import sys

# The tile framework's elide-DMA-wait pass coarsens DMA waits to the
# FIFO-max entry of the issuing ring, scheduled against CoreSim timings.
# On hardware (where SWDGE gathers run ~10x slower than modeled) those
# coarsened waits serialize the gather pipeline — keep precise waits.
os.environ["BACC_ELIDE_DMA_OPT_LIMIT"] = "0"

sys.path.insert(0, "/opt/trn_rl_repo")

import numpy as np
import ml_dtypes

import concourse.bass as bass
import concourse.bacc as bacc
import concourse.tile as tile
import concourse.mybir as mybir
from concourse import bass_utils

N = 20000
K = 16
H = 256
DIN = 256
NCORES = 8
NC_REAL = N // NCORES          # 2500 real nodes per core
NT = 512                       # node tile (matmul free dim / PSUM bank)
T = 5                          # tiles per core
NCP = NT * T                   # 2560 padded nodes per core
SHARD = 2528                   # table shard rows per core (28 zero pad rows)
ZROW = NC_REAL                 # all-zero table row (core0 pad) for invalid nbrs
NTAB = SHARD * NCORES          # 20224 table rows
NQ = (NT // 4) * K             # 2048 neighbor idxs per quarter-tile

F32 = mybir.dt.float32
BF16 = mybir.dt.bfloat16
I16 = mybir.dt.int16
BF = ml_dtypes.bfloat16

_CACHE = {}


def _build(depth: int):
    nc = bacc.Bacc("TRN2", target_bir_lowering=False, debug=False,
                   num_devices=NCORES, num_swdge_queues=4)

    featT = nc.dram_tensor("featT", [128, 2, NCP], BF16, kind="ExternalInput")
    invcnt = nc.dram_tensor("invcnt", [128, NCP // 128], F32,
                            kind="ExternalInput")
    nbr_idx = nc.dram_tensor("nbr_idx", [128, T, NT], I16, kind="ExternalInput")
    par_idx = nc.dram_tensor("par_idx", [128, NCP // 16], I16, kind="ExternalInput")
    w_in = nc.dram_tensor("w_in", [128, 2, H], BF16, kind="ExternalInput")
    w_ih = nc.dram_tensor("w_ih", [128, 2, 3 * H], BF16, kind="ExternalInput")
    w_hh = nc.dram_tensor("w_hh", [128, 2, 3 * H], BF16, kind="ExternalInput")
    w_p = nc.dram_tensor("w_p", [128, 2, H], BF16, kind="ExternalInput")
    w_n = nc.dram_tensor("w_n", [128, 2, H], BF16, kind="ExternalInput")
    # bias columns: 0-1 b_in, 2-3 b_p+b_n, 4-5 b_r, 6-7 b_z, 8-9 b_ih_n,
    # 10-11 b_hh_n  (per 128-feature chunk)
    biases = nc.dram_tensor("biases", [128, 12], F32, kind="ExternalInput")
    ident_b = nc.dram_tensor("ident_b", [128, 128], BF16, kind="ExternalInput")
    y = nc.dram_tensor("y", [128, 2, NCP], F32, kind="ExternalOutput")

    SIG = mybir.ActivationFunctionType.Sigmoid
    TANH = mybir.ActivationFunctionType.Tanh
    COPY = mybir.ActivationFunctionType.Copy
    ADD = mybir.AluOpType.add
    MULT = mybir.AluOpType.mult

    with tile.TileContext(nc) as tc:
        with (
            tc.tile_pool(name="const", bufs=1) as constp,
            tc.tile_pool(name="state", bufs=1) as statep,
            tc.tile_pool(name="dram", bufs=1, space="DRAM") as dramp,
            tc.tile_pool(name="pgat", bufs=2) as pgatp,
            tc.tile_pool(name="prm", bufs=1) as prmp,
            tc.tile_pool(name="gath", bufs=6) as gathp,
            tc.tile_pool(name="work", bufs=2) as workp,
            tc.tile_pool(name="tmp", bufs=2) as tmpp,
            tc.tile_pool(name="yout", bufs=2) as youtp,
            tc.tile_pool(name="ps", bufs=2, space="PSUM") as psp,
            tc.tile_pool(name="psg", bufs=6, space="PSUM") as psgp,
        ):
            # ---- resident constants -------------------------------------
            win_sb = constp.tile([128, 2, H], BF16, name="win_sb")
            nc.sync.dma_start(win_sb[:], w_in.ap())
            wih_sb = constp.tile([128, 2, 3 * H], BF16, name="wih_sb")
            nc.sync.dma_start(wih_sb[:], w_ih.ap())
            whh_sb = constp.tile([128, 2, 3 * H], BF16, name="whh_sb")
            nc.sync.dma_start(whh_sb[:], w_hh.ap())
            wp_sb = constp.tile([128, 2, H], BF16, name="wp_sb")
            nc.sync.dma_start(wp_sb[:], w_p.ap())
            wn_sb = constp.tile([128, 2, H], BF16, name="wn_sb")
            nc.sync.dma_start(wn_sb[:], w_n.ap())
            bias_sb = constp.tile([128, 12], F32, name="bias_sb")
            nc.sync.dma_start(bias_sb[:], biases.ap())
            idb_sb = constp.tile([128, 128], BF16, name="idb_sb")
            nc.sync.dma_start(idb_sb[:], ident_b.ap())
            feat_sb = constp.tile([128, 2, NCP], BF16, name="feat_sb")
            nc.sync.dma_start(feat_sb[:], featT.ap())
            inv_rm_sb = constp.tile([128, NCP // 128], F32, name="inv_rm_sb")
            nc.sync.dma_start(inv_rm_sb[:], invcnt.ap())
            nbr_sb = constp.tile([128, T, NT], I16, name="nbr_sb")
            nc.sync.dma_start(nbr_sb[:], nbr_idx.ap())
            par_sb = constp.tile([128, NCP // 16], I16, name="par_sb")
            nc.sync.dma_start(par_sb[:], par_idx.ap())

            xF = [statep.tile([128, 2, NCP], BF16, name=f"xF{i}") for i in range(2)]
            qctr = [0]  # SWDGE queue round-robin across all gathers

            def next_q():
                q = qctr[0] % 4
                qctr[0] += 1
                return q

            xloc = dramp.tile([SHARD, H], BF16, name="xloc")
            ntabs = max(depth, 1)
            xtabs = [dramp.tile([NTAB, H], BF16, name=f"xtab{i}",
                                addr_space="Shared") for i in range(ntabs)]

            # zero rows (shard pad; serve as invalid-neighbor targets)
            zero_sb = constp.tile([128, H], BF16, name="zero_sb")
            nc.vector.memset(zero_sb[:], 0.0)

            def write_table_tile(xf, t):
                """transpose tile t of xf (bf16) to row-major, one batched DMA
                to xloc (plus a partial-block DMA on the last tile)."""
                ts0 = t * NT
                nb = NT // 128
                rm = workp.tile([128, nb, H], BF16, tag="rm", name="rm")
                nfull = min(NT, NC_REAL - ts0) // 128      # full 128-row blocks
                nblk = nb if ts0 + NT <= NC_REAL else nfull + 1
                for b in range(nblk):
                    for c in range(2):
                        pst = psp.tile([128, 128], BF16, tag="sum", name="pst")
                        nc.tensor.transpose(
                            pst[:], xf[:, c, ts0 + b * 128:ts0 + (b + 1) * 128],
                            idb_sb[:])
                        nc.scalar.activation(rm[:, b, c * 128:(c + 1) * 128],
                                             pst[:], COPY)
                if nfull > 0:
                    nc.sync.dma_start(
                        xloc[ts0:ts0 + nfull * 128, :].rearrange(
                            "(b p) f -> p b f", p=128),
                        rm[:, 0:nfull, :])
                rem = min(NT, NC_REAL - ts0) - nfull * 128  # partial tail rows
                if rem > 0:
                    r0 = ts0 + nfull * 128
                    nc.sync.dma_start(xloc[r0:r0 + rem, :],
                                      rm[0:rem, nfull, :])

            def zero_pad_rows():
                nc.sync.dma_start(xloc[NC_REAL:SHARD, :],
                                  zero_sb[0:SHARD - NC_REAL, :])

            def allgather(xtab, r0=0, r1=SHARD):
                # chunked: gather shard rows [r0, r1) from every core into
                # the strided row-bands of the table
                xt_view = xtab[0:NTAB, :].rearrange("(c r) f -> c r f",
                                                    r=SHARD)
                nc.gpsimd.collective_compute(
                    "AllGather", mybir.AluOpType.bypass,
                    replica_groups=[list(range(NCORES))],
                    ins=[xloc[r0:r1, :].opt()],
                    outs=[xt_view[:, r0:r1, :].opt()],
                )

            # ---- layer 0: x0 = W_in @ feat + b_in ------------------------
            for t in range(T):
                ts = slice(t * NT, (t + 1) * NT)
                for oc in range(2):
                    ps = psp.tile([128, NT], F32, tag="sum", name="ps0")
                    for dc in range(2):
                        nc.tensor.matmul(ps[:], win_sb[:, dc, oc * 128:(oc + 1) * 128],
                                         feat_sb[:, dc, ts],
                                         start=(dc == 0), stop=(dc == 1))
                    if depth == 0:
                        yt = youtp.tile([128, NT], F32, tag="y", name="yt0")
                        nc.vector.tensor_scalar_add(yt[:], ps[:],
                                                    bias_sb[:, oc:oc + 1])
                        nc.sync.dma_start(y.ap()[:, oc, ts], yt[:])
                    else:
                        nc.vector.tensor_scalar_add(xF[0][:, oc, ts], ps[:],
                                                    bias_sb[:, oc:oc + 1])
                if depth > 0:
                    write_table_tile(xF[0], t)
            if depth > 0:
                zero_pad_rows()
                allgather(xtabs[0])

            # ---- GRU layers ---------------------------------------------
            cur = 0
            for layer in range(depth):
                last = layer == depth - 1
                xf_in, xf_out = xF[cur], xF[1 - cur]
                xtab = xtabs[layer]
                # one merged parent gather for the whole layer; row-major
                # node-major blocks [128 nodes, 256 feat], PE-transposed
                # per block into [feat, node] matmul layout
                prm = prmp.tile([128, NCP // 128, H], BF16, tag="prm", name="prm")
                nc.gpsimd.dma_gather(prm[:], xtab[:], par_sb[:],
                                     NCP, NCP, H, transpose=False,
                                     single_packet=False, queue_num=next_q())
                pgat = pgatp.tile([128, 2, NCP], BF16, tag="pgat", name="pgat")
                for blk in range(NCP // 128):
                    for hc in range(2):
                        pps = psp.tile([128, 128], BF16, tag="sum", name="pps")
                        nc.tensor.transpose(
                            pps[:], prm[:, blk, hc * 128:(hc + 1) * 128],
                            idb_sb[:])
                        nc.scalar.activation(
                            pgat[:, hc, blk * 128:(blk + 1) * 128], pps[:],
                            COPY)
                def gru_block(c0, w, nmean):
                    """summary + GRU for columns [c0, c0+w); nmean covers the
                    owning tile, sliced locally."""
                    bs = slice(c0, c0 + w)
                    t0 = c0 - (c0 % NT)
                    ls = slice(c0 - t0, c0 - t0 + w)
                    # summary = pgat @ Wp.T + nmean @ Wn.T + (b_p + b_n)
                    sT = workp.tile([128, 2, NT], BF16, tag="sT", name="sT")
                    for oc in range(2):
                        ps = psp.tile([128, NT], F32, tag="sum", name="psS")
                        for hc in range(2):
                            nc.tensor.matmul(ps[:, 0:w],
                                             wp_sb[:, hc, oc * 128:(oc + 1) * 128],
                                             pgat[:, hc, bs],
                                             start=(hc == 0), stop=False)
                        for hc in range(2):
                            nc.tensor.matmul(ps[:, 0:w],
                                             wn_sb[:, hc, oc * 128:(oc + 1) * 128],
                                             nmean[:, hc, ls],
                                             start=False, stop=(hc == 1))
                        nc.vector.tensor_scalar_add(sT[:, oc, ls], ps[:, 0:w],
                                                    bias_sb[:, 2 + oc:3 + oc])
                    # GRU gates, per output chunk
                    for oc in range(2):
                        rp = psgp.tile([128, NT], F32, tag="gate", name="rp")
                        zp = psgp.tile([128, NT], F32, tag="gate", name="zp")
                        ip = psgp.tile([128, NT], F32, tag="gate", name="ip")
                        hp = psgp.tile([128, NT], F32, tag="gate", name="hp")
                        for gate, pst in ((0, rp), (1, zp)):
                            o0 = gate * H + oc * 128
                            for hc in range(2):
                                nc.tensor.matmul(pst[:, 0:w],
                                                 wih_sb[:, hc, o0:o0 + 128],
                                                 xf_in[:, hc, bs],
                                                 start=(hc == 0), stop=False)
                            for hc in range(2):
                                nc.tensor.matmul(pst[:, 0:w],
                                                 whh_sb[:, hc, o0:o0 + 128],
                                                 sT[:, hc, ls],
                                                 start=False, stop=(hc == 1))
                        o0 = 2 * H + oc * 128
                        for hc in range(2):
                            nc.tensor.matmul(ip[:, 0:w], wih_sb[:, hc, o0:o0 + 128],
                                             xf_in[:, hc, bs],
                                             start=(hc == 0), stop=(hc == 1))
                        for hc in range(2):
                            nc.tensor.matmul(hp[:, 0:w], whh_sb[:, hc, o0:o0 + 128],
                                             sT[:, hc, ls],
                                             start=(hc == 0), stop=(hc == 1))
                        r = tmpp.tile([128, NT], F32, tag="r", name="r")
                        nc.scalar.activation(r[:, 0:w], rp[:, 0:w], SIG,
                                             bias=bias_sb[:, 4 + oc:5 + oc])
                        z = tmpp.tile([128, NT], F32, tag="z", name="z")
                        nc.scalar.activation(z[:, 0:w], zp[:, 0:w], SIG,
                                             bias=bias_sb[:, 6 + oc:7 + oc])
                        # n = tanh((i_n + b_ih_n) + r * (h_n + b_hh_n))
                        hnr = tmpp.tile([128, NT], F32, tag="hnr", name="hnr")
                        nc.vector.scalar_tensor_tensor(
                            hnr[:, 0:w], hp[:, 0:w], bias_sb[:, 10 + oc:11 + oc],
                            r[:, 0:w], op0=ADD, op1=MULT)
                        npre = tmpp.tile([128, NT], F32, tag="r", name="npre")
                        nc.vector.scalar_tensor_tensor(
                            npre[:, 0:w], ip[:, 0:w], bias_sb[:, 8 + oc:9 + oc],
                            hnr[:, 0:w], op0=ADD, op1=ADD)
                        nt_ = tmpp.tile([128, NT], F32, tag="nt", name="nt")
                        nc.scalar.activation(nt_[:, 0:w], npre[:, 0:w], TANH)
                        # x_new = n + z * (summary - n)
                        d = tmpp.tile([128, NT], F32, tag="d", name="d")
                        nc.vector.tensor_sub(d[:, 0:w], sT[:, oc, ls], nt_[:, 0:w])
                        dz = tmpp.tile([128, NT], F32, tag="d", name="dz")
                        nc.vector.tensor_mul(dz[:, 0:w], d[:, 0:w], z[:, 0:w])
                        if last:
                            yt = youtp.tile([128, NT], F32, tag="y", name="yt")
                            nc.vector.tensor_add(yt[:, 0:w], dz[:, 0:w], nt_[:, 0:w])
                            nc.sync.dma_start(y.ap()[:, oc, bs], yt[:, 0:w])
                        else:
                            nc.vector.tensor_add(xf_out[:, oc, bs], dz[:, 0:w],
                                                 nt_[:, 0:w])

                for t in range(T):
                    ts0 = t * NT
                    # neighbor rows: four node-major 2048-idx gathers per
                    # tile (idx i -> neighbor i//128 of node i%128), so each
                    # node's K=16 rows land on ONE partition across the free
                    # chunk axis: reduce = strided DVE free-dim reduce, no
                    # transpose DMAs.  Then scale by 1/cnt (per-partition
                    # scalar) and PE-transpose the 16x-smaller result into
                    # [feat, node] matmul layout.
                    nmean_t = workp.tile([128, 2, NT], BF16, tag="nmean",
                                         name="nmean")
                    for qf in range(4):
                        q0 = qf * 128
                        blk = t * 4 + qf
                        nrm = gathp.tile([128, K, H], BF16, tag="nrm",
                                         name="nrm")
                        nc.gpsimd.dma_gather(
                            nrm[:], xtab[:],
                            nbr_sb[:, t, q0:q0 + 128],
                            NQ, NQ, H, transpose=False, single_packet=False,
                            queue_num=next_q())
                        # contiguous tree-reduce over the chunk axis (a
                        # single strided reduce runs ~0.6 elem/cyc)
                        s1 = workp.tile([128, 8, H], BF16, tag="ts1",
                                        name="ts1")
                        nc.vector.tensor_add(s1[:], nrm[:, 0:8, :],
                                             nrm[:, 8:16, :])
                        s2 = workp.tile([128, 4, H], F32, tag="ts2",
                                        name="ts2")
                        nc.vector.tensor_add(s2[:], s1[:, 0:4, :],
                                             s1[:, 4:8, :])
                        s3 = workp.tile([128, 2, H], F32, tag="ts3",
                                        name="ts3")
                        nc.vector.tensor_add(s3[:], s2[:, 0:2, :],
                                             s2[:, 2:4, :])
                        nsum_rm = workp.tile([128, H], F32, tag="nsr",
                                             name="nsr")
                        nc.vector.tensor_add(nsum_rm[:], s3[:, 0, :],
                                             s3[:, 1, :])
                        nmean_rm = workp.tile([128, H], BF16, tag="nmr",
                                              name="nmr")
                        nc.vector.tensor_scalar_mul(
                            nmean_rm[:], nsum_rm[:],
                            inv_rm_sb[:, blk:blk + 1])
                        for hc in range(2):
                            nps = psp.tile([128, 128], BF16, tag="sum",
                                           name="nps")
                            nc.tensor.transpose(
                                nps[:], nmean_rm[:, hc * 128:(hc + 1) * 128],
                                idb_sb[:])
                            nc.scalar.activation(
                                nmean_t[:, hc, q0:q0 + 128], nps[:], COPY)
                        if t == T - 1 and qf == 1:
                            gru_block(ts0, NT // 2, nmean_t)
                    if t == T - 1:
                        gru_block(ts0 + NT // 2, NT // 2, nmean_t)
                    else:
                        gru_block(ts0, NT, nmean_t)
                    if not last:
                        write_table_tile(xf_out, t)
                if not last:
                    allgather(xtabs[layer + 1])
                cur = 1 - cur

    nc.compile()
    return nc


def _get_nc(depth: int):
    if depth not in _CACHE:
        _CACHE[depth] = _build(depth)
    return _CACHE[depth]


def _idx_layout(lin):
    """linear int16 idx list (len % 16 == 0) -> [128, len//16] wrapped in 16
    partitions, replicated across the 8 gpsimd core groups."""
    v = lin.reshape(-1, 16).T.astype(np.int16)        # [16, len//16]
    return np.tile(v, (8, 1))                         # [128, len//16]


def _chunk2(w):
    """[256, M] -> [128, 2, M] with [p, c, m] = w[c*128+p, m]."""
    M = w.shape[1]
    return np.ascontiguousarray(w.reshape(2, 128, M).transpose(1, 0, 2))


def prepare_inputs(inputs):
    """host-side preprocessing: returns in_maps for the 8 cores."""
    adj = np.asarray(inputs["nodeAdjacencySpecTensor"]).astype(np.int64)
    names = np.asarray(inputs["nodeNamesEncoded"], dtype=np.float32)
    attrs = np.asarray(inputs["nodeAttributesEncoded"], dtype=np.float32)

    parent = adj[:, 0]
    parent = np.clip(np.where(parent < 0, parent + N, parent), 0, N - 1)
    nbr = adj[:, 1:]
    mask = nbr >= 0
    cnt = np.maximum(mask.sum(1), 1).astype(np.float32)
    safe_n = np.where(mask, np.clip(nbr, 0, N - 1), 0).astype(np.int64)
    # node id -> table row in the [8 x SHARD] allgathered layout
    def _row(n):
        return SHARD * (n // NC_REAL) + (n % NC_REAL)
    safe = np.where(mask, _row(safe_n), ZROW).astype(np.int64)
    parent = _row(parent)
    inv = (1.0 / cnt).astype(np.float32)

    feat = np.concatenate([names, attrs], axis=1)      # [N, 256] f32

    W_in = np.asarray(inputs["W_in"], np.float32)
    W_p = np.asarray(inputs["W_parent"], np.float32)
    W_n = np.asarray(inputs["W_neighbor"], np.float32)
    W_ih = np.asarray(inputs["W_ih"], np.float32)
    W_hh = np.asarray(inputs["W_hh"], np.float32)
    b_in = np.asarray(inputs["b_in"], np.float32)
    b_p = np.asarray(inputs["b_parent"], np.float32)
    b_n = np.asarray(inputs["b_neighbor"], np.float32)
    b_ih = np.asarray(inputs["b_ih"], np.float32)
    b_hh = np.asarray(inputs["b_hh"], np.float32)

    w_in_a = _chunk2(W_in.T).astype(BF)                 # [128, 2, 256]
    w_ih_a = _chunk2(W_ih.T).astype(BF)                 # [128, 2, 768]
    w_hh_a = _chunk2(W_hh.T).astype(BF)
    w_p_a = _chunk2(W_p.T).astype(BF)
    w_n_a = _chunk2(W_n.T).astype(BF)

    bias = np.zeros((128, 12), np.float32)
    for col, vec in ((0, b_in), (2, b_p + b_n), (4, (b_ih + b_hh)[0:H]),
                     (6, (b_ih + b_hh)[H:2 * H]), (8, b_ih[2 * H:3 * H]),
                     (10, b_hh[2 * H:3 * H])):
        bias[:, col] = vec[0:128]
        bias[:, col + 1] = vec[128:256]

    ident_b = np.eye(128, dtype=BF)

    shared = dict(w_in=w_in_a, w_ih=w_ih_a, w_hh=w_hh_a, w_p=w_p_a, w_n=w_n_a,
                  biases=bias, ident_b=ident_b)

    in_maps = []
    for c in range(NCORES):
        g0 = c * NC_REAL
        # features, transposed + padded
        f = np.zeros((NCP, DIN), np.float32)
        f[:NC_REAL] = feat[g0:g0 + NC_REAL]
        featT_c = np.ascontiguousarray(
            f.T.reshape(2, 128, NCP).transpose(1, 0, 2)).astype(BF)
        # inv count, node-major blocks: inv_rm[p, b] = 1/cnt[128b + p]
        iv = np.ones(NCP, np.float32)
        iv[:NC_REAL] = inv[g0:g0 + NC_REAL]
        inv_c = np.ascontiguousarray(iv.reshape(NCP // 128, 128).T)
        # indices
        par = np.full(NCP, ZROW, np.int64)
        par[:NC_REAL] = parent[g0:g0 + NC_REAL]
        nbrs = np.full((NCP, K), ZROW, np.int64)
        nbrs[:NC_REAL] = safe[g0:g0 + NC_REAL]
        # node-major gather order: idx i of block b is neighbor i//128 of
        # node 128b + i%128 -> block's linear list = nbrs[block].T.ravel()
        nbr_t = np.zeros((128, T, NT), np.int16)
        for t in range(T):
            blks = [nbrs[t * NT + 128 * q: t * NT + 128 * (q + 1)].T.reshape(-1)
                    for q in range(4)]
            nbr_t[:, t, :] = _idx_layout(np.concatenate(blks))
        par_t = _idx_layout(par)                        # [128, 160]
        in_maps.append(dict(featT=featT_c, invcnt=inv_c, nbr_idx=nbr_t,
                            par_idx=par_t, **shared))
    return in_maps


def run(inputs, trace=False, **kw):
    depth = int(np.asarray(inputs["depth"]))
    nc = _get_nc(depth)
    in_maps = prepare_inputs(inputs)
    res = bass_utils.run_bass_kernel_spmd(nc, in_maps,
                                          core_ids=list(range(NCORES)),
                                          trace=trace, **kw)
    outs = []
    for c in range(NCORES):
        yc = np.asarray(res.results[c]["y"])            # [128, 2, NCP]
        xc = yc.transpose(2, 1, 0).reshape(NCP, H)      # [NCP, 256]
        outs.append(xc[:NC_REAL])
    return np.ascontiguousarray(
        np.concatenate(outs, axis=0).astype(np.float32)), res


def kernel(**inputs) -> np.ndarray:
    out, _ = run(inputs, trace=False)
    return out



# revision 28
# speedup vs baseline: 1.9473x; 1.0605x over previous
"""Trainium2 Bass kernel for nn_NodeInfoPropagate (GNN message passing).

Strategy (8 NeuronCores, node-parallel):
  - Shard the 20000 nodes across 8 cores (2500/core, padded to 2560 = 5 tiles
    of 512).  Weights replicated, all matmul operands bf16 (PSUM accumulates
    f32).
  - Activations live on-chip in "transposed" layout [feature-on-partition,
    node-on-free], so every matmul chains with zero transposes.
  - Per layer, the full x table [N, 256] bf16 is materialized in each core's
    HBM via AllGather (Shared scratchpad); parent + neighbor rows are fetched
    with dma_gather(transpose=False).  Gather descriptor generation runs at
    ~8ns/row on ONE Q7 core pair, so gathers are spread over all 4 SWDGE
    queues (4 Q7 pairs -> ~4x descgen).  transpose=True gathers corrupt each
    other when concurrent (shared XBAR spray state), so rows are gathered
    row-major and flipped to [feat, idx] layout with one HWDGE transpose-DMA
    per gather; those are all issued on the Sync queue (cross-engine
    transpose DMAs also corrupt each other).  All other work (reduces, GRU
    element-wise, activations, PSUM copies, table-write transposes) spreads
    over Vector/Scalar/PE so it hides under the gathers.
  - gather commutes with the linear maps: only the x table is gathered;
    summary = x[par] @ Wp.T + mean_k x[nbr_k] @ Wn.T + (b_p + b_n) accumulates
    in one PSUM bank.  Invalid (-1) neighbors point at an all-zero table row.
  - The final output is written directly in transposed layout [128, 2, NCP]
    f32 and unshuffled on the host (no on-chip output transposes).
"""

import os
import sys

# The tile framework's elide-DMA-wait pass coarsens DMA waits to the
# FIFO-max entry of the issuing ring, scheduled against CoreSim timings.
# On hardware (where SWDGE gathers run ~10x slower than modeled) those
# coarsened waits serialize the gather pipeline — keep precise waits.
os.environ["BACC_ELIDE_DMA_OPT_LIMIT"] = "0"

sys.path.insert(0, "/opt/trn_rl_repo")

import numpy as np
import ml_dtypes

import concourse.bass as bass
import concourse.bacc as bacc
import concourse.tile as tile
import concourse.mybir as mybir
from concourse import bass_utils

N = 20000
K = 16
H = 256
DIN = 256
NCORES = 8
NC_REAL = N // NCORES          # 2500 real nodes per core
NT = 512                       # node tile (matmul free dim / PSUM bank)
T = 5                          # tiles per core
NCP = NT * T                   # 2560 padded nodes per core
SHARD = 2528                   # table shard rows per core (28 zero pad rows)
ZROW = NC_REAL                 # all-zero table row (core0 pad) for invalid nbrs
NTAB = SHARD * NCORES          # 20224 table rows
NQ = (NT // 4) * K             # 2048 neighbor idxs per quarter-tile

F32 = mybir.dt.float32
BF16 = mybir.dt.bfloat16
I16 = mybir.dt.int16
BF = ml_dtypes.bfloat16

_CACHE = {}


def _build(depth: int):
    nc = bacc.Bacc("TRN2", target_bir_lowering=False, debug=False,
                   num_devices=NCORES, num_swdge_queues=4)

    featT = nc.dram_tensor("featT", [128, 2, NCP], BF16, kind="ExternalInput")
    invcnt = nc.dram_tensor("invcnt", [128, NCP // 128], F32,
                            kind="ExternalInput")
    nbr_idx = nc.dram_tensor("nbr_idx", [128, T, NT], I16, kind="ExternalInput")
    par_idx = nc.dram_tensor("par_idx", [128, NCP // 16], I16, kind="ExternalInput")
    w_in = nc.dram_tensor("w_in", [128, 2, H], BF16, kind="ExternalInput")
    w_ih = nc.dram_tensor("w_ih", [128, 2, 3 * H], BF16, kind="ExternalInput")
    w_hh = nc.dram_tensor("w_hh", [128, 2, 3 * H], BF16, kind="ExternalInput")
    w_p = nc.dram_tensor("w_p", [128, 2, H], BF16, kind="ExternalInput")
    w_n = nc.dram_tensor("w_n", [128, 2, H], BF16, kind="ExternalInput")
    # bias columns: 0-1 b_in, 2-3 b_p+b_n, 4-5 b_r, 6-7 b_z, 8-9 b_ih_n,
    # 10-11 b_hh_n  (per 128-feature chunk)
    biases = nc.dram_tensor("biases", [128, 12], F32, kind="ExternalInput")
    ident_b = nc.dram_tensor("ident_b", [128, 128], BF16, kind="ExternalInput")
    y = nc.dram_tensor("y", [128, 2, NCP], F32, kind="ExternalOutput")

    SIG = mybir.ActivationFunctionType.Sigmoid
    TANH = mybir.ActivationFunctionType.Tanh
    COPY = mybir.ActivationFunctionType.Copy
    ADD = mybir.AluOpType.add
    MULT = mybir.AluOpType.mult

    with tile.TileContext(nc) as tc:
        with (
            tc.tile_pool(name="const", bufs=1) as constp,
            tc.tile_pool(name="state", bufs=1) as statep,
            tc.tile_pool(name="dram", bufs=1, space="DRAM") as dramp,
            tc.tile_pool(name="pgat", bufs=2) as pgatp,
            tc.tile_pool(name="prm", bufs=1) as prmp,
            tc.tile_pool(name="gath", bufs=8) as gathp,
            tc.tile_pool(name="work", bufs=2) as workp,
            tc.tile_pool(name="tmp", bufs=2) as tmpp,
            tc.tile_pool(name="yout", bufs=2) as youtp,
            tc.tile_pool(name="ps", bufs=2, space="PSUM") as psp,
            tc.tile_pool(name="psg", bufs=6, space="PSUM") as psgp,
        ):
            # ---- resident constants -------------------------------------
            win_sb = constp.tile([128, 2, H], BF16, name="win_sb")
            nc.sync.dma_start(win_sb[:], w_in.ap())
            wih_sb = constp.tile([128, 2, 3 * H], BF16, name="wih_sb")
            nc.sync.dma_start(wih_sb[:], w_ih.ap())
            whh_sb = constp.tile([128, 2, 3 * H], BF16, name="whh_sb")
            nc.sync.dma_start(whh_sb[:], w_hh.ap())
            wp_sb = constp.tile([128, 2, H], BF16, name="wp_sb")
            nc.sync.dma_start(wp_sb[:], w_p.ap())
            wn_sb = constp.tile([128, 2, H], BF16, name="wn_sb")
            nc.sync.dma_start(wn_sb[:], w_n.ap())
            bias_sb = constp.tile([128, 12], F32, name="bias_sb")
            nc.sync.dma_start(bias_sb[:], biases.ap())
            idb_sb = constp.tile([128, 128], BF16, name="idb_sb")
            nc.sync.dma_start(idb_sb[:], ident_b.ap())
            feat_sb = constp.tile([128, 2, NCP], BF16, name="feat_sb")
            nc.sync.dma_start(feat_sb[:], featT.ap())
            inv_rm_sb = constp.tile([128, NCP // 128], F32, name="inv_rm_sb")
            nc.sync.dma_start(inv_rm_sb[:], invcnt.ap())
            nbr_sb = constp.tile([128, T, NT], I16, name="nbr_sb")
            nc.sync.dma_start(nbr_sb[:], nbr_idx.ap())
            par_sb = constp.tile([128, NCP // 16], I16, name="par_sb")
            nc.sync.dma_start(par_sb[:], par_idx.ap())

            xF = [statep.tile([128, 2, NCP], BF16, name=f"xF{i}") for i in range(2)]
            qctr = [0]  # SWDGE queue round-robin across all gathers

            def next_q():
                q = qctr[0] % 4
                qctr[0] += 1
                return q

            xloc = dramp.tile([SHARD, H], BF16, name="xloc")
            ntabs = max(depth, 1)
            xtabs = [dramp.tile([NTAB, H], BF16, name=f"xtab{i}",
                                addr_space="Shared") for i in range(ntabs)]

            # zero rows (shard pad; serve as invalid-neighbor targets)
            zero_sb = constp.tile([128, H], BF16, name="zero_sb")
            nc.vector.memset(zero_sb[:], 0.0)

            def write_table_tile(xf, t):
                """transpose tile t of xf (bf16) to row-major, one batched DMA
                to xloc (plus a partial-block DMA on the last tile)."""
                ts0 = t * NT
                nb = NT // 128
                rm = workp.tile([128, nb, H], BF16, tag="rm", name="rm")
                nfull = min(NT, NC_REAL - ts0) // 128      # full 128-row blocks
                nblk = nb if ts0 + NT <= NC_REAL else nfull + 1
                for b in range(nblk):
                    for c in range(2):
                        pst = psp.tile([128, 128], BF16, tag="sum", name="pst")
                        nc.tensor.transpose(
                            pst[:], xf[:, c, ts0 + b * 128:ts0 + (b + 1) * 128],
                            idb_sb[:])
                        nc.scalar.activation(rm[:, b, c * 128:(c + 1) * 128],
                                             pst[:], COPY)
                if nfull > 0:
                    nc.sync.dma_start(
                        xloc[ts0:ts0 + nfull * 128, :].rearrange(
                            "(b p) f -> p b f", p=128),
                        rm[:, 0:nfull, :])
                rem = min(NT, NC_REAL - ts0) - nfull * 128  # partial tail rows
                if rem > 0:
                    r0 = ts0 + nfull * 128
                    nc.sync.dma_start(xloc[r0:r0 + rem, :],
                                      rm[0:rem, nfull, :])

            def zero_pad_rows():
                nc.sync.dma_start(xloc[NC_REAL:SHARD, :],
                                  zero_sb[0:SHARD - NC_REAL, :])

            def allgather(xtab, r0=0, r1=SHARD):
                # chunked: gather shard rows [r0, r1) from every core into
                # the strided row-bands of the table
                xt_view = xtab[0:NTAB, :].rearrange("(c r) f -> c r f",
                                                    r=SHARD)
                nc.gpsimd.collective_compute(
                    "AllGather", mybir.AluOpType.bypass,
                    replica_groups=[list(range(NCORES))],
                    ins=[xloc[r0:r1, :].opt()],
                    outs=[xt_view[:, r0:r1, :].opt()],
                )

            # ---- layer 0: x0 = W_in @ feat + b_in ------------------------
            for t in range(T):
                ts = slice(t * NT, (t + 1) * NT)
                for oc in range(2):
                    ps = psp.tile([128, NT], F32, tag="sum", name="ps0")
                    for dc in range(2):
                        nc.tensor.matmul(ps[:], win_sb[:, dc, oc * 128:(oc + 1) * 128],
                                         feat_sb[:, dc, ts],
                                         start=(dc == 0), stop=(dc == 1))
                    if depth == 0:
                        yt = youtp.tile([128, NT], F32, tag="y", name="yt0")
                        nc.vector.tensor_scalar_add(yt[:], ps[:],
                                                    bias_sb[:, oc:oc + 1])
                        nc.sync.dma_start(y.ap()[:, oc, ts], yt[:])
                    else:
                        nc.vector.tensor_scalar_add(xF[0][:, oc, ts], ps[:],
                                                    bias_sb[:, oc:oc + 1])
                if depth > 0:
                    write_table_tile(xF[0], t)
            if depth > 0:
                zero_pad_rows()
                allgather(xtabs[0])

            # ---- GRU layers ---------------------------------------------
            cur = 0
            for layer in range(depth):
                last = layer == depth - 1
                xf_in, xf_out = xF[cur], xF[1 - cur]
                xtab = xtabs[layer]
                # one merged parent gather for the whole layer; row-major
                # node-major blocks [128 nodes, 256 feat], PE-transposed
                # per block into [feat, node] matmul layout
                prm = prmp.tile([128, NCP // 128, H], BF16, tag="prm", name="prm")
                nc.gpsimd.dma_gather(prm[:], xtab[:], par_sb[:],
                                     NCP, NCP, H, transpose=False,
                                     single_packet=False, queue_num=next_q())
                pgat = pgatp.tile([128, 2, NCP], BF16, tag="pgat", name="pgat")
                for blk in range(NCP // 128):
                    for hc in range(2):
                        pps = psp.tile([128, 128], BF16, tag="sum", name="pps")
                        nc.tensor.transpose(
                            pps[:], prm[:, blk, hc * 128:(hc + 1) * 128],
                            idb_sb[:])
                        nc.scalar.activation(
                            pgat[:, hc, blk * 128:(blk + 1) * 128], pps[:],
                            COPY)
                def gru_block(c0, w, nmean):
                    """summary + GRU for columns [c0, c0+w); nmean covers the
                    owning tile, sliced locally."""
                    bs = slice(c0, c0 + w)
                    t0 = c0 - (c0 % NT)
                    ls = slice(c0 - t0, c0 - t0 + w)
                    # summary = pgat @ Wp.T + nmean @ Wn.T + (b_p + b_n)
                    sT = workp.tile([128, 2, NT], BF16, tag="sT", name="sT")
                    for oc in range(2):
                        ps = psp.tile([128, NT], F32, tag="sum", name="psS")
                        for hc in range(2):
                            nc.tensor.matmul(ps[:, 0:w],
                                             wp_sb[:, hc, oc * 128:(oc + 1) * 128],
                                             pgat[:, hc, bs],
                                             start=(hc == 0), stop=False)
                        for hc in range(2):
                            nc.tensor.matmul(ps[:, 0:w],
                                             wn_sb[:, hc, oc * 128:(oc + 1) * 128],
                                             nmean[:, hc, ls],
                                             start=False, stop=(hc == 1))
                        nc.vector.tensor_scalar_add(sT[:, oc, ls], ps[:, 0:w],
                                                    bias_sb[:, 2 + oc:3 + oc])
                    # GRU gates, per output chunk
                    for oc in range(2):
                        rp = psgp.tile([128, NT], F32, tag="gate", name="rp")
                        zp = psgp.tile([128, NT], F32, tag="gate", name="zp")
                        ip = psgp.tile([128, NT], F32, tag="gate", name="ip")
                        hp = psgp.tile([128, NT], F32, tag="gate", name="hp")
                        for gate, pst in ((0, rp), (1, zp)):
                            o0 = gate * H + oc * 128
                            for hc in range(2):
                                nc.tensor.matmul(pst[:, 0:w],
                                                 wih_sb[:, hc, o0:o0 + 128],
                                                 xf_in[:, hc, bs],
                                                 start=(hc == 0), stop=False)
                            for hc in range(2):
                                nc.tensor.matmul(pst[:, 0:w],
                                                 whh_sb[:, hc, o0:o0 + 128],
                                                 sT[:, hc, ls],
                                                 start=False, stop=(hc == 1))
                        o0 = 2 * H + oc * 128
                        for hc in range(2):
                            nc.tensor.matmul(ip[:, 0:w], wih_sb[:, hc, o0:o0 + 128],
                                             xf_in[:, hc, bs],
                                             start=(hc == 0), stop=(hc == 1))
                        for hc in range(2):
                            nc.tensor.matmul(hp[:, 0:w], whh_sb[:, hc, o0:o0 + 128],
                                             sT[:, hc, ls],
                                             start=(hc == 0), stop=(hc == 1))
                        r = tmpp.tile([128, NT], F32, tag="r", name="r")
                        nc.scalar.activation(r[:, 0:w], rp[:, 0:w], SIG,
                                             bias=bias_sb[:, 4 + oc:5 + oc])
                        z = tmpp.tile([128, NT], F32, tag="z", name="z")
                        nc.scalar.activation(z[:, 0:w], zp[:, 0:w], SIG,
                                             bias=bias_sb[:, 6 + oc:7 + oc])
                        # n = tanh((i_n + b_ih_n) + r * (h_n + b_hh_n))
                        hnr = tmpp.tile([128, NT], F32, tag="hnr", name="hnr")
                        nc.vector.scalar_tensor_tensor(
                            hnr[:, 0:w], hp[:, 0:w], bias_sb[:, 10 + oc:11 + oc],
                            r[:, 0:w], op0=ADD, op1=MULT)
                        npre = tmpp.tile([128, NT], F32, tag="r", name="npre")
                        nc.vector.scalar_tensor_tensor(
                            npre[:, 0:w], ip[:, 0:w], bias_sb[:, 8 + oc:9 + oc],
                            hnr[:, 0:w], op0=ADD, op1=ADD)
                        nt_ = tmpp.tile([128, NT], F32, tag="nt", name="nt")
                        nc.scalar.activation(nt_[:, 0:w], npre[:, 0:w], TANH)
                        # x_new = n + z * (summary - n)
                        d = tmpp.tile([128, NT], F32, tag="d", name="d")
                        nc.vector.tensor_sub(d[:, 0:w], sT[:, oc, ls], nt_[:, 0:w])
                        dz = tmpp.tile([128, NT], F32, tag="d", name="dz")
                        nc.vector.tensor_mul(dz[:, 0:w], d[:, 0:w], z[:, 0:w])
                        if last:
                            yt = youtp.tile([128, NT], F32, tag="y", name="yt")
                            nc.vector.tensor_add(yt[:, 0:w], dz[:, 0:w], nt_[:, 0:w])
                            nc.sync.dma_start(y.ap()[:, oc, bs], yt[:, 0:w])
                        else:
                            nc.vector.tensor_add(xf_out[:, oc, bs], dz[:, 0:w],
                                                 nt_[:, 0:w])

                for t in range(T):
                    ts0 = t * NT
                    # neighbor rows: four node-major 2048-idx gathers per
                    # tile (idx i -> neighbor i//128 of node i%128), so each
                    # node's K=16 rows land on ONE partition across the free
                    # chunk axis: reduce = strided DVE free-dim reduce, no
                    # transpose DMAs.  Then scale by 1/cnt (per-partition
                    # scalar) and PE-transpose the 16x-smaller result into
                    # [feat, node] matmul layout.
                    nmean_t = workp.tile([128, 2, NT], BF16, tag="nmean",
                                         name="nmean")
                    for qf in range(4):
                        q0 = qf * 128
                        blk = t * 4 + qf
                        nrm = gathp.tile([128, K, H], BF16, tag="nrm",
                                         name="nrm")
                        nc.gpsimd.dma_gather(
                            nrm[:], xtab[:],
                            nbr_sb[:, t, q0:q0 + 128],
                            NQ, NQ, H, transpose=False, single_packet=False,
                            queue_num=next_q())
                        # contiguous tree-reduce over the chunk axis (a
                        # single strided reduce runs ~0.6 elem/cyc)
                        s1 = workp.tile([128, 8, H], BF16, tag="ts1",
                                        name="ts1")
                        nc.vector.tensor_add(s1[:], nrm[:, 0:8, :],
                                             nrm[:, 8:16, :])
                        s2 = workp.tile([128, 4, H], F32, tag="ts2",
                                        name="ts2")
                        nc.vector.tensor_add(s2[:], s1[:, 0:4, :],
                                             s1[:, 4:8, :])
                        s3 = workp.tile([128, 2, H], F32, tag="ts3",
                                        name="ts3")
                        nc.vector.tensor_add(s3[:], s2[:, 0:2, :],
                                             s2[:, 2:4, :])
                        nsum_rm = workp.tile([128, H], F32, tag="nsr",
                                             name="nsr")
                        nc.vector.tensor_add(nsum_rm[:], s3[:, 0, :],
                                             s3[:, 1, :])
                        nmean_rm = workp.tile([128, H], BF16, tag="nmr",
                                              name="nmr")
                        nc.vector.tensor_scalar_mul(
                            nmean_rm[:], nsum_rm[:],
                            inv_rm_sb[:, blk:blk + 1])
                        for hc in range(2):
                            nps = psp.tile([128, 128], BF16, tag="sum",
                                           name="nps")
                            nc.tensor.transpose(
                                nps[:], nmean_rm[:, hc * 128:(hc + 1) * 128],
                                idb_sb[:])
                            nc.scalar.activation(
                                nmean_t[:, hc, q0:q0 + 128], nps[:], COPY)
                        if t == T - 1 and qf == 1:
                            gru_block(ts0, NT // 2, nmean_t)
                    if t == T - 1:
                        gru_block(ts0 + NT // 2, NT // 2, nmean_t)
                    else:
                        gru_block(ts0, NT, nmean_t)
                    if not last:
                        write_table_tile(xf_out, t)
                if not last:
                    allgather(xtabs[layer + 1])
                cur = 1 - cur

    nc.compile()
    return nc


def _get_nc(depth: int):
    if depth not in _CACHE:
        _CACHE[depth] = _build(depth)
    return _CACHE[depth]


def _idx_layout(lin):
    """linear int16 idx list (len % 16 == 0) -> [128, len//16] wrapped in 16
    partitions, replicated across the 8 gpsimd core groups."""
    v = lin.reshape(-1, 16).T.astype(np.int16)        # [16, len//16]
    return np.tile(v, (8, 1))                         # [128, len//16]


def _chunk2(w):
    """[256, M] -> [128, 2, M] with [p, c, m] = w[c*128+p, m]."""
    M = w.shape[1]
    return np.ascontiguousarray(w.reshape(2, 128, M).transpose(1, 0, 2))


def prepare_inputs(inputs):
    """host-side preprocessing: returns in_maps for the 8 cores."""
    adj = np.asarray(inputs["nodeAdjacencySpecTensor"]).astype(np.int64)
    names = np.asarray(inputs["nodeNamesEncoded"], dtype=np.float32)
    attrs = np.asarray(inputs["nodeAttributesEncoded"], dtype=np.float32)

    parent = adj[:, 0]
    parent = np.clip(np.where(parent < 0, parent + N, parent), 0, N - 1)
    nbr = adj[:, 1:]
    mask = nbr >= 0
    cnt = np.maximum(mask.sum(1), 1).astype(np.float32)
    safe_n = np.where(mask, np.clip(nbr, 0, N - 1), 0).astype(np.int64)
    # node id -> table row in the [8 x SHARD] allgathered layout
    def _row(n):
        return SHARD * (n // NC_REAL) + (n % NC_REAL)
    safe = np.where(mask, _row(safe_n), ZROW).astype(np.int64)
    parent = _row(parent)
    inv = (1.0 / cnt).astype(np.float32)

    feat = np.concatenate([names, attrs], axis=1)      # [N, 256] f32

    W_in = np.asarray(inputs["W_in"], np.float32)
    W_p = np.asarray(inputs["W_parent"], np.float32)
    W_n = np.asarray(inputs["W_neighbor"], np.float32)
    W_ih = np.asarray(inputs["W_ih"], np.float32)
    W_hh = np.asarray(inputs["W_hh"], np.float32)
    b_in = np.asarray(inputs["b_in"], np.float32)
    b_p = np.asarray(inputs["b_parent"], np.float32)
    b_n = np.asarray(inputs["b_neighbor"], np.float32)
    b_ih = np.asarray(inputs["b_ih"], np.float32)
    b_hh = np.asarray(inputs["b_hh"], np.float32)

    w_in_a = _chunk2(W_in.T).astype(BF)                 # [128, 2, 256]
    w_ih_a = _chunk2(W_ih.T).astype(BF)                 # [128, 2, 768]
    w_hh_a = _chunk2(W_hh.T).astype(BF)
    w_p_a = _chunk2(W_p.T).astype(BF)
    w_n_a = _chunk2(W_n.T).astype(BF)

    bias = np.zeros((128, 12), np.float32)
    for col, vec in ((0, b_in), (2, b_p + b_n), (4, (b_ih + b_hh)[0:H]),
                     (6, (b_ih + b_hh)[H:2 * H]), (8, b_ih[2 * H:3 * H]),
                     (10, b_hh[2 * H:3 * H])):
        bias[:, col] = vec[0:128]
        bias[:, col + 1] = vec[128:256]

    ident_b = np.eye(128, dtype=BF)

    shared = dict(w_in=w_in_a, w_ih=w_ih_a, w_hh=w_hh_a, w_p=w_p_a, w_n=w_n_a,
                  biases=bias, ident_b=ident_b)

    in_maps = []
    for c in range(NCORES):
        g0 = c * NC_REAL
        # features, transposed + padded
        f = np.zeros((NCP, DIN), np.float32)
        f[:NC_REAL] = feat[g0:g0 + NC_REAL]
        featT_c = np.ascontiguousarray(
            f.T.reshape(2, 128, NCP).transpose(1, 0, 2)).astype(BF)
        # inv count, node-major blocks: inv_rm[p, b] = 1/cnt[128b + p]
        iv = np.ones(NCP, np.float32)
        iv[:NC_REAL] = inv[g0:g0 + NC_REAL]
        inv_c = np.ascontiguousarray(iv.reshape(NCP // 128, 128).T)
        # indices
        par = np.full(NCP, ZROW, np.int64)
        par[:NC_REAL] = parent[g0:g0 + NC_REAL]
        nbrs = np.full((NCP, K), ZROW, np.int64)
        nbrs[:NC_REAL] = safe[g0:g0 + NC_REAL]
        # node-major gather order: idx i of block b is neighbor i//128 of
        # node 128b + i%128 -> block's linear list = nbrs[block].T.ravel()
        nbr_t = np.zeros((128, T, NT), np.int16)
        for t in range(T):
            blks = [nbrs[t * NT + 128 * q: t * NT + 128 * (q + 1)].T.reshape(-1)
                    for q in range(4)]
            nbr_t[:, t, :] = _idx_layout(np.concatenate(blks))
        par_t = _idx_layout(par)                        # [128, 160]
        in_maps.append(dict(featT=featT_c, invcnt=inv_c, nbr_idx=nbr_t,
                            par_idx=par_t, **shared))
    return in_maps


def run(inputs, trace=False, **kw):
    depth = int(np.asarray(inputs["depth"]))
    nc = _get_nc(depth)
    in_maps = prepare_inputs(inputs)
    res = bass_utils.run_bass_kernel_spmd(nc, in_maps,
                                          core_ids=list(range(NCORES)),
                                          trace=trace, **kw)
    outs = []
    for c in range(NCORES):
        yc = np.asarray(res.results[c]["y"])            # [128, 2, NCP]
        xc = yc.transpose(2, 1, 0).reshape(NCP, H)      # [NCP, 256]
        outs.append(xc[:NC_REAL])
    return np.ascontiguousarray(
        np.concatenate(outs, axis=0).astype(np.float32)), res


def kernel(**inputs) -> np.ndarray:
    out, _ = run(inputs, trace=False)
    return out



# revision 33
# speedup vs baseline: 2.3142x; 1.1884x over previous
"""Trainium2 Bass kernel for nn_NodeInfoPropagate (GNN message passing).

Strategy (8 NeuronCores, node-parallel):
  - Shard the 20000 nodes across 8 cores (2500/core, padded to 2560 = 5 tiles
    of 512).  Weights replicated, all matmul operands bf16 (PSUM accumulates
    f32).
  - Activations live on-chip in "transposed" layout [feature-on-partition,
    node-on-free], so every matmul chains with zero transposes.
  - Per layer, the full x table [N, 256] bf16 is materialized in each core's
    HBM via AllGather (Shared scratchpad); parent + neighbor rows are fetched
    with dma_gather(transpose=False).  Gather descriptor generation runs at
    ~8ns/row on ONE Q7 core pair, so gathers are spread over all 4 SWDGE
    queues (4 Q7 pairs -> ~4x descgen).  transpose=True gathers corrupt each
    other when concurrent (shared XBAR spray state), so rows are gathered
    row-major and flipped to [feat, idx] layout with one HWDGE transpose-DMA
    per gather; those are all issued on the Sync queue (cross-engine
    transpose DMAs also corrupt each other).  All other work (reduces, GRU
    element-wise, activations, PSUM copies, table-write transposes) spreads
    over Vector/Scalar/PE so it hides under the gathers.
  - gather commutes with the linear maps: only the x table is gathered;
    summary = x[par] @ Wp.T + mean_k x[nbr_k] @ Wn.T + (b_p + b_n) accumulates
    in one PSUM bank.  Invalid (-1) neighbors point at an all-zero table row.
  - The final output is written directly in transposed layout [128, 2, NCP]
    f32 and unshuffled on the host (no on-chip output transposes).
"""

import os
import sys

# The tile framework's elide-DMA-wait pass coarsens DMA waits to the
# FIFO-max entry of the issuing ring, scheduled against CoreSim timings.
# On hardware (where SWDGE gathers run ~10x slower than modeled) those
# coarsened waits serialize the gather pipeline — keep precise waits.
os.environ["BACC_ELIDE_DMA_OPT_LIMIT"] = "0"

sys.path.insert(0, "/opt/trn_rl_repo")

import numpy as np
import ml_dtypes

import concourse.bass as bass
import concourse.bacc as bacc
import concourse.tile as tile
import concourse.mybir as mybir
from concourse import bass_utils

N = 20000
K = 16
H = 256
DIN = 256
NCORES = 8
NC_REAL = N // NCORES          # 2500 real nodes per core
NT = 512                       # node tile (matmul free dim / PSUM bank)
T = 5                          # tiles per core
NCP = NT * T                   # 2560 padded nodes per core
SHARD = 2528                   # table shard rows per core (28 zero pad rows)
ZROW = NC_REAL                 # all-zero table row (core0 pad) for invalid nbrs
NTAB = SHARD * NCORES          # 20224 table rows
NQ = (NT // 4) * K             # 2048 neighbor idxs per quarter-tile

F32 = mybir.dt.float32
BF16 = mybir.dt.bfloat16
I16 = mybir.dt.int16
BF = ml_dtypes.bfloat16

_CACHE = {}


def _build(depth: int):
    nc = bacc.Bacc("TRN2", target_bir_lowering=False, debug=False,
                   num_devices=NCORES, num_swdge_queues=4)

    featT = nc.dram_tensor("featT", [128, 2, NCP], BF16, kind="ExternalInput")
    # per-node-block diagonal 1/cnt matrices: transposing a node-major
    # [node, feat] block with rhs=diag(inv) scales each node column for free
    invcnt = nc.dram_tensor("invcnt", [128, NCP // 128, 128], BF16,
                            kind="ExternalInput")
    nbr_idx = nc.dram_tensor("nbr_idx", [128, T, NT], I16, kind="ExternalInput")
    par_idx = nc.dram_tensor("par_idx", [128, NCP // 16], I16, kind="ExternalInput")
    w_in = nc.dram_tensor("w_in", [128, 2, H], BF16, kind="ExternalInput")
    w_ih = nc.dram_tensor("w_ih", [128, 2, 3 * H], BF16, kind="ExternalInput")
    w_hh = nc.dram_tensor("w_hh", [128, 2, 3 * H], BF16, kind="ExternalInput")
    w_p = nc.dram_tensor("w_p", [128, 2, H], BF16, kind="ExternalInput")
    w_n = nc.dram_tensor("w_n", [128, 2, H], BF16, kind="ExternalInput")
    # bias columns: 0-1 b_in, 2-3 b_p+b_n, 4-5 b_r, 6-7 b_z, 8-9 b_ih_n,
    # 10-11 b_hh_n  (per 128-feature chunk)
    biases = nc.dram_tensor("biases", [128, 12], F32, kind="ExternalInput")
    ident_b = nc.dram_tensor("ident_b", [128, 128], BF16, kind="ExternalInput")
    y = nc.dram_tensor("y", [128, 2, NCP], F32, kind="ExternalOutput")

    SIG = mybir.ActivationFunctionType.Sigmoid
    TANH = mybir.ActivationFunctionType.Tanh
    COPY = mybir.ActivationFunctionType.Copy
    ADD = mybir.AluOpType.add
    MULT = mybir.AluOpType.mult

    with tile.TileContext(nc) as tc:
        with (
            tc.tile_pool(name="const", bufs=1) as constp,
            tc.tile_pool(name="state", bufs=1) as statep,
            tc.tile_pool(name="dram", bufs=1, space="DRAM") as dramp,
            tc.tile_pool(name="pgat", bufs=2) as pgatp,
            tc.tile_pool(name="prm", bufs=1) as prmp,
            tc.tile_pool(name="gath", bufs=8) as gathp,
            tc.tile_pool(name="work", bufs=2) as workp,
            tc.tile_pool(name="tmp", bufs=2) as tmpp,
            tc.tile_pool(name="yout", bufs=2) as youtp,
            tc.tile_pool(name="ps", bufs=2, space="PSUM") as psp,
            tc.tile_pool(name="psg", bufs=6, space="PSUM") as psgp,
        ):
            # ---- resident constants -------------------------------------
            win_sb = constp.tile([128, 2, H], BF16, name="win_sb")
            nc.sync.dma_start(win_sb[:], w_in.ap())
            wih_sb = constp.tile([128, 2, 3 * H], BF16, name="wih_sb")
            nc.sync.dma_start(wih_sb[:], w_ih.ap())
            whh_sb = constp.tile([128, 2, 3 * H], BF16, name="whh_sb")
            nc.sync.dma_start(whh_sb[:], w_hh.ap())
            wp_sb = constp.tile([128, 2, H], BF16, name="wp_sb")
            nc.sync.dma_start(wp_sb[:], w_p.ap())
            wn_sb = constp.tile([128, 2, H], BF16, name="wn_sb")
            nc.sync.dma_start(wn_sb[:], w_n.ap())
            bias_sb = constp.tile([128, 12], F32, name="bias_sb")
            nc.sync.dma_start(bias_sb[:], biases.ap())
            idb_sb = constp.tile([128, 128], BF16, name="idb_sb")
            nc.sync.dma_start(idb_sb[:], ident_b.ap())
            feat_sb = constp.tile([128, 2, NCP], BF16, name="feat_sb")
            nc.sync.dma_start(feat_sb[:], featT.ap())
            dinv_sb = constp.tile([128, NCP // 128, 128], BF16, name="dinv_sb")
            nc.sync.dma_start(dinv_sb[:], invcnt.ap())
            nbr_sb = constp.tile([128, T, NT], I16, name="nbr_sb")
            nc.sync.dma_start(nbr_sb[:], nbr_idx.ap())
            par_sb = constp.tile([128, NCP // 16], I16, name="par_sb")
            nc.sync.dma_start(par_sb[:], par_idx.ap())

            xF = [statep.tile([128, 2, NCP], BF16, name=f"xF{i}") for i in range(2)]
            qctr = [0]  # SWDGE queue round-robin across all gathers

            def next_q():
                q = qctr[0] % 4
                qctr[0] += 1
                return q

            xloc = dramp.tile([SHARD, H], BF16, name="xloc")
            ntabs = max(depth, 1)
            xtabs = [dramp.tile([NTAB, H], BF16, name=f"xtab{i}",
                                addr_space="Shared") for i in range(ntabs)]

            # zero rows (shard pad; serve as invalid-neighbor targets)
            zero_sb = constp.tile([128, H], BF16, name="zero_sb")
            nc.vector.memset(zero_sb[:], 0.0)

            def write_table_tile(xf, t):
                """transpose tile t of xf (bf16) to row-major, one batched DMA
                to xloc (plus a partial-block DMA on the last tile)."""
                ts0 = t * NT
                nb = NT // 128
                rm = workp.tile([128, nb, H], BF16, tag="rm", name="rm")
                nfull = min(NT, NC_REAL - ts0) // 128      # full 128-row blocks
                nblk = nb if ts0 + NT <= NC_REAL else nfull + 1
                for b in range(nblk):
                    for c in range(2):
                        pst = psp.tile([128, 128], BF16, tag="sum", name="pst")
                        nc.tensor.transpose(
                            pst[:], xf[:, c, ts0 + b * 128:ts0 + (b + 1) * 128],
                            idb_sb[:])
                        nc.scalar.activation(rm[:, b, c * 128:(c + 1) * 128],
                                             pst[:], COPY)
                if nfull > 0:
                    nc.sync.dma_start(
                        xloc[ts0:ts0 + nfull * 128, :].rearrange(
                            "(b p) f -> p b f", p=128),
                        rm[:, 0:nfull, :])
                rem = min(NT, NC_REAL - ts0) - nfull * 128  # partial tail rows
                if rem > 0:
                    r0 = ts0 + nfull * 128
                    nc.sync.dma_start(xloc[r0:r0 + rem, :],
                                      rm[0:rem, nfull, :])

            def zero_pad_rows():
                nc.sync.dma_start(xloc[NC_REAL:SHARD, :],
                                  zero_sb[0:SHARD - NC_REAL, :])

            def allgather(xtab, r0=0, r1=SHARD):
                # chunked: gather shard rows [r0, r1) from every core into
                # the strided row-bands of the table
                xt_view = xtab[0:NTAB, :].rearrange("(c r) f -> c r f",
                                                    r=SHARD)
                nc.gpsimd.collective_compute(
                    "AllGather", mybir.AluOpType.bypass,
                    replica_groups=[list(range(NCORES))],
                    ins=[xloc[r0:r1, :].opt()],
                    outs=[xt_view[:, r0:r1, :].opt()],
                )

            # ---- layer 0: x0 = W_in @ feat + b_in ------------------------
            for t in range(T):
                ts = slice(t * NT, (t + 1) * NT)
                for oc in range(2):
                    ps = psp.tile([128, NT], F32, tag="sum", name="ps0")
                    for dc in range(2):
                        nc.tensor.matmul(ps[:], win_sb[:, dc, oc * 128:(oc + 1) * 128],
                                         feat_sb[:, dc, ts],
                                         start=(dc == 0), stop=(dc == 1))
                    if depth == 0:
                        yt = youtp.tile([128, NT], F32, tag="y", name="yt0")
                        nc.vector.tensor_scalar_add(yt[:], ps[:],
                                                    bias_sb[:, oc:oc + 1])
                        nc.sync.dma_start(y.ap()[:, oc, ts], yt[:])
                    else:
                        nc.vector.tensor_scalar_add(xF[0][:, oc, ts], ps[:],
                                                    bias_sb[:, oc:oc + 1])
                if depth > 0:
                    write_table_tile(xF[0], t)
            if depth > 0:
                zero_pad_rows()
                allgather(xtabs[0])

            # ---- GRU layers ---------------------------------------------
            cur = 0
            for layer in range(depth):
                last = layer == depth - 1
                xf_in, xf_out = xF[cur], xF[1 - cur]
                xtab = xtabs[layer]
                # one merged parent gather for the whole layer; row-major
                # node-major blocks [128 nodes, 256 feat], PE-transposed
                # per block into [feat, node] matmul layout
                prm = prmp.tile([128, NCP // 128, H], BF16, tag="prm", name="prm")
                nc.gpsimd.dma_gather(prm[:], xtab[:], par_sb[:],
                                     NCP, NCP, H, transpose=False,
                                     single_packet=False, queue_num=next_q())
                pgat = pgatp.tile([128, 2, NCP], BF16, tag="pgat", name="pgat")
                for blk in range(NCP // 128):
                    for hc in range(2):
                        pps = psp.tile([128, 128], BF16, tag="sum", name="pps")
                        nc.tensor.transpose(
                            pps[:], prm[:, blk, hc * 128:(hc + 1) * 128],
                            idb_sb[:])
                        nc.scalar.activation(
                            pgat[:, hc, blk * 128:(blk + 1) * 128], pps[:],
                            COPY)
                def gru_block(c0, w, nmean):
                    """summary + GRU for columns [c0, c0+w); nmean covers the
                    owning tile, sliced locally."""
                    bs = slice(c0, c0 + w)
                    t0 = c0 - (c0 % NT)
                    ls = slice(c0 - t0, c0 - t0 + w)
                    # summary = pgat @ Wp.T + nmean @ Wn.T + (b_p + b_n)
                    sT = workp.tile([128, 2, NT], BF16, tag="sT", name="sT")
                    for oc in range(2):
                        ps = psp.tile([128, NT], F32, tag="sum", name="psS")
                        for hc in range(2):
                            nc.tensor.matmul(ps[:, 0:w],
                                             wp_sb[:, hc, oc * 128:(oc + 1) * 128],
                                             pgat[:, hc, bs],
                                             start=(hc == 0), stop=False)
                        for hc in range(2):
                            nc.tensor.matmul(ps[:, 0:w],
                                             wn_sb[:, hc, oc * 128:(oc + 1) * 128],
                                             nmean[:, hc, ls],
                                             start=False, stop=(hc == 1))
                        nc.vector.tensor_scalar_add(sT[:, oc, ls], ps[:, 0:w],
                                                    bias_sb[:, 2 + oc:3 + oc])
                    # GRU gates, per output chunk
                    for oc in range(2):
                        rp = psgp.tile([128, NT], F32, tag="gate", name="rp")
                        zp = psgp.tile([128, NT], F32, tag="gate", name="zp")
                        ip = psgp.tile([128, NT], F32, tag="gate", name="ip")
                        hp = psgp.tile([128, NT], F32, tag="gate", name="hp")
                        for gate, pst in ((0, rp), (1, zp)):
                            o0 = gate * H + oc * 128
                            for hc in range(2):
                                nc.tensor.matmul(pst[:, 0:w],
                                                 wih_sb[:, hc, o0:o0 + 128],
                                                 xf_in[:, hc, bs],
                                                 start=(hc == 0), stop=False)
                            for hc in range(2):
                                nc.tensor.matmul(pst[:, 0:w],
                                                 whh_sb[:, hc, o0:o0 + 128],
                                                 sT[:, hc, ls],
                                                 start=False, stop=(hc == 1))
                        o0 = 2 * H + oc * 128
                        for hc in range(2):
                            nc.tensor.matmul(ip[:, 0:w], wih_sb[:, hc, o0:o0 + 128],
                                             xf_in[:, hc, bs],
                                             start=(hc == 0), stop=(hc == 1))
                        for hc in range(2):
                            nc.tensor.matmul(hp[:, 0:w], whh_sb[:, hc, o0:o0 + 128],
                                             sT[:, hc, ls],
                                             start=(hc == 0), stop=(hc == 1))
                        r = tmpp.tile([128, NT], F32, tag="r", name="r")
                        nc.scalar.activation(r[:, 0:w], rp[:, 0:w], SIG,
                                             bias=bias_sb[:, 4 + oc:5 + oc])
                        z = tmpp.tile([128, NT], F32, tag="z", name="z")
                        nc.scalar.activation(z[:, 0:w], zp[:, 0:w], SIG,
                                             bias=bias_sb[:, 6 + oc:7 + oc])
                        # n = tanh((i_n + b_ih_n) + r * (h_n + b_hh_n))
                        hnr = tmpp.tile([128, NT], F32, tag="hnr", name="hnr")
                        nc.vector.scalar_tensor_tensor(
                            hnr[:, 0:w], hp[:, 0:w], bias_sb[:, 10 + oc:11 + oc],
                            r[:, 0:w], op0=ADD, op1=MULT)
                        npre = tmpp.tile([128, NT], F32, tag="r", name="npre")
                        nc.vector.scalar_tensor_tensor(
                            npre[:, 0:w], ip[:, 0:w], bias_sb[:, 8 + oc:9 + oc],
                            hnr[:, 0:w], op0=ADD, op1=ADD)
                        nt_ = tmpp.tile([128, NT], F32, tag="nt", name="nt")
                        nc.scalar.activation(nt_[:, 0:w], npre[:, 0:w], TANH)
                        # x_new = n + z * (summary - n)
                        d = tmpp.tile([128, NT], F32, tag="d", name="d")
                        nc.vector.tensor_sub(d[:, 0:w], sT[:, oc, ls], nt_[:, 0:w])
                        dz = tmpp.tile([128, NT], F32, tag="d", name="dz")
                        nc.vector.tensor_mul(dz[:, 0:w], d[:, 0:w], z[:, 0:w])
                        if last:
                            yt = youtp.tile([128, NT], F32, tag="y", name="yt")
                            nc.vector.tensor_add(yt[:, 0:w], dz[:, 0:w], nt_[:, 0:w])
                            nc.sync.dma_start(y.ap()[:, oc, bs], yt[:, 0:w])
                        else:
                            nc.vector.tensor_add(xf_out[:, oc, bs], dz[:, 0:w],
                                                 nt_[:, 0:w])

                for t in range(T):
                    ts0 = t * NT
                    # neighbor rows: four node-major 2048-idx gathers per
                    # tile (idx i -> neighbor i//128 of node i%128), so each
                    # node's K=16 rows land on ONE partition across the free
                    # chunk axis: reduce = strided DVE free-dim reduce, no
                    # transpose DMAs.  Then scale by 1/cnt (per-partition
                    # scalar) and PE-transpose the 16x-smaller result into
                    # [feat, node] matmul layout.
                    nmean_t = workp.tile([128, 2, NT], BF16, tag="nmean",
                                         name="nmean")
                    for qf in range(4):
                        q0 = qf * 128
                        blk = t * 4 + qf
                        nrm = gathp.tile([128, K, H], BF16, tag="nrm",
                                         name="nrm")
                        nc.gpsimd.dma_gather(
                            nrm[:], xtab[:],
                            nbr_sb[:, t, q0:q0 + 128],
                            NQ, NQ, H, transpose=False, single_packet=False,
                            queue_num=next_q())
                        # contiguous tree-reduce over the chunk axis (a
                        # single strided reduce runs ~0.6 elem/cyc)
                        s1 = workp.tile([128, 8, H], BF16, tag="ts1",
                                        name="ts1")
                        nc.vector.tensor_add(s1[:], nrm[:, 0:8, :],
                                             nrm[:, 8:16, :])
                        s2 = workp.tile([128, 4, H], F32, tag="ts2",
                                        name="ts2")
                        nc.vector.tensor_add(s2[:], s1[:, 0:4, :],
                                             s1[:, 4:8, :])
                        s3 = workp.tile([128, 2, H], F32, tag="ts3",
                                        name="ts3")
                        nc.vector.tensor_add(s3[:], s2[:, 0:2, :],
                                             s2[:, 2:4, :])
                        nsum_rm = workp.tile([128, H], BF16, tag="nsr",
                                             name="nsr")
                        nc.vector.tensor_add(nsum_rm[:], s3[:, 0, :],
                                             s3[:, 1, :])
                        for hc in range(2):
                            nps = psp.tile([128, 128], F32, tag="sum",
                                           name="nps")
                            # plain matmul with rhs=diag(inv): transposes the
                            # node-major block AND scales each node column
                            # (is_transpose mode ignores rhs values)
                            nc.tensor.matmul(
                                nps[:], nsum_rm[:, hc * 128:(hc + 1) * 128],
                                dinv_sb[:, blk, :], start=True, stop=True)
                            nc.scalar.activation(
                                nmean_t[:, hc, q0:q0 + 128], nps[:], COPY)
                        if t == T - 1 and qf == 1:
                            gru_block(ts0, NT // 2, nmean_t)
                    if t == T - 1:
                        gru_block(ts0 + NT // 2, NT // 2, nmean_t)
                    else:
                        gru_block(ts0, NT, nmean_t)
                    if not last:
                        write_table_tile(xf_out, t)
                if not last:
                    allgather(xtabs[layer + 1])
                cur = 1 - cur

    nc.compile()
    return nc


def _get_nc(depth: int):
    if depth not in _CACHE:
        _CACHE[depth] = _build(depth)
    return _CACHE[depth]


def _idx_layout(lin):
    """linear int16 idx list (len % 16 == 0) -> [128, len//16] wrapped in 16
    partitions, replicated across the 8 gpsimd core groups."""
    v = lin.reshape(-1, 16).T.astype(np.int16)        # [16, len//16]
    return np.tile(v, (8, 1))                         # [128, len//16]


def _chunk2(w):
    """[256, M] -> [128, 2, M] with [p, c, m] = w[c*128+p, m]."""
    M = w.shape[1]
    return np.ascontiguousarray(w.reshape(2, 128, M).transpose(1, 0, 2))


def prepare_inputs(inputs):
    """host-side preprocessing: returns in_maps for the 8 cores."""
    adj = np.asarray(inputs["nodeAdjacencySpecTensor"]).astype(np.int64)
    names = np.asarray(inputs["nodeNamesEncoded"], dtype=np.float32)
    attrs = np.asarray(inputs["nodeAttributesEncoded"], dtype=np.float32)

    parent = adj[:, 0]
    parent = np.clip(np.where(parent < 0, parent + N, parent), 0, N - 1)
    nbr = adj[:, 1:]
    mask = nbr >= 0
    cnt = np.maximum(mask.sum(1), 1).astype(np.float32)
    safe_n = np.where(mask, np.clip(nbr, 0, N - 1), 0).astype(np.int64)
    # node id -> table row in the [8 x SHARD] allgathered layout
    def _row(n):
        return SHARD * (n // NC_REAL) + (n % NC_REAL)
    safe = np.where(mask, _row(safe_n), ZROW).astype(np.int64)
    parent = _row(parent)
    inv = (1.0 / cnt).astype(np.float32)

    feat = np.concatenate([names, attrs], axis=1)      # [N, 256] f32

    W_in = np.asarray(inputs["W_in"], np.float32)
    W_p = np.asarray(inputs["W_parent"], np.float32)
    W_n = np.asarray(inputs["W_neighbor"], np.float32)
    W_ih = np.asarray(inputs["W_ih"], np.float32)
    W_hh = np.asarray(inputs["W_hh"], np.float32)
    b_in = np.asarray(inputs["b_in"], np.float32)
    b_p = np.asarray(inputs["b_parent"], np.float32)
    b_n = np.asarray(inputs["b_neighbor"], np.float32)
    b_ih = np.asarray(inputs["b_ih"], np.float32)
    b_hh = np.asarray(inputs["b_hh"], np.float32)

    w_in_a = _chunk2(W_in.T).astype(BF)                 # [128, 2, 256]
    w_ih_a = _chunk2(W_ih.T).astype(BF)                 # [128, 2, 768]
    w_hh_a = _chunk2(W_hh.T).astype(BF)
    w_p_a = _chunk2(W_p.T).astype(BF)
    w_n_a = _chunk2(W_n.T).astype(BF)

    bias = np.zeros((128, 12), np.float32)
    for col, vec in ((0, b_in), (2, b_p + b_n), (4, (b_ih + b_hh)[0:H]),
                     (6, (b_ih + b_hh)[H:2 * H]), (8, b_ih[2 * H:3 * H]),
                     (10, b_hh[2 * H:3 * H])):
        bias[:, col] = vec[0:128]
        bias[:, col + 1] = vec[128:256]

    ident_b = np.eye(128, dtype=BF)

    shared = dict(w_in=w_in_a, w_ih=w_ih_a, w_hh=w_hh_a, w_p=w_p_a, w_n=w_n_a,
                  biases=bias, ident_b=ident_b)

    in_maps = []
    for c in range(NCORES):
        g0 = c * NC_REAL
        # features, transposed + padded
        f = np.zeros((NCP, DIN), np.float32)
        f[:NC_REAL] = feat[g0:g0 + NC_REAL]
        featT_c = np.ascontiguousarray(
            f.T.reshape(2, 128, NCP).transpose(1, 0, 2)).astype(BF)
        # per-block diag(1/cnt): dinv[p, b, j] = (p==j) * iv[128b + j]
        iv = np.ones(NCP, np.float32)
        iv[:NC_REAL] = inv[g0:g0 + NC_REAL]
        inv_c = np.zeros((128, NCP // 128, 128), np.float32)
        for b in range(NCP // 128):
            inv_c[:, b, :] = np.diag(iv[128 * b:128 * (b + 1)])
        inv_c = inv_c.astype(BF)
        # indices
        par = np.full(NCP, ZROW, np.int64)
        par[:NC_REAL] = parent[g0:g0 + NC_REAL]
        nbrs = np.full((NCP, K), ZROW, np.int64)
        nbrs[:NC_REAL] = safe[g0:g0 + NC_REAL]
        # node-major gather order: idx i of block b is neighbor i//128 of
        # node 128b + i%128 -> block's linear list = nbrs[block].T.ravel()
        nbr_t = np.zeros((128, T, NT), np.int16)
        for t in range(T):
            blks = [nbrs[t * NT + 128 * q: t * NT + 128 * (q + 1)].T.reshape(-1)
                    for q in range(4)]
            nbr_t[:, t, :] = _idx_layout(np.concatenate(blks))
        par_t = _idx_layout(par)                        # [128, 160]
        in_maps.append(dict(featT=featT_c, invcnt=inv_c, nbr_idx=nbr_t,
                            par_idx=par_t, **shared))
    return in_maps


def run(inputs, trace=False, **kw):
    depth = int(np.asarray(inputs["depth"]))
    nc = _get_nc(depth)
    in_maps = prepare_inputs(inputs)
    res = bass_utils.run_bass_kernel_spmd(nc, in_maps,
                                          core_ids=list(range(NCORES)),
                                          trace=trace, **kw)
    outs = []
    for c in range(NCORES):
        yc = np.asarray(res.results[c]["y"])            # [128, 2, NCP]
        xc = yc.transpose(2, 1, 0).reshape(NCP, H)      # [NCP, 256]
        outs.append(xc[:NC_REAL])
    return np.ascontiguousarray(
        np.concatenate(outs, axis=0).astype(np.float32)), res


def kernel(**inputs) -> np.ndarray:
    out, _ = run(inputs, trace=False)
    return out



# revision 36
# speedup vs baseline: 2.3966x; 1.0356x over previous
"""Trainium2 Bass kernel for nn_NodeInfoPropagate (GNN message passing).

Strategy (8 NeuronCores, node-parallel):
  - Shard the 20000 nodes across 8 cores (2500/core, padded to 2560 = 5 tiles
    of 512).  Weights replicated, all matmul operands bf16 (PSUM accumulates
    f32).
  - Activations live on-chip in "transposed" layout [feature-on-partition,
    node-on-free], so every matmul chains with zero transposes.
  - Per layer, the full x table [N, 256] bf16 is materialized in each core's
    HBM via AllGather (Shared scratchpad); parent + neighbor rows are fetched
    with dma_gather(transpose=False).  Gather descriptor generation runs at
    ~8ns/row on ONE Q7 core pair, so gathers are spread over all 4 SWDGE
    queues (4 Q7 pairs -> ~4x descgen).  transpose=True gathers corrupt each
    other when concurrent (shared XBAR spray state), so rows are gathered
    row-major and flipped to [feat, idx] layout with one HWDGE transpose-DMA
    per gather; those are all issued on the Sync queue (cross-engine
    transpose DMAs also corrupt each other).  All other work (reduces, GRU
    element-wise, activations, PSUM copies, table-write transposes) spreads
    over Vector/Scalar/PE so it hides under the gathers.
  - gather commutes with the linear maps: only the x table is gathered;
    summary = x[par] @ Wp.T + mean_k x[nbr_k] @ Wn.T + (b_p + b_n) accumulates
    in one PSUM bank.  Invalid (-1) neighbors point at an all-zero table row.
  - The final output is written directly in transposed layout [128, 2, NCP]
    f32 and unshuffled on the host (no on-chip output transposes).
"""

import os
import sys

# The tile framework's elide-DMA-wait pass coarsens DMA waits to the
# FIFO-max entry of the issuing ring, scheduled against CoreSim timings.
# On hardware (where SWDGE gathers run ~10x slower than modeled) those
# coarsened waits serialize the gather pipeline — keep precise waits.
os.environ["BACC_ELIDE_DMA_OPT_LIMIT"] = "0"

sys.path.insert(0, "/opt/trn_rl_repo")

import numpy as np
import ml_dtypes

import concourse.bass as bass
import concourse.bacc as bacc
import concourse.tile as tile
import concourse.mybir as mybir
from concourse import bass_utils

N = 20000
K = 16
H = 256
DIN = 256
NCORES = 8
NC_REAL = N // NCORES          # 2500 real nodes per core
NT = 512                       # node tile (matmul free dim / PSUM bank)
T = 5                          # tiles per core
NCP = NT * T                   # 2560 padded nodes per core
SHARD = 2528                   # table shard rows per core (28 zero pad rows)
ZROW = NC_REAL                 # all-zero table row (core0 pad) for invalid nbrs
NTAB = SHARD * NCORES          # 20224 table rows
NQ = (NT // 4) * K             # 2048 neighbor idxs per quarter-tile

F32 = mybir.dt.float32
BF16 = mybir.dt.bfloat16
I16 = mybir.dt.int16
BF = ml_dtypes.bfloat16

_CACHE = {}


def _build(depth: int):
    nc = bacc.Bacc("TRN2", target_bir_lowering=False, debug=False,
                   num_devices=NCORES, num_swdge_queues=4)

    featT = nc.dram_tensor("featT", [128, 2, NCP], BF16, kind="ExternalInput")
    # per-node-block diagonal 1/cnt matrices: transposing a node-major
    # [node, feat] block with rhs=diag(inv) scales each node column for free
    invcnt = nc.dram_tensor("invcnt", [128, NCP // 128, 128], BF16,
                            kind="ExternalInput")
    nbr_idx = nc.dram_tensor("nbr_idx", [128, T, NT], I16, kind="ExternalInput")
    par_idx = nc.dram_tensor("par_idx", [128, NCP // 16], I16, kind="ExternalInput")
    w_in = nc.dram_tensor("w_in", [128, 2, H], BF16, kind="ExternalInput")
    w_ih = nc.dram_tensor("w_ih", [128, 2, 3 * H], BF16, kind="ExternalInput")
    w_hh = nc.dram_tensor("w_hh", [128, 2, 3 * H], BF16, kind="ExternalInput")
    w_p = nc.dram_tensor("w_p", [128, 2, H], BF16, kind="ExternalInput")
    w_n = nc.dram_tensor("w_n", [128, 2, H], BF16, kind="ExternalInput")
    # bias columns: 0-1 b_in, 2-3 b_p+b_n, 4-5 b_r, 6-7 b_z, 8-9 b_ih_n,
    # 10-11 b_hh_n  (per 128-feature chunk)
    biases = nc.dram_tensor("biases", [128, 12], F32, kind="ExternalInput")
    ident_b = nc.dram_tensor("ident_b", [128, 128], BF16, kind="ExternalInput")
    y = nc.dram_tensor("y", [128, 2, NCP], F32, kind="ExternalOutput")

    SIG = mybir.ActivationFunctionType.Sigmoid
    TANH = mybir.ActivationFunctionType.Tanh
    COPY = mybir.ActivationFunctionType.Copy
    ADD = mybir.AluOpType.add
    MULT = mybir.AluOpType.mult

    with tile.TileContext(nc) as tc:
        with (
            tc.tile_pool(name="const", bufs=1) as constp,
            tc.tile_pool(name="state", bufs=1) as statep,
            tc.tile_pool(name="dram", bufs=1, space="DRAM") as dramp,
            tc.tile_pool(name="pgat", bufs=2) as pgatp,
            tc.tile_pool(name="prm", bufs=1) as prmp,
            tc.tile_pool(name="gath", bufs=9) as gathp,
            tc.tile_pool(name="work", bufs=2) as workp,
            tc.tile_pool(name="tmp", bufs=2) as tmpp,
            tc.tile_pool(name="yout", bufs=1) as youtp,
            tc.tile_pool(name="ps", bufs=2, space="PSUM") as psp,
            tc.tile_pool(name="psg", bufs=6, space="PSUM") as psgp,
        ):
            # ---- resident constants -------------------------------------
            win_sb = constp.tile([128, 2, H], BF16, name="win_sb")
            nc.sync.dma_start(win_sb[:], w_in.ap())
            wih_sb = constp.tile([128, 2, 3 * H], BF16, name="wih_sb")
            nc.sync.dma_start(wih_sb[:], w_ih.ap())
            whh_sb = constp.tile([128, 2, 3 * H], BF16, name="whh_sb")
            nc.sync.dma_start(whh_sb[:], w_hh.ap())
            wp_sb = constp.tile([128, 2, H], BF16, name="wp_sb")
            nc.sync.dma_start(wp_sb[:], w_p.ap())
            wn_sb = constp.tile([128, 2, H], BF16, name="wn_sb")
            nc.sync.dma_start(wn_sb[:], w_n.ap())
            bias_sb = constp.tile([128, 12], F32, name="bias_sb")
            nc.sync.dma_start(bias_sb[:], biases.ap())
            idb_sb = constp.tile([128, 128], BF16, name="idb_sb")
            nc.sync.dma_start(idb_sb[:], ident_b.ap())
            feat_sb = constp.tile([128, 2, NCP], BF16, name="feat_sb")
            nc.sync.dma_start(feat_sb[:], featT.ap())
            dinv_sb = constp.tile([128, NCP // 128, 128], BF16, name="dinv_sb")
            nc.sync.dma_start(dinv_sb[:], invcnt.ap())
            nbr_sb = constp.tile([128, T, NT], I16, name="nbr_sb")
            nc.sync.dma_start(nbr_sb[:], nbr_idx.ap())
            par_sb = constp.tile([128, NCP // 16], I16, name="par_sb")
            nc.sync.dma_start(par_sb[:], par_idx.ap())

            xF = [statep.tile([128, 2, NCP], BF16, name=f"xF{i}") for i in range(2)]
            qctr = [0]  # SWDGE queue round-robin across all gathers

            def next_q():
                q = qctr[0] % 4
                qctr[0] += 1
                return q

            xloc = dramp.tile([SHARD, H], BF16, name="xloc")
            ntabs = max(depth, 1)
            xtabs = [dramp.tile([NTAB, H], BF16, name=f"xtab{i}",
                                addr_space="Shared") for i in range(ntabs)]

            # zero rows (shard pad; serve as invalid-neighbor targets)
            zero_sb = constp.tile([128, H], BF16, name="zero_sb")
            nc.vector.memset(zero_sb[:], 0.0)

            def write_table_tile(xf, t):
                """transpose tile t of xf (bf16) to row-major, one batched DMA
                to xloc (plus a partial-block DMA on the last tile)."""
                ts0 = t * NT
                nb = NT // 128
                rm = workp.tile([128, nb, H], BF16, tag="rm", name="rm")
                nfull = min(NT, NC_REAL - ts0) // 128      # full 128-row blocks
                nblk = nb if ts0 + NT <= NC_REAL else nfull + 1
                for b in range(nblk):
                    for c in range(2):
                        pst = psp.tile([128, 128], BF16, tag="sum", name="pst")
                        nc.tensor.transpose(
                            pst[:], xf[:, c, ts0 + b * 128:ts0 + (b + 1) * 128],
                            idb_sb[:])
                        nc.scalar.activation(rm[:, b, c * 128:(c + 1) * 128],
                                             pst[:], COPY)
                if nfull > 0:
                    nc.sync.dma_start(
                        xloc[ts0:ts0 + nfull * 128, :].rearrange(
                            "(b p) f -> p b f", p=128),
                        rm[:, 0:nfull, :])
                rem = min(NT, NC_REAL - ts0) - nfull * 128  # partial tail rows
                if rem > 0:
                    r0 = ts0 + nfull * 128
                    nc.sync.dma_start(xloc[r0:r0 + rem, :],
                                      rm[0:rem, nfull, :])

            def zero_pad_rows():
                nc.sync.dma_start(xloc[NC_REAL:SHARD, :],
                                  zero_sb[0:SHARD - NC_REAL, :])

            def allgather(xtab, r0=0, r1=SHARD):
                # chunked: gather shard rows [r0, r1) from every core into
                # the strided row-bands of the table
                xt_view = xtab[0:NTAB, :].rearrange("(c r) f -> c r f",
                                                    r=SHARD)
                nc.gpsimd.collective_compute(
                    "AllGather", mybir.AluOpType.bypass,
                    replica_groups=[list(range(NCORES))],
                    ins=[xloc[r0:r1, :].opt()],
                    outs=[xt_view[:, r0:r1, :].opt()],
                )

            # ---- layer 0: x0 = W_in @ feat + b_in ------------------------
            for t in range(T):
                ts = slice(t * NT, (t + 1) * NT)
                for oc in range(2):
                    ps = psp.tile([128, NT], F32, tag="sum", name="ps0")
                    for dc in range(2):
                        nc.tensor.matmul(ps[:], win_sb[:, dc, oc * 128:(oc + 1) * 128],
                                         feat_sb[:, dc, ts],
                                         start=(dc == 0), stop=(dc == 1))
                    if depth == 0:
                        yt = youtp.tile([128, NT], F32, tag="y", name="yt0")
                        nc.vector.tensor_scalar_add(yt[:], ps[:],
                                                    bias_sb[:, oc:oc + 1])
                        nc.sync.dma_start(y.ap()[:, oc, ts], yt[:])
                    else:
                        nc.vector.tensor_scalar_add(xF[0][:, oc, ts], ps[:],
                                                    bias_sb[:, oc:oc + 1])
                if depth > 0:
                    write_table_tile(xF[0], t)
            if depth > 0:
                zero_pad_rows()
                allgather(xtabs[0])

            # ---- GRU layers ---------------------------------------------
            cur = 0
            for layer in range(depth):
                last = layer == depth - 1
                xf_in, xf_out = xF[cur], xF[1 - cur]
                xtab = xtabs[layer]
                # one merged parent gather for the whole layer; row-major
                # node-major blocks [128 nodes, 256 feat], PE-transposed
                # per block into [feat, node] matmul layout
                prm = prmp.tile([128, NCP // 128, H], BF16, tag="prm", name="prm")
                nc.gpsimd.dma_gather(prm[:], xtab[:], par_sb[:],
                                     NCP, NCP, H, transpose=False,
                                     single_packet=False, queue_num=next_q())
                pgat = pgatp.tile([128, 2, NCP], BF16, tag="pgat", name="pgat")
                for blk in range(NCP // 128):
                    for hc in range(2):
                        pps = psp.tile([128, 128], BF16, tag="sum", name="pps")
                        nc.tensor.transpose(
                            pps[:], prm[:, blk, hc * 128:(hc + 1) * 128],
                            idb_sb[:])
                        nc.scalar.activation(
                            pgat[:, hc, blk * 128:(blk + 1) * 128], pps[:],
                            COPY)
                def gru_block(c0, w, nmean):
                    """summary + GRU for columns [c0, c0+w); nmean covers the
                    owning tile, sliced locally."""
                    bs = slice(c0, c0 + w)
                    t0 = c0 - (c0 % NT)
                    ls = slice(c0 - t0, c0 - t0 + w)
                    # summary = pgat @ Wp.T + nmean @ Wn.T + (b_p + b_n)
                    sT = workp.tile([128, 2, NT], BF16, tag="sT", name="sT")
                    for oc in range(2):
                        ps = psp.tile([128, NT], F32, tag="sum", name="psS")
                        for hc in range(2):
                            nc.tensor.matmul(ps[:, 0:w],
                                             wp_sb[:, hc, oc * 128:(oc + 1) * 128],
                                             pgat[:, hc, bs],
                                             start=(hc == 0), stop=False)
                        for hc in range(2):
                            nc.tensor.matmul(ps[:, 0:w],
                                             wn_sb[:, hc, oc * 128:(oc + 1) * 128],
                                             nmean[:, hc, ls],
                                             start=False, stop=(hc == 1))
                        nc.vector.tensor_scalar_add(sT[:, oc, ls], ps[:, 0:w],
                                                    bias_sb[:, 2 + oc:3 + oc])
                    # GRU gates, per output chunk
                    for oc in range(2):
                        rp = psgp.tile([128, NT], F32, tag="gate", name="rp")
                        zp = psgp.tile([128, NT], F32, tag="gate", name="zp")
                        ip = psgp.tile([128, NT], F32, tag="gate", name="ip")
                        hp = psgp.tile([128, NT], F32, tag="gate", name="hp")
                        for gate, pst in ((0, rp), (1, zp)):
                            o0 = gate * H + oc * 128
                            for hc in range(2):
                                nc.tensor.matmul(pst[:, 0:w],
                                                 wih_sb[:, hc, o0:o0 + 128],
                                                 xf_in[:, hc, bs],
                                                 start=(hc == 0), stop=False)
                            for hc in range(2):
                                nc.tensor.matmul(pst[:, 0:w],
                                                 whh_sb[:, hc, o0:o0 + 128],
                                                 sT[:, hc, ls],
                                                 start=False, stop=(hc == 1))
                        o0 = 2 * H + oc * 128
                        for hc in range(2):
                            nc.tensor.matmul(ip[:, 0:w], wih_sb[:, hc, o0:o0 + 128],
                                             xf_in[:, hc, bs],
                                             start=(hc == 0), stop=(hc == 1))
                        for hc in range(2):
                            nc.tensor.matmul(hp[:, 0:w], whh_sb[:, hc, o0:o0 + 128],
                                             sT[:, hc, ls],
                                             start=(hc == 0), stop=(hc == 1))
                        r = tmpp.tile([128, NT], F32, tag="r", name="r")
                        nc.scalar.activation(r[:, 0:w], rp[:, 0:w], SIG,
                                             bias=bias_sb[:, 4 + oc:5 + oc])
                        z = tmpp.tile([128, NT], F32, tag="z", name="z")
                        nc.scalar.activation(z[:, 0:w], zp[:, 0:w], SIG,
                                             bias=bias_sb[:, 6 + oc:7 + oc])
                        # n = tanh((i_n + b_ih_n) + r * (h_n + b_hh_n))
                        hnr = tmpp.tile([128, NT], F32, tag="hnr", name="hnr")
                        nc.vector.scalar_tensor_tensor(
                            hnr[:, 0:w], hp[:, 0:w], bias_sb[:, 10 + oc:11 + oc],
                            r[:, 0:w], op0=ADD, op1=MULT)
                        npre = tmpp.tile([128, NT], F32, tag="r", name="npre")
                        nc.vector.scalar_tensor_tensor(
                            npre[:, 0:w], ip[:, 0:w], bias_sb[:, 8 + oc:9 + oc],
                            hnr[:, 0:w], op0=ADD, op1=ADD)
                        nt_ = tmpp.tile([128, NT], F32, tag="nt", name="nt")
                        nc.scalar.activation(nt_[:, 0:w], npre[:, 0:w], TANH)
                        # x_new = n + z * (summary - n)
                        d = tmpp.tile([128, NT], F32, tag="d", name="d")
                        nc.vector.tensor_sub(d[:, 0:w], sT[:, oc, ls], nt_[:, 0:w])
                        dz = tmpp.tile([128, NT], F32, tag="d", name="dz")
                        nc.vector.tensor_mul(dz[:, 0:w], d[:, 0:w], z[:, 0:w])
                        if last:
                            yt = youtp.tile([128, NT], F32, tag="y", name="yt")
                            nc.vector.tensor_add(yt[:, 0:w], dz[:, 0:w], nt_[:, 0:w])
                            nc.sync.dma_start(y.ap()[:, oc, bs], yt[:, 0:w])
                        else:
                            nc.vector.tensor_add(xf_out[:, oc, bs], dz[:, 0:w],
                                                 nt_[:, 0:w])

                for t in range(T):
                    ts0 = t * NT
                    # neighbor rows: four node-major 2048-idx gathers per
                    # tile (idx i -> neighbor i//128 of node i%128), so each
                    # node's K=16 rows land on ONE partition across the free
                    # chunk axis: reduce = strided DVE free-dim reduce, no
                    # transpose DMAs.  Then scale by 1/cnt (per-partition
                    # scalar) and PE-transpose the 16x-smaller result into
                    # [feat, node] matmul layout.
                    nmean_t = workp.tile([128, 2, NT], BF16, tag="nmean",
                                         name="nmean")
                    for qf in range(4):
                        q0 = qf * 128
                        blk = t * 4 + qf
                        nrm = gathp.tile([128, K, H], BF16, tag="nrm",
                                         name="nrm")
                        nc.gpsimd.dma_gather(
                            nrm[:], xtab[:],
                            nbr_sb[:, t, q0:q0 + 128],
                            NQ, NQ, H, transpose=False, single_packet=False,
                            queue_num=next_q())
                        # contiguous tree-reduce over the chunk axis (a
                        # single strided reduce runs ~0.6 elem/cyc)
                        s1 = workp.tile([128, 8, H], BF16, tag="ts1",
                                        name="ts1")
                        nc.vector.tensor_add(s1[:], nrm[:, 0:8, :],
                                             nrm[:, 8:16, :])
                        s2 = workp.tile([128, 4, H], F32, tag="ts2",
                                        name="ts2")
                        nc.vector.tensor_add(s2[:], s1[:, 0:4, :],
                                             s1[:, 4:8, :])
                        s3 = workp.tile([128, 2, H], F32, tag="ts3",
                                        name="ts3")
                        nc.vector.tensor_add(s3[:], s2[:, 0:2, :],
                                             s2[:, 2:4, :])
                        nsum_rm = workp.tile([128, H], BF16, tag="nsr",
                                             name="nsr")
                        nc.vector.tensor_add(nsum_rm[:], s3[:, 0, :],
                                             s3[:, 1, :])
                        for hc in range(2):
                            nps = psp.tile([128, 128], F32, tag="sum",
                                           name="nps")
                            # plain matmul with rhs=diag(inv): transposes the
                            # node-major block AND scales each node column
                            # (is_transpose mode ignores rhs values)
                            nc.tensor.matmul(
                                nps[:], nsum_rm[:, hc * 128:(hc + 1) * 128],
                                dinv_sb[:, blk, :], start=True, stop=True)
                            nc.scalar.activation(
                                nmean_t[:, hc, q0:q0 + 128], nps[:], COPY)
                        if t == T - 1 and qf == 1:
                            gru_block(ts0, NT // 2, nmean_t)
                    if t == T - 1:
                        gru_block(ts0 + NT // 2, NT // 2, nmean_t)
                    else:
                        gru_block(ts0, NT, nmean_t)
                    if not last:
                        write_table_tile(xf_out, t)
                if not last:
                    allgather(xtabs[layer + 1])
                cur = 1 - cur

    nc.compile()
    return nc


def _get_nc(depth: int):
    if depth not in _CACHE:
        _CACHE[depth] = _build(depth)
    return _CACHE[depth]


def _idx_layout(lin):
    """linear int16 idx list (len % 16 == 0) -> [128, len//16] wrapped in 16
    partitions, replicated across the 8 gpsimd core groups."""
    v = lin.reshape(-1, 16).T.astype(np.int16)        # [16, len//16]
    return np.tile(v, (8, 1))                         # [128, len//16]


def _chunk2(w):
    """[256, M] -> [128, 2, M] with [p, c, m] = w[c*128+p, m]."""
    M = w.shape[1]
    return np.ascontiguousarray(w.reshape(2, 128, M).transpose(1, 0, 2))


def prepare_inputs(inputs):
    """host-side preprocessing: returns in_maps for the 8 cores."""
    adj = np.asarray(inputs["nodeAdjacencySpecTensor"]).astype(np.int64)
    names = np.asarray(inputs["nodeNamesEncoded"], dtype=np.float32)
    attrs = np.asarray(inputs["nodeAttributesEncoded"], dtype=np.float32)

    parent = adj[:, 0]
    parent = np.clip(np.where(parent < 0, parent + N, parent), 0, N - 1)
    nbr = adj[:, 1:]
    mask = nbr >= 0
    cnt = np.maximum(mask.sum(1), 1).astype(np.float32)
    safe_n = np.where(mask, np.clip(nbr, 0, N - 1), 0).astype(np.int64)
    # node id -> table row in the [8 x SHARD] allgathered layout
    def _row(n):
        return SHARD * (n // NC_REAL) + (n % NC_REAL)
    safe = np.where(mask, _row(safe_n), ZROW).astype(np.int64)
    parent = _row(parent)
    inv = (1.0 / cnt).astype(np.float32)

    feat = np.concatenate([names, attrs], axis=1)      # [N, 256] f32

    W_in = np.asarray(inputs["W_in"], np.float32)
    W_p = np.asarray(inputs["W_parent"], np.float32)
    W_n = np.asarray(inputs["W_neighbor"], np.float32)
    W_ih = np.asarray(inputs["W_ih"], np.float32)
    W_hh = np.asarray(inputs["W_hh"], np.float32)
    b_in = np.asarray(inputs["b_in"], np.float32)
    b_p = np.asarray(inputs["b_parent"], np.float32)
    b_n = np.asarray(inputs["b_neighbor"], np.float32)
    b_ih = np.asarray(inputs["b_ih"], np.float32)
    b_hh = np.asarray(inputs["b_hh"], np.float32)

    w_in_a = _chunk2(W_in.T).astype(BF)                 # [128, 2, 256]
    w_ih_a = _chunk2(W_ih.T).astype(BF)                 # [128, 2, 768]
    w_hh_a = _chunk2(W_hh.T).astype(BF)
    w_p_a = _chunk2(W_p.T).astype(BF)
    w_n_a = _chunk2(W_n.T).astype(BF)

    bias = np.zeros((128, 12), np.float32)
    for col, vec in ((0, b_in), (2, b_p + b_n), (4, (b_ih + b_hh)[0:H]),
                     (6, (b_ih + b_hh)[H:2 * H]), (8, b_ih[2 * H:3 * H]),
                     (10, b_hh[2 * H:3 * H])):
        bias[:, col] = vec[0:128]
        bias[:, col + 1] = vec[128:256]

    ident_b = np.eye(128, dtype=BF)

    shared = dict(w_in=w_in_a, w_ih=w_ih_a, w_hh=w_hh_a, w_p=w_p_a, w_n=w_n_a,
                  biases=bias, ident_b=ident_b)

    in_maps = []
    for c in range(NCORES):
        g0 = c * NC_REAL
        # features, transposed + padded
        f = np.zeros((NCP, DIN), np.float32)
        f[:NC_REAL] = feat[g0:g0 + NC_REAL]
        featT_c = np.ascontiguousarray(
            f.T.reshape(2, 128, NCP).transpose(1, 0, 2)).astype(BF)
        # per-block diag(1/cnt): dinv[p, b, j] = (p==j) * iv[128b + j]
        iv = np.ones(NCP, np.float32)
        iv[:NC_REAL] = inv[g0:g0 + NC_REAL]
        inv_c = np.zeros((128, NCP // 128, 128), np.float32)
        for b in range(NCP // 128):
            inv_c[:, b, :] = np.diag(iv[128 * b:128 * (b + 1)])
        inv_c = inv_c.astype(BF)
        # indices
        par = np.full(NCP, ZROW, np.int64)
        par[:NC_REAL] = parent[g0:g0 + NC_REAL]
        nbrs = np.full((NCP, K), ZROW, np.int64)
        nbrs[:NC_REAL] = safe[g0:g0 + NC_REAL]
        # node-major gather order: idx i of block b is neighbor i//128 of
        # node 128b + i%128 -> block's linear list = nbrs[block].T.ravel()
        nbr_t = np.zeros((128, T, NT), np.int16)
        for t in range(T):
            blks = [nbrs[t * NT + 128 * q: t * NT + 128 * (q + 1)].T.reshape(-1)
                    for q in range(4)]
            nbr_t[:, t, :] = _idx_layout(np.concatenate(blks))
        par_t = _idx_layout(par)                        # [128, 160]
        in_maps.append(dict(featT=featT_c, invcnt=inv_c, nbr_idx=nbr_t,
                            par_idx=par_t, **shared))
    return in_maps


def run(inputs, trace=False, **kw):
    depth = int(np.asarray(inputs["depth"]))
    nc = _get_nc(depth)
    in_maps = prepare_inputs(inputs)
    res = bass_utils.run_bass_kernel_spmd(nc, in_maps,
                                          core_ids=list(range(NCORES)),
                                          trace=trace, **kw)
    outs = []
    for c in range(NCORES):
        yc = np.asarray(res.results[c]["y"])            # [128, 2, NCP]
        xc = yc.transpose(2, 1, 0).reshape(NCP, H)      # [NCP, 256]
        outs.append(xc[:NC_REAL])
    return np.ascontiguousarray(
        np.concatenate(outs, axis=0).astype(np.float32)), res


def kernel(**inputs) -> np.ndarray:
    out, _ = run(inputs, trace=False)
    return out

